# revision 17
# baseline (speedup 1.0000x reference)
"""Trainium2 Bass kernel for nn_MultiHeadAttention_52862457480066.

Reference computation (B=2, N=1024, D=512, H=16, DH=32):
    qkv = x @ att_w.T ; q,k,v per head
    score = q.k/sqrt(DH) - gamma_p*pdist + angle@w_bias.T + gamma_adj*adj
    score = where(mask, -1e9, score) ; prob = softmax_j(score)
    att = prob @ v ; ff = att @ ff_w.T + ff_b ; y = x + ff ; out = LayerNorm(y)*ln_w+ln_b

Sharding over 8 cores: (batch b in 2) x (query-half ih in 2) x (head-half hg in 2).
Each core: 8 heads, 512 query rows (i), all 1024 key rows (j).

End-to-end wall time is dominated by host->device transfer over the axon
tunnel (~200 MB/s marginal, ~80 ms fixed dispatch), so the layout is chosen
to minimize uploaded bytes and host-side reshuffling:
- All large tensors ship as bf16 (halves bytes; rel-err budget is 2e-2).
- pdist and adj only appear as P0 = gamma_adj*adj - gamma_p*pdist when the
  gammas are head-uniform (they are for this module's inputs), so the host
  combines them into ONE bf16 tensor. Non-uniform gammas fall back to an
  exact numpy implementation.
- Bias tensors upload in natural [i,j] layout (contiguous host slices, full
  2KB DMA lines) and are transposed to the [j,i] score layout on-device by
  the PE (identity matmuls), which has large headroom.
- The key axis j is rotated per-core by -i0 (host-side half-swap) so each
  core's query rows sit at columns 0..511 of its rotated x[b].T upload; the
  q projection then reads a fixed column range (SPMD-uniform) and no separate
  q-slice upload is needed. Softmax/AV reduce over j, so any consistent
  permutation of {k, v, biases, mask} along j leaves the output unchanged.
- Scores are computed TRANSPOSED ([j_part, i_free]) so softmax'd probs feed
  the attention*V matmul directly as the moving operand.
- Softmax denominators come from N=1 matmuls (ones moving operand) giving
  rowsums in [i_part, head_free] layout; normalization is deferred to after
  the AV matmul (divides 16*512 values per core instead of 4.2M).
- PSUM accumulators written by interleaved matmul chains are zero-initialized
  by one full-coverage start=True matmul; everything after runs start=False.
- The FF projection is computed per-head-group and pair-ReduceScattered
  (cores 2k<->2k+1) before the residual+LayerNorm epilogue.
- The jitted PJRT executable is built once and cached; per-call work is just
  host slicing/casting, the sharded transfer, execution, and one bf16 fetch.
"""

import math

import numpy as np

import concourse.bass as bass
import concourse.tile as tile
from concourse import bacc, mybir
from concourse.masks import make_identity

B, N, D, H, DH = 2, 1024, 512, 16, 32
NH = H // 2          # heads per core
NI = N // 2          # query rows per core
NJT = N // 128       # key tiles (partition dim j)
NEG_INF = -1e9
LN_EPS = 1e-5
QSCALE = 1.0 / math.sqrt(DH)
F32 = mybir.dt.float32
F32R = mybir.dt.float32r
BF16 = mybir.dt.bfloat16
N_CORES = 8
RS_GROUPS = [[0, 1], [2, 3], [4, 5], [6, 7]]

BF = np.dtype(mybir.dt.np(BF16))  # ml_dtypes.bfloat16


def _r(ap):
    return ap.bitcast(F32R)


# Big inputs are chunked into one ExternalInput per 128-row partition tile:
# the axon relay transfers different jit args in parallel streams, so ~30
# medium args upload ~5x faster than 9 large ones.
CHUNKED = [
    ("xT", 4, N), ("wqkT", 4, 512), ("wvT", 4, 256),
    ("ffwT", 2, D), ("xrows", 2, D),
    ("p0n", 4, N), ("a0n", 4, N), ("a1n", 4, N),
]


def build_program(trivial_ln: bool):
    """Build the SPMD bass program (identical on all 8 cores)."""
    nc = bacc.Bacc("TRN2", target_bir_lowering=False, debug=False, num_devices=N_CORES)

    # ---- DRAM I/O (per-core views, host-sharded; big tensors bf16) ----
    t = {}
    for nm, k, cols in CHUNKED:
        for i in range(k):
            t[f"d_{nm}{i}"] = nc.dram_tensor(
                f"{nm}{i}", [128, cols], BF16, kind="ExternalInput").ap()
    # smalls layout: [0:512) lnw, [512:1024) lnb, [1024:1536) ffb,
    # [1536:2560) maskb (j-rolled), [2560:2560+2*NH) hcoef w0/w1 interleaved
    t["h_smalls"] = nc.dram_tensor("smalls", [6 * 512], F32, kind="ExternalInput")
    t["d_ffpart"] = nc.dram_tensor("ffpart", [NI, D], F32).ap()
    t["d_ffrs"] = nc.dram_tensor("ffrs", [256, D], F32).ap()
    t["d_out0"] = nc.dram_tensor("out0", [128, D], BF16, kind="ExternalOutput").ap()
    t["d_out1"] = nc.dram_tensor("out1", [128, D], BF16, kind="ExternalOutput").ap()

    with tile.TileContext(nc) as tc:
        _emit(nc, tc, t, trivial_ln)
    nc.compile()
    return nc


def _emit(nc, tc, t, trivial_ln):
    AL = mybir.AluOpType
    AF = mybir.ActivationFunctionType
    from contextlib import ExitStack

    ctx = ExitStack()
    with ctx:
        consts = ctx.enter_context(tc.tile_pool(name="consts", bufs=1))
        big = ctx.enter_context(tc.tile_pool(name="big", bufs=1))
        stream = ctx.enter_context(tc.tile_pool(name="stream", bufs=6))
        tiny = ctx.enter_context(tc.tile_pool(name="tiny", bufs=8))
        ppool = ctx.enter_context(tc.tile_pool(name="ppool", bufs=6))
        ps_mm = ctx.enter_context(tc.tile_pool(name="ps_mm", bufs=4, space="PSUM"))
        ps_sc = ps_mm
        ps_av = ctx.enter_context(tc.tile_pool(name="ps_av", bufs=4, space="PSUM"))
        ps_rs = ps_av
        proj_ctx = ExitStack()
        proj = proj_ctx.enter_context(tc.tile_pool(name="proj", bufs=1))

        sm = t["h_smalls"]

        # ---------------- constants / small tiles ----------------
        identity_f = consts.tile([128, 128], F32)  # f32 transposes (recip path)
        make_identity(nc, identity_f[:])
        identity_b = consts.tile([128, 128], BF16)  # bf16 transposes (bias loads)
        nc.vector.tensor_copy(identity_b[:], identity_f[:])
        ind4 = consts.tile([4, 128], F32)  # ind4[k, m] = (m//32 == k)
        nc.gpsimd.memset(ind4[:], 1.0)
        nc.gpsimd.affine_select(
            out=ind4[:], in_=ind4[:], compare_op=AL.is_ge, fill=0.0,
            base=0, pattern=[[1, 128]], channel_multiplier=-32,
        )
        nc.gpsimd.affine_select(
            out=ind4[:], in_=ind4[:], compare_op=AL.is_ge, fill=0.0,
            base=31, pattern=[[-1, 128]], channel_multiplier=32,
        )
        ones_col = consts.tile([128, 1], BF16)
        nc.gpsimd.memset(ones_col[:], 1.0)
        ones_row_f = consts.tile([1, 128], F32)
        nc.gpsimd.memset(ones_row_f[:], 1.0)
        ones_row_b = consts.tile([1, 128], BF16)
        nc.vector.tensor_copy(ones_row_b[:], ones_row_f[:])
        zeros_row_b = consts.tile([1, 512], BF16)
        nc.gpsimd.memset(zeros_row_b[:], 0.0)

        maskb = consts.tile([128, NJT], F32)
        nc.gpsimd.dma_start(
            out=maskb[:],
            in_=bass.AP(tensor=sm, offset=3 * 512, ap=[[1, 128], [128, NJT]]),
        )
        hbc = []  # w0, w1 broadcast [128, NH] (f32: tensor_scalar scalars must be f32)
        for c in range(2):
            bc = consts.tile([128, NH], F32, tag=f"hbc{c}")
            nc.gpsimd.dma_start(
                out=bc[:], in_=bass.AP(tensor=sm, offset=5 * 512 + c, ap=[[0, 128], [2, NH]])
            )
            hbc.append(bc)

        # per-head scaled identities for the angle-feature PSUM adds
        idw = []  # idw[c][hl] = identity * w_bias[head, c]
        for c, wbc in enumerate(hbc):
            row = []
            for hl in range(NH):
                it_ = consts.tile([128, 128], BF16, tag=f"idw{c}_{hl}")
                nc.vector.tensor_scalar(
                    it_[:], identity_b[:], wbc[:, hl : hl + 1], None, AL.mult
                )
                row.append(it_)
            idw.append(row)

        ffb_f = consts.tile([1, D], F32)
        nc.gpsimd.dma_start(
            out=ffb_f[:],
            in_=bass.AP(tensor=sm, offset=2 * 512, ap=[[0, 1], [1, D]]),
        )
        ffb_row = consts.tile([1, D], BF16)
        nc.vector.tensor_copy(ffb_row[:], ffb_f[:])

        lnw_bc = lnb_bc = None
        if not trivial_ln:
            lnw_row = consts.tile([1, D], F32)
            nc.gpsimd.dma_start(
                out=lnw_row[:], in_=bass.AP(tensor=sm, offset=0, ap=[[0, 1], [1, D]])
            )
            lnb_row = consts.tile([1, D], F32)
            nc.gpsimd.dma_start(
                out=lnb_row[:], in_=bass.AP(tensor=sm, offset=512, ap=[[0, 1], [1, D]])
            )
            lnw_bc = consts.tile([128, D], F32)
            lnb_bc = consts.tile([128, D], F32)
            for row, bc in ((lnw_row, lnw_bc), (lnb_row, lnb_bc)):
                ps = ps_mm.tile([128, D], F32, tag="mmps")
                nc.tensor.matmul(ps[:], ones_row_f[0:1, :], row[0:1, :], start=True, stop=True)
                nc.vector.tensor_copy(bc[:], ps[:])

        # ---------------- load big bf16 inputs (one DMA per chunk) ----------------
        wqkT = proj.tile([128, 4, 512], BF16)
        xT = big.tile([128, 4, N], BF16)
        wvT = proj.tile([128, 4, 256], BF16)
        xrows_t = big.tile([128, 2, D], BF16)
        ffwT = big.tile([128, 2, D], BF16)
        for tl, nm, k in ((wqkT, "wqkT", 4), (xT, "xT", 4), (wvT, "wvT", 4),
                          (xrows_t, "xrows", 2), (ffwT, "ffwT", 2)):
            for i in range(k):
                nc.sync.dma_start(out=tl[:, i, :], in_=t[f"d_{nm}{i}"])

        # ---------------- q/k projection (transposed: [feat, n]) ----------------
        # wqkT f-cols: [0:256) = q rows of att_w (local heads), [256:512) = k
        # q reads xT columns 0:NI — the host rotated the key axis so the core's
        # own query rows are the first NI columns.
        qT = big.tile([128, 2, NI], BF16)   # [dh-part(4h), qtile, i]
        kT = big.tile([128, 2, N], BF16)    # [dh-part(4h), ktile, n]
        for ft in range(4):
            is_q = ft < 2
            nch = 1 if is_q else 2
            for nc_i in range(nch):
                ps = ps_mm.tile([128, 512], F32, tag="mmps")
                for dc in range(4):
                    rhs = xT[:, dc, nc_i * 512 : nc_i * 512 + 512]
                    lhsT = wqkT[:, dc, ft * 128 : (ft + 1) * 128]
                    nc.tensor.matmul(ps[:], lhsT, rhs, start=(dc == 0), stop=(dc == 3))
                if is_q:
                    nc.vector.tensor_scalar(
                        qT[:, ft, :], ps[:, 0:NI], QSCALE, None, AL.mult
                    )
                else:
                    nc.vector.tensor_copy(
                        kT[:, ft - 2, nc_i * 512 : nc_i * 512 + 512], ps[:]
                    )

        # ---------------- v projection (natural: [n, feat]) ----------------
        v = big.tile([128, NJT, 256], BF16)  # [j-part, jt, 8h*32]
        for nt in range(NJT):
            ps = ps_mm.tile([128, 256], F32, tag="mmps")
            for dc in range(4):
                lhsT = xT[:, dc, nt * 128 : (nt + 1) * 128]
                nc.tensor.matmul(ps[:], lhsT, wvT[:, dc, :], start=(dc == 0), stop=(dc == 3))
            nc.scalar.copy(v[:, nt, :], ps[:])
        proj_ctx.close()

        # ------- bias features: load natural, transpose to [j, i] on the PE ------
        nat_ctx = ExitStack()
        nat = nat_ctx.enter_context(tc.tile_pool(name="nat", bufs=1))
        p0nat = nat.tile([128, 4, N], BF16)
        a0nat = nat.tile([128, 4, N], BF16)
        a1nat = nat.tile([128, 4, N], BF16)
        for tl, nm in ((p0nat, "p0n"), (a0nat, "a0n"), (a1nat, "a1n")):
            for i in range(4):
                nc.sync.dma_start(out=tl[:, i, :], in_=t[f"d_{nm}{i}"])

        P0 = big.tile([128, NJT, NI], BF16)
        a0 = big.tile([128, NJT, NI], BF16)
        a1 = big.tile([128, NJT, NI], BF16)
        for natt, dst in ((p0nat, P0), (a0nat, a0), (a1nat, a1)):
            for jt in range(NJT):
                ps = ps_mm.tile([128, NI], F32, tag="mmps")
                for it in range(4):
                    nc.tensor.matmul(
                        ps[:, it * 128 : (it + 1) * 128],
                        natt[:, it, jt * 128 : (jt + 1) * 128],
                        identity_b[:],
                        start=True, stop=True, skip_group_check=True,
                    )
                nc.scalar.copy(dst[:, jt, :], ps[:])
        nat_ctx.close()

        # ---------------- attention: 2 waves of 4 heads ----------------
        attn = big.tile([128, 2, NI], BF16)  # normalized att.T  [4h*32dh, wave, i]
        for w in range(2):
            av_ps = ps_av.tile([128, NI], F32, tag="avps")
            rs_ps = ps_rs.tile([128, 16], F32, tag="avps")
            # zero-init accumulator banks (see module docstring)
            nc.tensor.matmul(
                av_ps[:], ones_row_b[0:1, :], zeros_row_b[0:1, 0:NI],
                start=True, stop=False, skip_group_check=True,
            )
            nc.tensor.matmul(
                rs_ps[:], ones_row_b[0:1, :], zeros_row_b[0:1, 0:16],
                start=True, stop=False, skip_group_check=True,
            )
            for jt in range(NJT):
                p_tiles = []
                for hh in range(4):
                    hl = w * 4 + hh
                    sc = ps_sc.tile([128, NI], F32, tag="mmps")
                    lhsT = kT[hh * 32 : (hh + 1) * 32, w, jt * 128 : (jt + 1) * 128]
                    rhs = qT[hh * 32 : (hh + 1) * 32, w, :]
                    nc.tensor.matmul(
                        sc[:], lhsT, rhs, start=True, stop=False,
                        tile_position=(hh * 32, 0),
                    )
                    nc.tensor.matmul(
                        sc[:], idw[0][hl][:], a0[:, jt, :], start=False, stop=False,
                    )
                    nc.tensor.matmul(
                        sc[:], idw[1][hl][:], a1[:, jt, :], start=False, stop=True,
                    )
                    # P0 add on the DVE, fused with the PSUM evacuation the
                    # exp would otherwise do.
                    xs = stream.tile([128, NI], F32, tag="xs")
                    nc.vector.scalar_tensor_tensor(
                        xs[:], P0[:, jt, :], 1.0, sc[:], AL.mult, AL.add
                    )
                    pT = ppool.tile([128, NI], BF16, tag="pT")
                    nc.scalar.activation(
                        pT[:], xs[:], AF.Exp, bias=maskb[:, jt : jt + 1], scale=1.0
                    )
                    p_tiles.append(pT)
                for hh in range(4):
                    pT = p_tiles[hh]
                    vcol = (w * 4 + hh) * 32
                    nc.tensor.matmul(
                        av_ps[hh * 32 : (hh + 1) * 32, :],
                        v[:, jt, vcol : vcol + 32],
                        pT[:],
                        start=False, stop=(jt == NJT - 1 and hh == 3),
                        tile_position=(0, hh * 32),
                        skip_group_check=True,
                    )
                    for ic in range(4):
                        col = ic * 4 + hh
                        nc.tensor.matmul(
                            rs_ps[:, col : col + 1],
                            pT[:, ic * 128 : (ic + 1) * 128],
                            ones_col[:],
                            start=False,
                            stop=(jt == NJT - 1 and hh == 3 and ic == 3),
                            skip_group_check=True,
                        )
            # normalize: attn = av / rowsum
            rs_sb = stream.tile([128, 16], F32, tag="t512")
            nc.vector.tensor_copy(rs_sb[:], rs_ps[:])
            recip = stream.tile([128, 16], F32, tag="t512")
            nc.vector.reciprocal(recip[:], rs_sb[:])
            recipT = stream.tile([4, NI], F32, tag="t512")
            for ic in range(4):
                trp = ps_mm.tile([4, 128], F32, tag="mmps")
                nc.tensor.transpose(trp[:], recip[:, ic * 4 : (ic + 1) * 4], identity_f[:])
                nc.vector.tensor_copy(recipT[:, ic * 128 : (ic + 1) * 128], trp[:])
            rbc_ps = ps_mm.tile([128, NI], F32, tag="mmps")
            nc.tensor.matmul(rbc_ps[:], ind4[:], recipT[:], start=True, stop=True)
            rbc = stream.tile([128, NI], F32, tag="t512")
            nc.vector.tensor_copy(rbc[:], rbc_ps[:])
            nc.vector.scalar_tensor_tensor(
                attn[:, w, :], rbc[:], 1.0, av_ps[:], AL.mult, AL.mult
            )

        # ---------------- FF projection + ff_b ----------------
        for it in range(4):
            ps = ps_mm.tile([128, D], F32, tag="mmps")
            for w in range(2):
                nc.tensor.matmul(
                    ps[:],
                    attn[:, w, it * 128 : (it + 1) * 128],
                    ffwT[:, w, :],
                    start=(w == 0), stop=False,
                )
            nc.tensor.matmul(
                ps[:], ones_row_b[0:1, :], ffb_row[0:1, :], start=False, stop=True
            )
            ff_sb = stream.tile([128, D], F32, tag="t512")
            nc.vector.tensor_copy(ff_sb[:], ps[:])
            nc.sync.dma_start(
                out=t["d_ffpart"][it * 128 : (it + 1) * 128, :], in_=ff_sb[:]
            )

        # ---------------- pair ReduceScatter ----------------
        nc.gpsimd.collective_compute(
            "ReduceScatter",
            mybir.AluOpType.add,
            replica_groups=RS_GROUPS,
            ins=[t["d_ffpart"]],
            outs=[t["d_ffrs"]],
        )

        # ---------------- residual + LayerNorm on own 256 rows ----------------
        for ot in range(2):
            ff_ld = stream.tile([128, D], F32, tag="t512")
            nc.sync.dma_start(out=ff_ld[:], in_=t["d_ffrs"][ot * 128 : (ot + 1) * 128, :])
            x_ld = stream.tile([128, D], F32, tag="t512")
            nc.scalar.copy(x_ld[:], xrows_t[:, ot, :])
            y = stream.tile([128, D], F32, tag="t512")
            ysum = tiny.tile([128, 1], F32, tag="t1")
            nc.vector.scalar_tensor_tensor(
                y[:], x_ld[:], 1.0, ff_ld[:], AL.mult, AL.add, accum_out=ysum[:],
            )
            negmu = tiny.tile([128, 1], F32, tag="t1")
            nc.vector.tensor_scalar(negmu[:], ysum[:], -1.0 / D, None, AL.mult)
            sq = stream.tile([128, D], F32, tag="t512")
            ssq = tiny.tile([128, 1], F32, tag="t1")
            nc.scalar.activation(
                sq[:], y[:], AF.Square, bias=negmu[:], scale=1.0, accum_out=ssq[:]
            )
            veps = tiny.tile([128, 1], F32, tag="t1")
            nc.vector.tensor_scalar(veps[:], ssq[:], 1.0 / D, LN_EPS, AL.mult, AL.add)
            std = tiny.tile([128, 1], F32, tag="t1")
            nc.scalar.activation(std[:], veps[:], AF.Sqrt)
            rstd = tiny.tile([128, 1], F32, tag="t1")
            nc.vector.reciprocal(rstd[:], std[:])
            if trivial_ln:
                o = stream.tile([128, D], BF16, tag="to")
                nc.vector.tensor_scalar(o[:], y[:], negmu[:], rstd[:], AL.add, AL.mult)
            else:
                z = stream.tile([128, D], F32, tag="t512")
                nc.vector.tensor_scalar(z[:], y[:], negmu[:], rstd[:], AL.add, AL.mult)
                zw = stream.tile([128, D], F32, tag="t512")
                nc.vector.scalar_tensor_tensor(zw[:], lnw_bc[:], 1.0, z[:], AL.mult, AL.mult)
                o = stream.tile([128, D], BF16, tag="to")
                nc.vector.scalar_tensor_tensor(o[:], lnb_bc[:], 1.0, zw[:], AL.mult, AL.add)
            nc.sync.dma_start(out=t[f"d_out{ot}"], in_=o[:])


# ---------------------------------------------------------------------------
# Host side: program cache, cached PJRT runner, shard prep
# ---------------------------------------------------------------------------

_PROGRAM_CACHE = {}
_RUNNER_CACHE = {}


def _get_program(trivial_ln):
    key = (bool(trivial_ln),)
    if key not in _PROGRAM_CACHE:
        _PROGRAM_CACHE[key] = build_program(bool(trivial_ln))
    return _PROGRAM_CACHE[key]


def _get_runner(nc):
    """Build (once) a persistent jitted sharded callable for `nc`.

    Mirrors concourse.bass2jax.run_bass_via_pjrt (the axon execution path of
    bass_utils.run_bass_kernel_spmd) but hoists the jax.jit out of the
    per-call path and assembles the global arrays without an extra concat.
    """
    key = id(nc)
    if key in _RUNNER_CACHE:
        return _RUNNER_CACHE[key]

    import jax
    from jax.sharding import Mesh, PartitionSpec
    from jax.experimental.shard_map import shard_map
    from concourse.bass2jax import (_bass_exec_p, install_neuronx_cc_hook,
                                    partition_id_tensor)

    install_neuronx_cc_hook()
    assert nc.dbg_addr is None or not nc.dbg_callbacks

    partition_name = nc.partition_id_tensor.name if nc.partition_id_tensor else None
    in_names, out_names, out_avals = [], [], []
    for alloc in nc.m.functions[0].allocations:
        if not isinstance(alloc, mybir.MemoryLocationSet):
            continue
        name = alloc.memorylocations[0].name
        if alloc.kind == "ExternalInput":
            if name != partition_name:
                in_names.append(name)
        elif alloc.kind == "ExternalOutput":
            out_names.append(name)
            out_avals.append(jax.core.ShapedArray(
                tuple(alloc.tensor_shape), mybir.dt.np(alloc.dtype)))
    n_params = len(in_names)
    n_outs = len(out_avals)
    all_in_names = list(in_names) + out_names
    if partition_name is not None:
        all_in_names.append(partition_name)
    donate = tuple(range(n_params, n_params + n_outs))

    def _body(*args):
        operands = list(args)
        if partition_name is not None:
            operands.append(partition_id_tensor())
        outs = _bass_exec_p.bind(
            *operands, out_avals=tuple(out_avals), in_names=tuple(all_in_names),
            out_names=tuple(out_names), lowering_input_output_aliases=(),
            sim_require_finite=True, sim_require_nnan=True, nc=nc)
        return tuple(outs)

    devices = jax.devices()[:N_CORES]
    mesh = Mesh(np.asarray(devices), ("core",))
    in_specs = (PartitionSpec("core"),) * (n_params + n_outs)
    out_specs = (PartitionSpec("core"),) * n_outs
    sharded = jax.jit(
        shard_map(_body, mesh=mesh, in_specs=in_specs, out_specs=out_specs,
                  check_rep=False),
        donate_argnums=donate, keep_unused=True)

    zero_shapes = [(N_CORES * a.shape[0], *a.shape[1:]) for a in out_avals]
    zero_dtypes = [a.dtype for a in out_avals]
    from concurrent.futures import ThreadPoolExecutor
    fetch_pool = ThreadPoolExecutor(max_workers=len(out_names) or 1)

    def run(globals_by_name):
        concat_in = [globals_by_name[name] for name in in_names]
        concat_zeros = [np.zeros(s, d) for s, d in zip(zero_shapes, zero_dtypes)]
        out_arrs = sharded(*concat_in, *concat_zeros)
        futs = [fetch_pool.submit(np.asarray, o) for o in out_arrs]
        return {
            name: futs[i].result().reshape(N_CORES, *out_avals[i].shape)
            for i, name in enumerate(out_names)
        }

    _RUNNER_CACHE[key] = run
    return run


def _roll_j(arr, i0):
    """Rotate the key axis (last axis) left by i0 (i0 in {0, NI})."""
    if i0 == 0:
        return arr
    return np.concatenate([arr[..., i0:], arr[..., :i0]], axis=-1)


def _shard_globals(x, pdist, angle, adj, mask, gp, ga, w_bias,
                   att_w, ff_w, ff_b, ln_w, ln_b):
    """Build the concatenated [8*128, cols] chunked global input arrays."""
    g = {}
    for nm, k, cols in CHUNKED:
        for i in range(k):
            g[f"{nm}{i}"] = np.empty((N_CORES * 128, cols), BF)
    g["smalls"] = np.zeros((N_CORES * 6 * 512,), np.float32)

    awT = att_w.T.astype(BF)      # [D, 3*H*DH]
    ffwT_b = ff_w.T.astype(BF)    # [2*256, D]
    # per-hg weight slices (2 variants)
    wqk_v, wv_v, ffw_v = [], [], []
    for hg in range(2):
        fsl = slice(hg * 256, hg * 256 + 256)
        wqk_v.append(np.concatenate([awT[:, 0:512][:, fsl], awT[:, 512:1024][:, fsl]],
                                    axis=1))
        wv_v.append(awT[:, 1024:1536][:, fsl])
        ffw_v.append(ffwT_b[hg * 256 : hg * 256 + 256, :])
    xT_b = [x[b].T.astype(BF) for b in range(B)]   # [D, N] per batch
    maskf = [np.where(mask[b, 0, 0, :], np.float32(NEG_INF), np.float32(0.0))
             for b in range(B)]
    simple_g = gp == 1.0 and ga == 1.0

    def rollset(nm, c, src, i0, k):
        """Write src [k*128, cols] into chunk globals, key axis rolled by i0."""
        ncols = src.shape[1]
        for t_ in range(k):
            dst = g[f"{nm}{t_}"][c * 128 : (c + 1) * 128]
            blk = src[t_ * 128 : (t_ + 1) * 128]
            if i0:
                dst[:, : ncols - i0] = blk[:, i0:]
                dst[:, ncols - i0 :] = blk[:, :i0]
            else:
                dst[:] = blk

    for c in range(N_CORES):
        b, ih, hg = c // 4, (c % 4) // 2, c % 2
        i0 = ih * NI
        irows = slice(i0, i0 + NI)
        hsl = slice(hg * NH, (hg + 1) * NH)
        csl = slice(c * 128, (c + 1) * 128)

        rollset("xT", c, xT_b[b], i0, 4)
        for t_ in range(2):
            r = i0 + hg * 256 + t_ * 128
            g[f"xrows{t_}"][csl] = x[b, r : r + 128]
        for t_ in range(4):
            g[f"wqkT{t_}"][csl] = wqk_v[hg][t_ * 128 : (t_ + 1) * 128]
            g[f"wvT{t_}"][csl] = wv_v[hg][t_ * 128 : (t_ + 1) * 128]
        for t_ in range(2):
            g[f"ffwT{t_}"][csl] = ffw_v[hg][t_ * 128 : (t_ + 1) * 128]
        if simple_g:
            p0c = adj[b, irows] - pdist[b, irows]
        else:
            p0c = np.float32(ga) * adj[b, irows] - np.float32(gp) * pdist[b, irows]
        rollset("p0n", c, p0c, i0, 4)
        rollset("a0n", c, angle[b, irows, :, 0], i0, 4)
        rollset("a1n", c, angle[b, irows, :, 1], i0, 4)

        s = g["smalls"][c * 6 * 512 : (c + 1) * 6 * 512]
        s[0:512] = ln_w
        s[512:1024] = ln_b
        # ffb is added per-core before the pair ReduceScatter sums 2 cores
        s[1024:1536] = 0.5 * ff_b
        s[1536:2560] = _roll_j(maskf[b], i0)
        hcoef = np.stack([w_bias[hsl, 0], w_bias[hsl, 1]], axis=1)  # [NH, 2]
        s[2560 : 2560 + 2 * NH] = hcoef.reshape(-1)
    return g


def _reference_numpy(x, pdist, angle, adj, mask, gamma_p, gamma_adj, w_bias,
                     att_w, ff_w, ff_b, ln_w, ln_b):
    """Exact fallback (used only for non-head-uniform gammas)."""
    f8 = np.float64
    x64 = x.astype(f8)
    qkv = x64 @ att_w.astype(f8).T
    wq, wk, wv = np.split(qkv, 3, axis=-1)
    bsz, n = x.shape[0], x.shape[1]
    wq = wq.reshape(bsz, n, H, DH)
    wk = wk.reshape(bsz, n, H, DH)
    wv = wv.reshape(bsz, n, H, DH)
    score = np.einsum('bihd,bjhd->bhij', wq, wk, optimize=True) / np.sqrt(f8(DH))
    score = score - gamma_p.astype(f8)[None, :, None, None] * pdist.astype(f8)[:, None]
    score = score + np.einsum('bijc,hc->bhij', angle.astype(f8), w_bias.astype(f8),
                              optimize=True)
    score = score + gamma_adj.astype(f8)[None, :, None, None] * adj.astype(f8)[:, None]
    score = np.where(mask, NEG_INF, score)
    score -= score.max(-1, keepdims=True)
    p = np.exp(score)
    p /= p.sum(-1, keepdims=True)
    att = np.einsum('bhij,bjhd->bihd', p, wv, optimize=True).reshape(bsz, n, H * DH)
    y = x64 + att @ ff_w.astype(f8).T + ff_b.astype(f8)
    mu = y.mean(-1, keepdims=True)
    var = np.square(y - mu).mean(-1, keepdims=True)
    out = (y - mu) / np.sqrt(var + LN_EPS) * ln_w.astype(f8) + ln_b.astype(f8)
    return out.astype(np.float32)


def kernel(x, pdist, angle, adj, mask, gamma_p, gamma_adj, w_bias,
           att_w, ff_w, ff_b, ln_w, ln_b, **_unused):
    x = np.asarray(x, dtype=np.float32)
    pdist = np.asarray(pdist, dtype=np.float32)
    angle = np.asarray(angle, dtype=np.float32)
    adj = np.asarray(adj, dtype=np.float32)
    mask = np.asarray(mask)
    gamma_p = np.asarray(gamma_p, dtype=np.float32)
    gamma_adj = np.asarray(gamma_adj, dtype=np.float32)
    w_bias = np.asarray(w_bias, dtype=np.float32)
    att_w = np.asarray(att_w, dtype=np.float32)
    ff_w = np.asarray(ff_w, dtype=np.float32)
    ff_b = np.asarray(ff_b, dtype=np.float32)
    ln_w = np.asarray(ln_w, dtype=np.float32)
    ln_b = np.asarray(ln_b, dtype=np.float32)

    uniform = bool(
        np.all(gamma_p == gamma_p.flat[0]) and np.all(gamma_adj == gamma_adj.flat[0])
    )
    if not uniform:
        return _reference_numpy(x, pdist, angle, adj, mask, gamma_p, gamma_adj,
                                w_bias, att_w, ff_w, ff_b, ln_w, ln_b)
    gp = float(gamma_p.flat[0])
    ga = float(gamma_adj.flat[0])

    trivial_ln = bool(np.all(ln_w == 1.0) and np.all(ln_b == 0.0))
    nc = _get_program(trivial_ln)
    run = _get_runner(nc)
    g = _shard_globals(x, pdist, angle, adj, mask, gp, ga, w_bias,
                       att_w, ff_w, ff_b, ln_w, ln_b)
    res = run(g)  # out0/out1: [8, 128, D] bf16

    out = np.empty((B, N, D), dtype=np.float32)
    for c in range(N_CORES):
        b, ih, hg = c // 4, (c % 4) // 2, c % 2
        r0 = ih * NI + hg * 256
        out[b, r0 : r0 + 128, :] = res["out0"][c]
        out[b, r0 + 128 : r0 + 256, :] = res["out1"][c]
    return out


# revision 22
# speedup vs baseline: 2.0242x; 2.0242x over previous
"""Trainium2 Bass kernel for nn_MultiHeadAttention_52862457480066.

Reference computation (B=2, N=1024, D=512, H=16, DH=32):
    qkv = x @ att_w.T ; q,k,v per head
    score = q.k/sqrt(DH) - gamma_p*pdist + angle@w_bias.T + gamma_adj*adj
    score = where(mask, -1e9, score) ; prob = softmax_j(score)
    att = prob @ v ; ff = att @ ff_w.T + ff_b ; y = x + ff ; out = LayerNorm(y)*ln_w+ln_b

Sharding over 8 cores: (batch b in 2) x (query-quarter ih in 4). Each core owns
ALL 16 heads for its 256 query rows, so its FF output rows are complete and no
cross-core reduction of activations is needed.

End-to-end wall time is dominated by host->device transfer over the axon
tunnel (~115-170 MB/s, ~85 ms fixed, ~6 ms per extra jit arg), so the design
minimizes uploaded bytes and arg count:
- All large tensors ship as ONE bf16 blob arg per core (bias slices, x.T
  slices, weight shard); bf16 halves bytes against a 2e-2 rel-err budget.
- pdist and adj only appear as P0 = gamma_adj*adj - gamma_p*pdist when the
  gammas are head-uniform (they are for this module's inputs), so the host
  combines them into ONE tensor. Non-uniform gammas fall back to exact numpy.
- Bias slices are per-core-unique; with the all-heads sharding nothing is
  uploaded twice. x[b].T (needed in full for K/V) is uploaded as per-core
  quarters and AllGathered on-device within each batch's 4-core group; the
  weights are uploaded as 1/8 shards and AllGathered across all 8 cores.
- Bias tensors upload in natural [i,j] layout (contiguous host slices) and
  are transposed to the [j,i] score layout on-device by the PE, which has
  large headroom. x rows for the residual are likewise recovered on-device by
  transposing the uploaded x[b,irows].T slice.
- Scores are computed TRANSPOSED ([j_part, i_free]) so softmax'd probs feed
  the attention*V matmul directly as the moving operand. All score-bias terms
  enter via PE identity matmuls (angle features) or a DVE add fused with the
  PSUM evacuation (P0), so the hot softmax path is one DVE + one ACT pass.
- Softmax denominators come from N=1 matmuls (ones moving operand) giving
  rowsums in [i_part, head_free] layout; normalization is deferred to after
  the AV matmul (divides 16*256 values per core instead of 4.2M).
- PSUM accumulators written by interleaved matmul chains are zero-initialized
  by one full-coverage start=True matmul; everything after runs start=False.
- The jitted PJRT executable is built once and cached; per-call work is host
  slicing/casting, one sharded transfer, execution, and two parallel bf16
  fetches.
"""

import math

import numpy as np

import concourse.bass as bass
import concourse.tile as tile
from concourse import bacc, mybir
from concourse.masks import make_identity

B, N, D, H, DH = 2, 1024, 512, 16, 32
NI = 256             # query rows per core
NJT = N // 128       # key tiles (partition dim j)
NEG_INF = -1e9
LN_EPS = 1e-5
QSCALE = 1.0 / math.sqrt(DH)
F32 = mybir.dt.float32
BF16 = mybir.dt.bfloat16
N_CORES = 8
XG_GROUPS = [[0, 1, 2, 3], [4, 5, 6, 7]]   # x[b].T AllGather within batch
WG_GROUPS = [[0, 1, 2, 3, 4, 5, 6, 7]]     # weight AllGather across all cores

BF = np.dtype(mybir.dt.np(BF16))  # ml_dtypes.bfloat16

# blob row ranges (per core, [1152, 1024] bf16)
R_P0, R_A0, R_A1 = 0, 256, 512       # bias slices [256, N] natural
R_XQ = 768                           # x[b,irows].T packed [512,256]->[128,1024]
R_XP = 896                           # x[b].T rows [ih*128,(ih+1)*128) for AllGather
R_W = 1024                           # weight-pack shard W[c*128:(c+1)*128]
BLOB_ROWS = 1152
# weight pack W [1024, 1024] bf16 (same on all cores before sharding):
#   rows 0:512   att_w.T[:, 0:1024]            (q feats 0:512 | k feats 512:1024)
#   rows 512:768 att_w.T[:, 1024:1536] flat    (v)
#   rows 768:1024 ff_w.T flat
# smalls [6*512] f32: lnw, lnb, ffb, maskb(1024), hcoef w0/w1 interleaved (32)


def build_program(trivial_ln: bool):
    """Build the SPMD bass program (identical on all 8 cores)."""
    nc = bacc.Bacc("TRN2", target_bir_lowering=False, debug=False, num_devices=N_CORES)

    t = {}
    t["h_blob"] = nc.dram_tensor("blob", [BLOB_ROWS, 1024], BF16, kind="ExternalInput")
    t["h_smalls"] = nc.dram_tensor("smalls", [6 * 512], F32, kind="ExternalInput")
    # collectives may not read IO tensors: bounce the blob slices to internal
    t["d_xp"] = nc.dram_tensor("xp", [128, N], BF16).ap()
    t["d_wp"] = nc.dram_tensor("wp", [128, 1024], BF16).ap()
    t["d_xg"] = nc.dram_tensor("xg", [512, N], BF16).ap()
    t["h_wg"] = nc.dram_tensor("wg", [1024, 1024], BF16, addr_space="Shared")
    t["d_out0"] = nc.dram_tensor("out0", [128, D], BF16, kind="ExternalOutput").ap()
    t["d_out1"] = nc.dram_tensor("out1", [128, D], BF16, kind="ExternalOutput").ap()

    with tile.TileContext(nc) as tc:
        _emit(nc, tc, t, trivial_ln)
    nc.compile()
    return nc


def _emit(nc, tc, t, trivial_ln):
    AL = mybir.AluOpType
    AF = mybir.ActivationFunctionType
    from contextlib import ExitStack

    blob = t["h_blob"]
    sm = t["h_smalls"]
    wg = t["h_wg"]

    def blob_ap(row0, shape3):
        """AP over blob rows: [128, k, cols] with partition-major packing."""
        _, k, cols = shape3
        return bass.AP(tensor=blob, offset=row0 * 1024,
                       ap=[[cols, 128], [128 * cols, k], [1, cols]])

    def wg_ap(off, k, cols):
        return bass.AP(tensor=wg, offset=off,
                       ap=[[cols, 128], [128 * cols, k], [1, cols]])

    ctx = ExitStack()
    with ctx:
        consts = ctx.enter_context(tc.tile_pool(name="consts", bufs=1))
        big = ctx.enter_context(tc.tile_pool(name="big", bufs=1))
        stream = ctx.enter_context(tc.tile_pool(name="stream", bufs=6))
        tiny = ctx.enter_context(tc.tile_pool(name="tiny", bufs=8))
        ppool = ctx.enter_context(tc.tile_pool(name="ppool", bufs=6))
        ps_mm = ctx.enter_context(tc.tile_pool(name="ps_mm", bufs=4, space="PSUM"))
        ps_sc = ps_mm
        ps_av = ctx.enter_context(tc.tile_pool(name="ps_av", bufs=4, space="PSUM"))
        ps_rs = ps_av

        # ---------------- collectives: gather x[b].T and the weight pack ------
        nc.sync.dma_start(
            out=t["d_xp"],
            in_=bass.AP(tensor=blob, offset=R_XP * 1024, ap=[[1024, 128], [1, 1024]]),
        )
        nc.sync.dma_start(
            out=t["d_wp"],
            in_=bass.AP(tensor=blob, offset=R_W * 1024, ap=[[1024, 128], [1, 1024]]),
        )
        nc.gpsimd.collective_compute(
            "AllGather", AL.bypass, replica_groups=XG_GROUPS,
            ins=[t["d_xp"]], outs=[t["d_xg"]],
        )
        nc.gpsimd.collective_compute(
            "AllGather", AL.bypass, replica_groups=WG_GROUPS,
            ins=[t["d_wp"]],
            outs=[bass.AP(tensor=wg, offset=0, ap=[[1024, 1024], [1, 1024]])],
        )

        # ---------------- constants / small tiles ----------------
        identity_f = consts.tile([128, 128], F32)  # f32 transposes (recip path)
        make_identity(nc, identity_f[:])
        identity_b = consts.tile([128, 128], BF16)  # bf16 transposes (loads)
        nc.vector.tensor_copy(identity_b[:], identity_f[:])
        ind4 = consts.tile([4, 128], F32)  # ind4[k, m] = (m//32 == k)
        nc.gpsimd.memset(ind4[:], 1.0)
        nc.gpsimd.affine_select(
            out=ind4[:], in_=ind4[:], compare_op=AL.is_ge, fill=0.0,
            base=0, pattern=[[1, 128]], channel_multiplier=-32,
        )
        nc.gpsimd.affine_select(
            out=ind4[:], in_=ind4[:], compare_op=AL.is_ge, fill=0.0,
            base=31, pattern=[[-1, 128]], channel_multiplier=32,
        )
        ones_col = consts.tile([128, 1], BF16)
        nc.gpsimd.memset(ones_col[:], 1.0)
        ones_row_f = consts.tile([1, 128], F32)
        nc.gpsimd.memset(ones_row_f[:], 1.0)
        ones_row_b = consts.tile([1, 128], BF16)
        nc.vector.tensor_copy(ones_row_b[:], ones_row_f[:])
        zeros_row_b = consts.tile([1, 512], BF16)
        nc.gpsimd.memset(zeros_row_b[:], 0.0)

        maskb = consts.tile([128, NJT], F32)
        nc.gpsimd.dma_start(
            out=maskb[:],
            in_=bass.AP(tensor=sm, offset=3 * 512, ap=[[1, 128], [128, NJT]]),
        )
        hbc = []  # w0, w1 broadcast [128, H]
        for c in range(2):
            bc = consts.tile([128, H], F32, tag=f"hbc{c}")
            nc.gpsimd.dma_start(
                out=bc[:], in_=bass.AP(tensor=sm, offset=5 * 512 + c, ap=[[0, 128], [2, H]])
            )
            hbc.append(bc)

        # per-head scaled identities for the angle-feature PSUM adds
        idw = []  # idw[c][hl] = identity * w_bias[head, c]
        for c, wbc in enumerate(hbc):
            row = []
            for hl in range(H):
                it_ = consts.tile([128, 128], BF16, tag=f"idw{c}_{hl}")
                nc.vector.tensor_scalar(
                    it_[:], identity_b[:], wbc[:, hl : hl + 1], None, AL.mult
                )
                row.append(it_)
            idw.append(row)

        ffb_f = consts.tile([1, D], F32)
        nc.gpsimd.dma_start(
            out=ffb_f[:], in_=bass.AP(tensor=sm, offset=2 * 512, ap=[[0, 1], [1, D]])
        )
        ffb_row = consts.tile([1, D], BF16)
        nc.vector.tensor_copy(ffb_row[:], ffb_f[:])

        lnw_bc = lnb_bc = None
        if not trivial_ln:
            lnw_row = consts.tile([1, D], F32)
            nc.gpsimd.dma_start(
                out=lnw_row[:], in_=bass.AP(tensor=sm, offset=0, ap=[[0, 1], [1, D]])
            )
            lnb_row = consts.tile([1, D], F32)
            nc.gpsimd.dma_start(
                out=lnb_row[:], in_=bass.AP(tensor=sm, offset=512, ap=[[0, 1], [1, D]])
            )
            lnw_bc = consts.tile([128, D], F32)
            lnb_bc = consts.tile([128, D], F32)
            for row, bc in ((lnw_row, lnw_bc), (lnb_row, lnb_bc)):
                ps = ps_mm.tile([128, D], F32, tag="mmps")
                nc.tensor.matmul(ps[:], ones_row_f[0:1, :], row[0:1, :], start=True, stop=True)
                nc.vector.tensor_copy(bc[:], ps[:])

        # ---------------- load big bf16 inputs ----------------
        xq_t = big.tile([128, 4, NI], BF16)      # x[b,irows].T  [d-part, dc, i]
        nc.sync.dma_start(out=xq_t[:], in_=blob_ap(R_XQ, [128, 4, NI]))
        xg_t = big.tile([128, 4, N], BF16)       # gathered x[b].T [d-part, dc, n]
        nc.sync.dma_start(out=xg_t[:], in_=t["d_xg"].rearrange("(c p) n -> p c n", p=128))
        wqk_t = big.tile([128, 4, 1024], BF16)   # att_w.T[:, 0:1024]
        nc.sync.dma_start(out=wqk_t[:], in_=wg_ap(0, 4, 1024))
        wv_t = big.tile([128, 4, 512], BF16)     # att_w.T[:, 1024:1536]
        nc.sync.dma_start(out=wv_t[:], in_=wg_ap(512 * 1024, 4, 512))
        ffw_t = big.tile([128, 4, 512], BF16)    # ff_w.T
        nc.sync.dma_start(out=ffw_t[:], in_=wg_ap(768 * 1024, 4, 512))

        # ---------------- q/k projection (transposed: [feat, n]) ----------------
        qT = big.tile([128, 4, NI], BF16)   # [dh-part(4h), ft, i]
        for ft in range(4):
            ps = ps_mm.tile([128, NI], F32, tag="mmps")
            for dc in range(4):
                nc.tensor.matmul(
                    ps[:], wqk_t[:, dc, ft * 128 : (ft + 1) * 128], xq_t[:, dc, :],
                    start=(dc == 0), stop=(dc == 3),
                )
            nc.vector.tensor_scalar(qT[:, ft, :], ps[:], QSCALE, None, AL.mult)
        kT = big.tile([128, 4, N], BF16)    # [dh-part(4h), ft, n]
        for ft in range(4):
            for nc_i in range(2):
                ps = ps_mm.tile([128, 512], F32, tag="mmps")
                for dc in range(4):
                    nc.tensor.matmul(
                        ps[:], wqk_t[:, dc, 512 + ft * 128 : 512 + (ft + 1) * 128],
                        xg_t[:, dc, nc_i * 512 : nc_i * 512 + 512],
                        start=(dc == 0), stop=(dc == 3),
                    )
                nc.vector.tensor_copy(kT[:, ft, nc_i * 512 : nc_i * 512 + 512], ps[:])

        # ---------------- v projection (natural: [n, feat]) ----------------
        v = big.tile([128, NJT, 512], BF16)  # [j-part, jt, 16h*32]
        for nt in range(NJT):
            ps = ps_mm.tile([128, 512], F32, tag="mmps")
            for dc in range(4):
                nc.tensor.matmul(
                    ps[:], xg_t[:, dc, nt * 128 : (nt + 1) * 128], wv_t[:, dc, :],
                    start=(dc == 0), stop=(dc == 3),
                )
            nc.scalar.copy(v[:, nt, :], ps[:])

        # ---------------- x rows for the residual: transpose xq_t -------------
        xrows_t = big.tile([128, 2, D], BF16)  # [i-part, it, d]
        for it in range(2):
            ps = ps_mm.tile([128, D], F32, tag="mmps")
            for dc in range(4):
                nc.tensor.matmul(
                    ps[:, dc * 128 : (dc + 1) * 128],
                    xq_t[:, dc, it * 128 : (it + 1) * 128],
                    identity_b[:],
                    start=True, stop=True, skip_group_check=True,
                )
            nc.scalar.copy(xrows_t[:, it, :], ps[:])

        # ------- bias features: load natural, transpose to [j, i] on the PE ------
        nat_ctx = ExitStack()
        nat = nat_ctx.enter_context(tc.tile_pool(name="nat", bufs=1))
        p0nat = nat.tile([128, 2, N], BF16)
        nc.sync.dma_start(out=p0nat[:], in_=blob_ap(R_P0, [128, 2, N]))
        a0nat = nat.tile([128, 2, N], BF16)
        nc.sync.dma_start(out=a0nat[:], in_=blob_ap(R_A0, [128, 2, N]))
        a1nat = nat.tile([128, 2, N], BF16)
        nc.sync.dma_start(out=a1nat[:], in_=blob_ap(R_A1, [128, 2, N]))

        P0 = big.tile([128, NJT, NI], BF16)
        a0 = big.tile([128, NJT, NI], BF16)
        a1 = big.tile([128, NJT, NI], BF16)
        for natt, dst in ((p0nat, P0), (a0nat, a0), (a1nat, a1)):
            for jt in range(NJT):
                ps = ps_mm.tile([128, NI], F32, tag="mmps")
                for it in range(2):
                    nc.tensor.matmul(
                        ps[:, it * 128 : (it + 1) * 128],
                        natt[:, it, jt * 128 : (jt + 1) * 128],
                        identity_b[:],
                        start=True, stop=True, skip_group_check=True,
                    )
                nc.scalar.copy(dst[:, jt, :], ps[:])
        nat_ctx.close()

        # ---------------- attention: 4 waves of 4 heads ----------------
        attn = big.tile([128, 4, NI], BF16)  # normalized att.T  [4h*32dh, wave, i]
        for w in range(4):
            av_ps = ps_av.tile([128, NI], F32, tag="avps")
            rs_ps = ps_rs.tile([128, 8], F32, tag="avps")
            # zero-init accumulator banks (see module docstring)
            nc.tensor.matmul(
                av_ps[:], ones_row_b[0:1, :], zeros_row_b[0:1, 0:NI],
                start=True, stop=False, skip_group_check=True,
            )
            nc.tensor.matmul(
                rs_ps[:], ones_row_b[0:1, :], zeros_row_b[0:1, 0:8],
                start=True, stop=False, skip_group_check=True,
            )
            for jt in range(NJT):
                p_tiles = []
                for hh in range(4):
                    hl = w * 4 + hh
                    sc = ps_sc.tile([128, NI], F32, tag="mmps")
                    nc.tensor.matmul(
                        sc[:],
                        kT[hh * 32 : (hh + 1) * 32, w, jt * 128 : (jt + 1) * 128],
                        qT[hh * 32 : (hh + 1) * 32, w, :],
                        start=True, stop=False, tile_position=(hh * 32, 0),
                    )
                    nc.tensor.matmul(
                        sc[:], idw[0][hl][:], a0[:, jt, :], start=False, stop=False,
                    )
                    nc.tensor.matmul(
                        sc[:], idw[1][hl][:], a1[:, jt, :], start=False, stop=True,
                    )
                    # P0 add on the DVE, fused with the PSUM evacuation the
                    # exp would otherwise need.
                    xs = stream.tile([128, NI], F32, tag="xs")
                    nc.vector.scalar_tensor_tensor(
                        xs[:], P0[:, jt, :], 1.0, sc[:], AL.mult, AL.add
                    )
                    pT = ppool.tile([128, NI], BF16, tag="pT")
                    nc.scalar.activation(
                        pT[:], xs[:], AF.Exp, bias=maskb[:, jt : jt + 1], scale=1.0
                    )
                    p_tiles.append(pT)
                for hh in range(4):
                    pT = p_tiles[hh]
                    vcol = (w * 4 + hh) * 32
                    nc.tensor.matmul(
                        av_ps[hh * 32 : (hh + 1) * 32, :],
                        v[:, jt, vcol : vcol + 32],
                        pT[:],
                        start=False, stop=(jt == NJT - 1 and hh == 3),
                        tile_position=(0, hh * 32),
                        skip_group_check=True,
                    )
                    for ic in range(2):
                        col = ic * 4 + hh
                        nc.tensor.matmul(
                            rs_ps[:, col : col + 1],
                            pT[:, ic * 128 : (ic + 1) * 128],
                            ones_col[:],
                            start=False,
                            stop=(jt == NJT - 1 and hh == 3 and ic == 1),
                            skip_group_check=True,
                        )
            # normalize: attn = av / rowsum
            rs_sb = stream.tile([128, 8], F32, tag="t512")
            nc.vector.tensor_copy(rs_sb[:], rs_ps[:])
            recip = stream.tile([128, 8], F32, tag="t512")
            nc.vector.reciprocal(recip[:], rs_sb[:])
            recipT = stream.tile([4, NI], F32, tag="t512")
            for ic in range(2):
                trp = ps_mm.tile([4, 128], F32, tag="mmps")
                nc.tensor.transpose(trp[:], recip[:, ic * 4 : (ic + 1) * 4], identity_f[:])
                nc.vector.tensor_copy(recipT[:, ic * 128 : (ic + 1) * 128], trp[:])
            rbc_ps = ps_mm.tile([128, NI], F32, tag="mmps")
            nc.tensor.matmul(rbc_ps[:], ind4[:], recipT[:], start=True, stop=True)
            rbc = stream.tile([128, NI], F32, tag="t512")
            nc.vector.tensor_copy(rbc[:], rbc_ps[:])
            nc.vector.scalar_tensor_tensor(
                attn[:, w, :], rbc[:], 1.0, av_ps[:], AL.mult, AL.mult
            )

        # -------- FF projection + ff_b + residual + LayerNorm, direct out -------
        for it in range(2):
            ps = ps_mm.tile([128, D], F32, tag="mmps")
            for w in range(4):
                nc.tensor.matmul(
                    ps[:],
                    attn[:, w, it * 128 : (it + 1) * 128],
                    ffw_t[:, w, :],
                    start=(w == 0), stop=False,
                )
            nc.tensor.matmul(
                ps[:], ones_row_b[0:1, :], ffb_row[0:1, :], start=False, stop=True
            )
            x_ld = stream.tile([128, D], F32, tag="t512")
            nc.scalar.copy(x_ld[:], xrows_t[:, it, :])
            y = stream.tile([128, D], F32, tag="t512")
            ysum = tiny.tile([128, 1], F32, tag="t1")
            nc.vector.scalar_tensor_tensor(
                y[:], x_ld[:], 1.0, ps[:], AL.mult, AL.add, accum_out=ysum[:],
            )
            negmu = tiny.tile([128, 1], F32, tag="t1")
            nc.vector.tensor_scalar(negmu[:], ysum[:], -1.0 / D, None, AL.mult)
            sq = stream.tile([128, D], F32, tag="t512")
            ssq = tiny.tile([128, 1], F32, tag="t1")
            nc.scalar.activation(
                sq[:], y[:], AF.Square, bias=negmu[:], scale=1.0, accum_out=ssq[:]
            )
            veps = tiny.tile([128, 1], F32, tag="t1")
            nc.vector.tensor_scalar(veps[:], ssq[:], 1.0 / D, LN_EPS, AL.mult, AL.add)
            std = tiny.tile([128, 1], F32, tag="t1")
            nc.scalar.activation(std[:], veps[:], AF.Sqrt)
            rstd = tiny.tile([128, 1], F32, tag="t1")
            nc.vector.reciprocal(rstd[:], std[:])
            if trivial_ln:
                o = stream.tile([128, D], BF16, tag="to")
                nc.vector.tensor_scalar(o[:], y[:], negmu[:], rstd[:], AL.add, AL.mult)
            else:
                z = stream.tile([128, D], F32, tag="t512")
                nc.vector.tensor_scalar(z[:], y[:], negmu[:], rstd[:], AL.add, AL.mult)
                zw = stream.tile([128, D], F32, tag="t512")
                nc.vector.scalar_tensor_tensor(zw[:], lnw_bc[:], 1.0, z[:], AL.mult, AL.mult)
                o = stream.tile([128, D], BF16, tag="to")
                nc.vector.scalar_tensor_tensor(o[:], lnb_bc[:], 1.0, zw[:], AL.mult, AL.add)
            nc.sync.dma_start(out=t[f"d_out{it}"], in_=o[:])


# ---------------------------------------------------------------------------
# Host side: program cache, cached PJRT runner, shard prep
# ---------------------------------------------------------------------------

_PROGRAM_CACHE = {}
_RUNNER_CACHE = {}


def _get_program(trivial_ln):
    key = (bool(trivial_ln),)
    if key not in _PROGRAM_CACHE:
        _PROGRAM_CACHE[key] = build_program(bool(trivial_ln))
    return _PROGRAM_CACHE[key]


def _get_runner(nc):
    """Build (once) a persistent jitted sharded callable for `nc`.

    Mirrors concourse.bass2jax.run_bass_via_pjrt (the axon execution path of
    bass_utils.run_bass_kernel_spmd) but hoists the jax.jit out of the
    per-call path and assembles the global arrays without an extra concat.
    """
    key = id(nc)
    if key in _RUNNER_CACHE:
        return _RUNNER_CACHE[key]

    import jax
    from jax.sharding import Mesh, PartitionSpec
    from jax.experimental.shard_map import shard_map
    from concourse.bass2jax import (_bass_exec_p, install_neuronx_cc_hook,
                                    partition_id_tensor)

    install_neuronx_cc_hook()
    assert nc.dbg_addr is None or not nc.dbg_callbacks

    partition_name = nc.partition_id_tensor.name if nc.partition_id_tensor else None
    in_names, out_names, out_avals = [], [], []
    for alloc in nc.m.functions[0].allocations:
        if not isinstance(alloc, mybir.MemoryLocationSet):
            continue
        name = alloc.memorylocations[0].name
        if alloc.kind == "ExternalInput":
            if name != partition_name:
                in_names.append(name)
        elif alloc.kind == "ExternalOutput":
            out_names.append(name)
            out_avals.append(jax.core.ShapedArray(
                tuple(alloc.tensor_shape), mybir.dt.np(alloc.dtype)))
    n_params = len(in_names)
    n_outs = len(out_avals)
    all_in_names = list(in_names) + out_names
    if partition_name is not None:
        all_in_names.append(partition_name)
    donate = tuple(range(n_params, n_params + n_outs))

    def _body(*args):
        operands = list(args)
        if partition_name is not None:
            operands.append(partition_id_tensor())
        outs = _bass_exec_p.bind(
            *operands, out_avals=tuple(out_avals), in_names=tuple(all_in_names),
            out_names=tuple(out_names), lowering_input_output_aliases=(),
            sim_require_finite=True, sim_require_nnan=True, nc=nc)
        return tuple(outs)

    devices = jax.devices()[:N_CORES]
    mesh = Mesh(np.asarray(devices), ("core",))
    in_specs = (PartitionSpec("core"),) * (n_params + n_outs)
    out_specs = (PartitionSpec("core"),) * n_outs
    sharded = jax.jit(
        shard_map(_body, mesh=mesh, in_specs=in_specs, out_specs=out_specs,
                  check_rep=False),
        donate_argnums=donate, keep_unused=True)

    zero_shapes = [(N_CORES * a.shape[0], *a.shape[1:]) for a in out_avals]
    zero_dtypes = [a.dtype for a in out_avals]
    from concurrent.futures import ThreadPoolExecutor
    fetch_pool = ThreadPoolExecutor(max_workers=max(len(out_names), 1))

    def run(globals_by_name):
        concat_in = [globals_by_name[name] for name in in_names]
        concat_zeros = [np.zeros(s, d) for s, d in zip(zero_shapes, zero_dtypes)]
        out_arrs = sharded(*concat_in, *concat_zeros)
        futs = [fetch_pool.submit(np.asarray, o) for o in out_arrs]
        return {
            name: futs[i].result().reshape(N_CORES, *out_avals[i].shape)
            for i, name in enumerate(out_names)
        }

    _RUNNER_CACHE[key] = run
    return run


def _shard_globals(x, pdist, angle, adj, mask, gp, ga, w_bias,
                   att_w, ff_w, ff_b, ln_w, ln_b):
    """Build the concatenated global input arrays (blob + smalls)."""
    blob = np.empty((N_CORES * BLOB_ROWS, 1024), BF)
    smalls = np.zeros((N_CORES * 6 * 512,), np.float32)

    awT = att_w.T.astype(BF)                       # [D, 1536]
    W = np.empty((1024, 1024), BF)
    W[0:512] = awT[:, 0:1024]
    W[512:768] = awT[:, 1024:1536].reshape(256, 1024)
    W[768:1024] = ff_w.T.astype(BF).reshape(256, 1024)
    xT_b = [x[b].T.astype(BF) for b in range(B)]   # [D, N] per batch
    maskf = [np.where(mask[b, 0, 0, :], np.float32(NEG_INF), np.float32(0.0))
             for b in range(B)]
    simple_g = gp == 1.0 and ga == 1.0

    for c in range(N_CORES):
        b, ih = c // 4, c % 4
        i0 = ih * NI
        irows = slice(i0, i0 + NI)
        bl = blob[c * BLOB_ROWS : (c + 1) * BLOB_ROWS]

        if simple_g:
            p0c = adj[b, irows] - pdist[b, irows]
        else:
            p0c = np.float32(ga) * adj[b, irows] - np.float32(gp) * pdist[b, irows]
        bl[R_P0 : R_P0 + NI] = p0c
        bl[R_A0 : R_A0 + NI] = angle[b, irows, :, 0]
        bl[R_A1 : R_A1 + NI] = angle[b, irows, :, 1]
        bl[R_XQ : R_XQ + 128] = xT_b[b][:, irows].reshape(128, 1024)
        bl[R_XP : R_XP + 128] = xT_b[b][ih * 128 : (ih + 1) * 128]
        bl[R_W : R_W + 128] = W[c * 128 : (c + 1) * 128]

        s = smalls[c * 6 * 512 : (c + 1) * 6 * 512]
        s[0:512] = ln_w
        s[512:1024] = ln_b
        s[1024:1536] = ff_b
        s[1536:2560] = maskf[b]
        s[2560 : 2560 + 2 * H] = w_bias[:, 0:2].reshape(-1)
    return {"blob": blob, "smalls": smalls}


def _reference_numpy(x, pdist, angle, adj, mask, gamma_p, gamma_adj, w_bias,
                     att_w, ff_w, ff_b, ln_w, ln_b):
    """Exact fallback (used only for non-head-uniform gammas)."""
    f8 = np.float64
    x64 = x.astype(f8)
    qkv = x64 @ att_w.astype(f8).T
    wq, wk, wv = np.split(qkv, 3, axis=-1)
    bsz, n = x.shape[0], x.shape[1]
    wq = wq.reshape(bsz, n, H, DH)
    wk = wk.reshape(bsz, n, H, DH)
    wv = wv.reshape(bsz, n, H, DH)
    score = np.einsum('bihd,bjhd->bhij', wq, wk, optimize=True) / np.sqrt(f8(DH))
    score = score - gamma_p.astype(f8)[None, :, None, None] * pdist.astype(f8)[:, None]
    score = score + np.einsum('bijc,hc->bhij', angle.astype(f8), w_bias.astype(f8),
                              optimize=True)
    score = score + gamma_adj.astype(f8)[None, :, None, None] * adj.astype(f8)[:, None]
    score = np.where(mask, NEG_INF, score)
    score -= score.max(-1, keepdims=True)
    p = np.exp(score)
    p /= p.sum(-1, keepdims=True)
    att = np.einsum('bhij,bjhd->bihd', p, wv, optimize=True).reshape(bsz, n, H * DH)
    y = x64 + att @ ff_w.astype(f8).T + ff_b.astype(f8)
    mu = y.mean(-1, keepdims=True)
    var = np.square(y - mu).mean(-1, keepdims=True)
    out = (y - mu) / np.sqrt(var + LN_EPS) * ln_w.astype(f8) + ln_b.astype(f8)
    return out.astype(np.float32)


def kernel(x, pdist, angle, adj, mask, gamma_p, gamma_adj, w_bias,
           att_w, ff_w, ff_b, ln_w, ln_b, **_unused):
    x = np.asarray(x, dtype=np.float32)
    pdist = np.asarray(pdist, dtype=np.float32)
    angle = np.asarray(angle, dtype=np.float32)
    adj = np.asarray(adj, dtype=np.float32)
    mask = np.asarray(mask)
    gamma_p = np.asarray(gamma_p, dtype=np.float32)
    gamma_adj = np.asarray(gamma_adj, dtype=np.float32)
    w_bias = np.asarray(w_bias, dtype=np.float32)
    att_w = np.asarray(att_w, dtype=np.float32)
    ff_w = np.asarray(ff_w, dtype=np.float32)
    ff_b = np.asarray(ff_b, dtype=np.float32)
    ln_w = np.asarray(ln_w, dtype=np.float32)
    ln_b = np.asarray(ln_b, dtype=np.float32)

    uniform = bool(
        np.all(gamma_p == gamma_p.flat[0]) and np.all(gamma_adj == gamma_adj.flat[0])
    )
    if not uniform:
        return _reference_numpy(x, pdist, angle, adj, mask, gamma_p, gamma_adj,
                                w_bias, att_w, ff_w, ff_b, ln_w, ln_b)
    gp = float(gamma_p.flat[0])
    ga = float(gamma_adj.flat[0])

    trivial_ln = bool(np.all(ln_w == 1.0) and np.all(ln_b == 0.0))
    nc = _get_program(trivial_ln)
    run = _get_runner(nc)
    g = _shard_globals(x, pdist, angle, adj, mask, gp, ga, w_bias,
                       att_w, ff_w, ff_b, ln_w, ln_b)
    res = run(g)  # out0/out1: [8, 128, D] bf16

    out = np.empty((B, N, D), dtype=np.float32)
    for c in range(N_CORES):
        b, ih = c // 4, c % 4
        i0 = ih * NI
        out[b, i0 : i0 + 128, :] = res["out0"][c]
        out[b, i0 + 128 : i0 + 256, :] = res["out1"][c]
    return out


# revision 24
# speedup vs baseline: 2.1679x; 1.0710x over previous
"""Trainium2 Bass kernel for nn_MultiHeadAttention_52862457480066.

Reference computation (B=2, N=1024, D=512, H=16, DH=32):
    qkv = x @ att_w.T ; q,k,v per head
    score = q.k/sqrt(DH) - gamma_p*pdist + angle@w_bias.T + gamma_adj*adj
    score = where(mask, -1e9, score) ; prob = softmax_j(score)
    att = prob @ v ; ff = att @ ff_w.T + ff_b ; y = x + ff ; out = LayerNorm(y)*ln_w+ln_b

Sharding over 8 cores: (batch b in 2) x (query-quarter ih in 4). Each core owns
ALL 16 heads for its 256 query rows, so its FF output rows are complete and no
cross-core reduction of activations is needed.

End-to-end wall time is dominated by host->device transfer over the axon
tunnel (~115-170 MB/s, ~85 ms fixed, ~6 ms per extra jit arg), so the design
minimizes uploaded bytes and arg count:
- All large tensors ship as ONE bf16 blob arg per core (bias slices, x.T
  slices, weight shard); bf16 halves bytes against a 2e-2 rel-err budget.
- pdist and adj only appear as P0 = gamma_adj*adj - gamma_p*pdist when the
  gammas are head-uniform (they are for this module's inputs), so the host
  combines them into ONE tensor. Non-uniform gammas fall back to exact numpy.
- Bias slices are per-core-unique; with the all-heads sharding nothing is
  uploaded twice. x[b].T (needed in full for K/V) is uploaded as per-core
  quarters and AllGathered on-device within each batch's 4-core group; the
  weights are uploaded as 1/8 shards and AllGathered across all 8 cores.
- Bias tensors upload in natural [i,j] layout (contiguous host slices) and
  are transposed to the [j,i] score layout on-device by the PE, which has
  large headroom. x rows for the residual are likewise recovered on-device by
  transposing the uploaded x[b,irows].T slice.
- Scores are computed TRANSPOSED ([j_part, i_free]) so softmax'd probs feed
  the attention*V matmul directly as the moving operand. All score-bias terms
  enter via PE identity matmuls (angle features) or a DVE add fused with the
  PSUM evacuation (P0), so the hot softmax path is one DVE + one ACT pass.
- Softmax denominators come from N=1 matmuls (ones moving operand) giving
  rowsums in [i_part, head_free] layout; normalization is deferred to after
  the AV matmul (divides 16*256 values per core instead of 4.2M).
- PSUM accumulators written by interleaved matmul chains are zero-initialized
  by one full-coverage start=True matmul; everything after runs start=False.
- The jitted PJRT executable is built once and cached; per-call work is host
  slicing/casting, one sharded transfer, execution, and two parallel bf16
  fetches.
"""

import math

import numpy as np

import concourse.bass as bass
import concourse.tile as tile
from concourse import bacc, mybir
from concourse.masks import make_identity

B, N, D, H, DH = 2, 1024, 512, 16, 32
NI = 256             # query rows per core
NJT = N // 128       # key tiles (partition dim j)
NEG_INF = -1e9
LN_EPS = 1e-5
QSCALE = 1.0 / math.sqrt(DH)
F32 = mybir.dt.float32
BF16 = mybir.dt.bfloat16
N_CORES = 8
XG_GROUPS = [[0, 1, 2, 3], [4, 5, 6, 7]]   # x[b].T AllGather within batch
WG_GROUPS = [[0, 1, 2, 3, 4, 5, 6, 7]]     # weight AllGather across all cores

BF = np.dtype(mybir.dt.np(BF16))  # ml_dtypes.bfloat16

# blob row ranges (per core, [1152, 1024] bf16)
R_P0, R_A0, R_A1 = 0, 256, 512       # bias slices [256, N] natural
R_XQ = 768                           # x[b,irows].T packed [512,256]->[128,1024]
R_XP = 896                           # x[b].T rows [ih*128,(ih+1)*128) for AllGather
R_W = 1024                           # weight-pack shard W[c*128:(c+1)*128]
BLOB_ROWS = 1152
# weight pack W [1024, 1024] bf16 (same on all cores before sharding):
#   rows 0:512   att_w.T[:, 0:1024]            (q feats 0:512 | k feats 512:1024)
#   rows 512:768 att_w.T[:, 1024:1536] flat    (v)
#   rows 768:1024 ff_w.T flat
# smalls [6*512] f32: lnw, lnb, ffb, maskb(1024), hcoef w0/w1 interleaved (32)


def build_program(trivial_ln: bool):
    """Build the SPMD bass program (identical on all 8 cores)."""
    nc = bacc.Bacc("TRN2", target_bir_lowering=False, debug=False, num_devices=N_CORES)

    t = {}
    t["h_blob"] = nc.dram_tensor("blob", [BLOB_ROWS, 1024], BF16, kind="ExternalInput")
    t["h_smalls"] = nc.dram_tensor("smalls", [6 * 512], F32, kind="ExternalInput")
    # collectives may not read IO tensors: bounce the blob slices to internal
    t["d_xp"] = nc.dram_tensor("xp", [128, N], BF16).ap()
    t["d_wp"] = nc.dram_tensor("wp", [128, 1024], BF16).ap()
    t["d_xg"] = nc.dram_tensor("xg", [512, N], BF16).ap()
    t["h_wg"] = nc.dram_tensor("wg", [1024, 1024], BF16, addr_space="Shared")
    t["d_out0"] = nc.dram_tensor("out0", [128, D], BF16, kind="ExternalOutput").ap()
    t["d_out1"] = nc.dram_tensor("out1", [128, D], BF16, kind="ExternalOutput").ap()

    with tile.TileContext(nc) as tc:
        _emit(nc, tc, t, trivial_ln)
    nc.compile()
    return nc


def _emit(nc, tc, t, trivial_ln):
    AL = mybir.AluOpType
    AF = mybir.ActivationFunctionType
    from contextlib import ExitStack

    blob = t["h_blob"]
    sm = t["h_smalls"]
    wg = t["h_wg"]

    def blob_ap(row0, shape3):
        """AP over blob rows: [128, k, cols] with partition-major packing."""
        _, k, cols = shape3
        return bass.AP(tensor=blob, offset=row0 * 1024,
                       ap=[[cols, 128], [128 * cols, k], [1, cols]])

    def wg_ap(off, k, cols):
        return bass.AP(tensor=wg, offset=off,
                       ap=[[cols, 128], [128 * cols, k], [1, cols]])

    ctx = ExitStack()
    with ctx:
        consts = ctx.enter_context(tc.tile_pool(name="consts", bufs=1))
        big = ctx.enter_context(tc.tile_pool(name="big", bufs=1))
        stream = ctx.enter_context(tc.tile_pool(name="stream", bufs=6))
        tiny = ctx.enter_context(tc.tile_pool(name="tiny", bufs=8))
        ppool = ctx.enter_context(tc.tile_pool(name="ppool", bufs=6))
        ps_mm = ctx.enter_context(tc.tile_pool(name="ps_mm", bufs=4, space="PSUM"))
        ps_sc = ps_mm
        ps_av = ctx.enter_context(tc.tile_pool(name="ps_av", bufs=4, space="PSUM"))
        ps_rs = ps_av

        # ---------------- collectives: gather x[b].T and the weight pack ------
        nc.sync.dma_start(
            out=t["d_xp"],
            in_=bass.AP(tensor=blob, offset=R_XP * 1024, ap=[[1024, 128], [1, 1024]]),
        )
        nc.sync.dma_start(
            out=t["d_wp"],
            in_=bass.AP(tensor=blob, offset=R_W * 1024, ap=[[1024, 128], [1, 1024]]),
        )
        nc.gpsimd.collective_compute(
            "AllGather", AL.bypass, replica_groups=XG_GROUPS,
            ins=[t["d_xp"]], outs=[t["d_xg"]],
        )
        nc.gpsimd.collective_compute(
            "AllGather", AL.bypass, replica_groups=WG_GROUPS,
            ins=[t["d_wp"]],
            outs=[bass.AP(tensor=wg, offset=0, ap=[[1024, 1024], [1, 1024]])],
        )

        # ---------------- constants / small tiles ----------------
        identity_f = consts.tile([128, 128], F32)  # f32 transposes (recip path)
        make_identity(nc, identity_f[:])
        identity_b = consts.tile([128, 128], BF16)  # bf16 transposes (loads)
        nc.vector.tensor_copy(identity_b[:], identity_f[:])
        ind4 = consts.tile([4, 128], F32)  # ind4[k, m] = (m//32 == k)
        nc.gpsimd.memset(ind4[:], 1.0)
        nc.gpsimd.affine_select(
            out=ind4[:], in_=ind4[:], compare_op=AL.is_ge, fill=0.0,
            base=0, pattern=[[1, 128]], channel_multiplier=-32,
        )
        nc.gpsimd.affine_select(
            out=ind4[:], in_=ind4[:], compare_op=AL.is_ge, fill=0.0,
            base=31, pattern=[[-1, 128]], channel_multiplier=32,
        )
        ones_col = consts.tile([128, 1], BF16)
        nc.gpsimd.memset(ones_col[:], 1.0)
        ones_row_f = consts.tile([1, 128], F32)
        nc.gpsimd.memset(ones_row_f[:], 1.0)
        ones_row_b = consts.tile([1, 128], BF16)
        nc.vector.tensor_copy(ones_row_b[:], ones_row_f[:])
        zeros_row_b = consts.tile([1, 512], BF16)
        nc.gpsimd.memset(zeros_row_b[:], 0.0)

        maskb = consts.tile([128, NJT], F32)
        nc.gpsimd.dma_start(
            out=maskb[:],
            in_=bass.AP(tensor=sm, offset=3 * 512, ap=[[1, 128], [128, NJT]]),
        )
        hbc = []  # w0, w1 broadcast [128, H]
        for c in range(2):
            bc = consts.tile([128, H], F32, tag=f"hbc{c}")
            nc.gpsimd.dma_start(
                out=bc[:], in_=bass.AP(tensor=sm, offset=5 * 512 + c, ap=[[0, 128], [2, H]])
            )
            hbc.append(bc)

        # per-head scaled identities for the angle-feature PSUM adds
        idw = []  # idw[c][hl] = identity * w_bias[head, c]
        for c, wbc in enumerate(hbc):
            row = []
            for hl in range(H):
                it_ = consts.tile([128, 128], BF16, tag=f"idw{c}_{hl}")
                nc.vector.tensor_scalar(
                    it_[:], identity_b[:], wbc[:, hl : hl + 1], None, AL.mult
                )
                row.append(it_)
            idw.append(row)

        ffb_f = consts.tile([1, D], F32)
        nc.gpsimd.dma_start(
            out=ffb_f[:], in_=bass.AP(tensor=sm, offset=2 * 512, ap=[[0, 1], [1, D]])
        )
        ffb_row = consts.tile([1, D], BF16)
        nc.vector.tensor_copy(ffb_row[:], ffb_f[:])

        lnw_bc = lnb_bc = None
        if not trivial_ln:
            lnw_row = consts.tile([1, D], F32)
            nc.gpsimd.dma_start(
                out=lnw_row[:], in_=bass.AP(tensor=sm, offset=0, ap=[[0, 1], [1, D]])
            )
            lnb_row = consts.tile([1, D], F32)
            nc.gpsimd.dma_start(
                out=lnb_row[:], in_=bass.AP(tensor=sm, offset=512, ap=[[0, 1], [1, D]])
            )
            lnw_bc = consts.tile([128, D], F32)
            lnb_bc = consts.tile([128, D], F32)
            for row, bc in ((lnw_row, lnw_bc), (lnb_row, lnb_bc)):
                ps = ps_mm.tile([128, D], F32, tag="mmps")
                nc.tensor.matmul(ps[:], ones_row_f[0:1, :], row[0:1, :], start=True, stop=True)
                nc.vector.tensor_copy(bc[:], ps[:])

        # ---------------- load big bf16 inputs ----------------
        xq_t = big.tile([128, 4, NI], BF16)      # x[b,irows].T  [d-part, dc, i]
        nc.sync.dma_start(out=xq_t[:], in_=blob_ap(R_XQ, [128, 4, NI]))
        xg_t = big.tile([128, 4, N], BF16)       # gathered x[b].T [d-part, dc, n]
        nc.sync.dma_start(out=xg_t[:], in_=t["d_xg"].rearrange("(c p) n -> p c n", p=128))
        wqk_t = big.tile([128, 4, 1024], BF16)   # att_w.T[:, 0:1024]
        nc.sync.dma_start(out=wqk_t[:], in_=wg_ap(0, 4, 1024))
        wv_t = big.tile([128, 4, 512], BF16)     # att_w.T[:, 1024:1536]
        nc.sync.dma_start(out=wv_t[:], in_=wg_ap(512 * 1024, 4, 512))
        ffw_t = big.tile([128, 4, 512], BF16)    # ff_w.T
        nc.sync.dma_start(out=ffw_t[:], in_=wg_ap(768 * 1024, 4, 512))

        # ---------------- q/k projection (transposed: [feat, n]) ----------------
        qT = big.tile([128, 4, NI], BF16)   # [dh-part(4h), ft, i]
        for ft in range(4):
            ps = ps_mm.tile([128, NI], F32, tag="mmps")
            for dc in range(4):
                nc.tensor.matmul(
                    ps[:], wqk_t[:, dc, ft * 128 : (ft + 1) * 128], xq_t[:, dc, :],
                    start=(dc == 0), stop=(dc == 3),
                )
            nc.vector.tensor_scalar(qT[:, ft, :], ps[:], QSCALE, None, AL.mult)
        kT = big.tile([128, 4, N], BF16)    # [dh-part(4h), ft, n]
        for ft in range(4):
            for nc_i in range(2):
                ps = ps_mm.tile([128, 512], F32, tag="mmps")
                for dc in range(4):
                    nc.tensor.matmul(
                        ps[:], wqk_t[:, dc, 512 + ft * 128 : 512 + (ft + 1) * 128],
                        xg_t[:, dc, nc_i * 512 : nc_i * 512 + 512],
                        start=(dc == 0), stop=(dc == 3),
                    )
                nc.vector.tensor_copy(kT[:, ft, nc_i * 512 : nc_i * 512 + 512], ps[:])

        # ---------------- v projection (natural: [n, feat]) ----------------
        v = big.tile([128, NJT, 512], BF16)  # [j-part, jt, 16h*32]
        for nt in range(NJT):
            ps = ps_mm.tile([128, 512], F32, tag="mmps")
            for dc in range(4):
                nc.tensor.matmul(
                    ps[:], xg_t[:, dc, nt * 128 : (nt + 1) * 128], wv_t[:, dc, :],
                    start=(dc == 0), stop=(dc == 3),
                )
            nc.scalar.copy(v[:, nt, :], ps[:])

        # ---------------- x rows for the residual: transpose xq_t -------------
        xrows_t = big.tile([128, 2, D], BF16)  # [i-part, it, d]
        for it in range(2):
            ps = ps_mm.tile([128, D], F32, tag="mmps")
            for dc in range(4):
                nc.tensor.matmul(
                    ps[:, dc * 128 : (dc + 1) * 128],
                    xq_t[:, dc, it * 128 : (it + 1) * 128],
                    identity_b[:],
                    start=True, stop=True, skip_group_check=True,
                )
            nc.scalar.copy(xrows_t[:, it, :], ps[:])

        # ------- bias features: load natural, transpose to [j, i] on the PE ------
        nat_ctx = ExitStack()
        nat = nat_ctx.enter_context(tc.tile_pool(name="nat", bufs=1))
        p0nat = nat.tile([128, 2, N], BF16)
        nc.sync.dma_start(out=p0nat[:], in_=blob_ap(R_P0, [128, 2, N]))
        a0nat = nat.tile([128, 2, N], BF16)
        nc.sync.dma_start(out=a0nat[:], in_=blob_ap(R_A0, [128, 2, N]))
        a1nat = nat.tile([128, 2, N], BF16)
        nc.sync.dma_start(out=a1nat[:], in_=blob_ap(R_A1, [128, 2, N]))

        P0 = big.tile([128, NJT, NI], BF16)
        a0 = big.tile([128, NJT, NI], BF16)
        a1 = big.tile([128, NJT, NI], BF16)
        for natt, dst in ((p0nat, P0), (a0nat, a0), (a1nat, a1)):
            for jt in range(NJT):
                ps = ps_mm.tile([128, NI], F32, tag="mmps")
                for it in range(2):
                    nc.tensor.matmul(
                        ps[:, it * 128 : (it + 1) * 128],
                        natt[:, it, jt * 128 : (jt + 1) * 128],
                        identity_b[:],
                        start=True, stop=True, skip_group_check=True,
                    )
                nc.scalar.copy(dst[:, jt, :], ps[:])
        nat_ctx.close()

        # ---------------- attention: 4 waves of 4 heads ----------------
        attn = big.tile([128, 4, NI], BF16)  # normalized att.T  [4h*32dh, wave, i]
        for w in range(4):
            av_ps = ps_av.tile([128, NI], F32, tag="avps")
            rs_ps = ps_rs.tile([128, 8], F32, tag="avps")
            # zero-init accumulator banks (see module docstring)
            nc.tensor.matmul(
                av_ps[:], ones_row_b[0:1, :], zeros_row_b[0:1, 0:NI],
                start=True, stop=False, skip_group_check=True,
            )
            nc.tensor.matmul(
                rs_ps[:], ones_row_b[0:1, :], zeros_row_b[0:1, 0:8],
                start=True, stop=False, skip_group_check=True,
            )
            for jt in range(NJT):
                p_tiles = []
                for hh in range(4):
                    hl = w * 4 + hh
                    sc = ps_sc.tile([128, NI], F32, tag="mmps")
                    nc.tensor.matmul(
                        sc[:],
                        kT[hh * 32 : (hh + 1) * 32, w, jt * 128 : (jt + 1) * 128],
                        qT[hh * 32 : (hh + 1) * 32, w, :],
                        start=True, stop=False, tile_position=(hh * 32, 0),
                    )
                    nc.tensor.matmul(
                        sc[:], idw[0][hl][:], a0[:, jt, :], start=False, stop=False,
                    )
                    nc.tensor.matmul(
                        sc[:], idw[1][hl][:], a1[:, jt, :], start=False, stop=True,
                    )
                    # P0 add on the DVE, fused with the PSUM evacuation the
                    # exp would otherwise need.
                    xs = stream.tile([128, NI], F32, tag="xs")
                    nc.vector.scalar_tensor_tensor(
                        xs[:], P0[:, jt, :], 1.0, sc[:], AL.mult, AL.add
                    )
                    pT = ppool.tile([128, NI], BF16, tag="pT")
                    nc.scalar.activation(
                        pT[:], xs[:], AF.Exp, bias=maskb[:, jt : jt + 1], scale=1.0
                    )
                    p_tiles.append(pT)
                for hh in range(4):
                    pT = p_tiles[hh]
                    vcol = (w * 4 + hh) * 32
                    nc.tensor.matmul(
                        av_ps[hh * 32 : (hh + 1) * 32, :],
                        v[:, jt, vcol : vcol + 32],
                        pT[:],
                        start=False, stop=(jt == NJT - 1 and hh == 3),
                        tile_position=(0, hh * 32),
                        skip_group_check=True,
                    )
                    for ic in range(2):
                        col = ic * 4 + hh
                        nc.tensor.matmul(
                            rs_ps[:, col : col + 1],
                            pT[:, ic * 128 : (ic + 1) * 128],
                            ones_col[:],
                            start=False,
                            stop=(jt == NJT - 1 and hh == 3 and ic == 1),
                            skip_group_check=True,
                        )
            # normalize: attn = av / rowsum
            rs_sb = stream.tile([128, 8], F32, tag="t512")
            nc.vector.tensor_copy(rs_sb[:], rs_ps[:])
            recip = stream.tile([128, 8], F32, tag="t512")
            nc.vector.reciprocal(recip[:], rs_sb[:])
            recipT = stream.tile([4, NI], F32, tag="t512")
            for ic in range(2):
                trp = ps_mm.tile([4, 128], F32, tag="mmps")
                nc.tensor.transpose(trp[:], recip[:, ic * 4 : (ic + 1) * 4], identity_f[:])
                nc.vector.tensor_copy(recipT[:, ic * 128 : (ic + 1) * 128], trp[:])
            rbc_ps = ps_mm.tile([128, NI], F32, tag="mmps")
            nc.tensor.matmul(rbc_ps[:], ind4[:], recipT[:], start=True, stop=True)
            rbc = stream.tile([128, NI], F32, tag="t512")
            nc.vector.tensor_copy(rbc[:], rbc_ps[:])
            nc.vector.scalar_tensor_tensor(
                attn[:, w, :], rbc[:], 1.0, av_ps[:], AL.mult, AL.mult
            )

        # -------- FF projection + ff_b + residual + LayerNorm, direct out -------
        for it in range(2):
            ps = ps_mm.tile([128, D], F32, tag="mmps")
            for w in range(4):
                nc.tensor.matmul(
                    ps[:],
                    attn[:, w, it * 128 : (it + 1) * 128],
                    ffw_t[:, w, :],
                    start=(w == 0), stop=False,
                )
            nc.tensor.matmul(
                ps[:], ones_row_b[0:1, :], ffb_row[0:1, :], start=False, stop=True
            )
            x_ld = stream.tile([128, D], F32, tag="t512")
            nc.scalar.copy(x_ld[:], xrows_t[:, it, :])
            y = stream.tile([128, D], F32, tag="t512")
            ysum = tiny.tile([128, 1], F32, tag="t1")
            nc.vector.scalar_tensor_tensor(
                y[:], x_ld[:], 1.0, ps[:], AL.mult, AL.add, accum_out=ysum[:],
            )
            negmu = tiny.tile([128, 1], F32, tag="t1")
            nc.vector.tensor_scalar(negmu[:], ysum[:], -1.0 / D, None, AL.mult)
            sq = stream.tile([128, D], F32, tag="t512")
            ssq = tiny.tile([128, 1], F32, tag="t1")
            nc.scalar.activation(
                sq[:], y[:], AF.Square, bias=negmu[:], scale=1.0, accum_out=ssq[:]
            )
            veps = tiny.tile([128, 1], F32, tag="t1")
            nc.vector.tensor_scalar(veps[:], ssq[:], 1.0 / D, LN_EPS, AL.mult, AL.add)
            std = tiny.tile([128, 1], F32, tag="t1")
            nc.scalar.activation(std[:], veps[:], AF.Sqrt)
            rstd = tiny.tile([128, 1], F32, tag="t1")
            nc.vector.reciprocal(rstd[:], std[:])
            if trivial_ln:
                o = stream.tile([128, D], BF16, tag="to")
                nc.vector.tensor_scalar(o[:], y[:], negmu[:], rstd[:], AL.add, AL.mult)
            else:
                z = stream.tile([128, D], F32, tag="t512")
                nc.vector.tensor_scalar(z[:], y[:], negmu[:], rstd[:], AL.add, AL.mult)
                zw = stream.tile([128, D], F32, tag="t512")
                nc.vector.scalar_tensor_tensor(zw[:], lnw_bc[:], 1.0, z[:], AL.mult, AL.mult)
                o = stream.tile([128, D], BF16, tag="to")
                nc.vector.scalar_tensor_tensor(o[:], lnb_bc[:], 1.0, zw[:], AL.mult, AL.add)
            nc.sync.dma_start(out=t[f"d_out{it}"], in_=o[:])


# ---------------------------------------------------------------------------
# Host side: program cache, cached PJRT runner, shard prep
# ---------------------------------------------------------------------------

_PROGRAM_CACHE = {}
_RUNNER_CACHE = {}
from concurrent.futures import ThreadPoolExecutor as _TPE

_PREP_POOL = _TPE(max_workers=8)


def _get_program(trivial_ln):
    key = (bool(trivial_ln),)
    if key not in _PROGRAM_CACHE:
        _PROGRAM_CACHE[key] = build_program(bool(trivial_ln))
    return _PROGRAM_CACHE[key]


def _get_runner(nc):
    """Build (once) a persistent jitted sharded callable for `nc`.

    Mirrors concourse.bass2jax.run_bass_via_pjrt (the axon execution path of
    bass_utils.run_bass_kernel_spmd) but hoists the jax.jit out of the
    per-call path and assembles the global arrays without an extra concat.
    """
    key = id(nc)
    if key in _RUNNER_CACHE:
        return _RUNNER_CACHE[key]

    import jax
    from jax.sharding import Mesh, PartitionSpec
    from jax.experimental.shard_map import shard_map
    from concourse.bass2jax import (_bass_exec_p, install_neuronx_cc_hook,
                                    partition_id_tensor)

    install_neuronx_cc_hook()
    assert nc.dbg_addr is None or not nc.dbg_callbacks

    partition_name = nc.partition_id_tensor.name if nc.partition_id_tensor else None
    in_names, out_names, out_avals = [], [], []
    for alloc in nc.m.functions[0].allocations:
        if not isinstance(alloc, mybir.MemoryLocationSet):
            continue
        name = alloc.memorylocations[0].name
        if alloc.kind == "ExternalInput":
            if name != partition_name:
                in_names.append(name)
        elif alloc.kind == "ExternalOutput":
            out_names.append(name)
            out_avals.append(jax.core.ShapedArray(
                tuple(alloc.tensor_shape), mybir.dt.np(alloc.dtype)))
    n_params = len(in_names)
    n_outs = len(out_avals)
    all_in_names = list(in_names) + out_names
    if partition_name is not None:
        all_in_names.append(partition_name)
    donate = tuple(range(n_params, n_params + n_outs))

    def _body(*args):
        operands = list(args)
        if partition_name is not None:
            operands.append(partition_id_tensor())
        outs = _bass_exec_p.bind(
            *operands, out_avals=tuple(out_avals), in_names=tuple(all_in_names),
            out_names=tuple(out_names), lowering_input_output_aliases=(),
            sim_require_finite=True, sim_require_nnan=True, nc=nc)
        return tuple(outs)

    devices = jax.devices()[:N_CORES]
    mesh = Mesh(np.asarray(devices), ("core",))
    in_specs = (PartitionSpec("core"),) * (n_params + n_outs)
    out_specs = (PartitionSpec("core"),) * n_outs
    sharded = jax.jit(
        shard_map(_body, mesh=mesh, in_specs=in_specs, out_specs=out_specs,
                  check_rep=False),
        donate_argnums=donate, keep_unused=True)

    zero_shapes = [(N_CORES * a.shape[0], *a.shape[1:]) for a in out_avals]
    zero_dtypes = [a.dtype for a in out_avals]
    from concurrent.futures import ThreadPoolExecutor
    fetch_pool = ThreadPoolExecutor(max_workers=max(len(out_names), 1))

    def run(globals_by_name):
        concat_in = [globals_by_name[name] for name in in_names]
        concat_zeros = [np.zeros(s, d) for s, d in zip(zero_shapes, zero_dtypes)]
        out_arrs = sharded(*concat_in, *concat_zeros)
        futs = [fetch_pool.submit(np.asarray, o) for o in out_arrs]
        return {
            name: futs[i].result().reshape(N_CORES, *out_avals[i].shape)
            for i, name in enumerate(out_names)
        }

    _RUNNER_CACHE[key] = run
    return run


def _shard_globals(x, pdist, angle, adj, mask, gp, ga, w_bias,
                   att_w, ff_w, ff_b, ln_w, ln_b):
    """Build the concatenated global input arrays (blob + smalls)."""
    blob = np.empty((N_CORES * BLOB_ROWS, 1024), BF)
    smalls = np.zeros((N_CORES * 6 * 512,), np.float32)

    awT = att_w.T.astype(BF)                       # [D, 1536]
    W = np.empty((1024, 1024), BF)
    W[0:512] = awT[:, 0:1024]
    W[512:768] = awT[:, 1024:1536].reshape(256, 1024)
    W[768:1024] = ff_w.T.astype(BF).reshape(256, 1024)
    xT_b = [x[b].T.astype(BF) for b in range(B)]   # [D, N] per batch
    maskf = [np.where(mask[b, 0, 0, :], np.float32(NEG_INF), np.float32(0.0))
             for b in range(B)]
    simple_g = gp == 1.0 and ga == 1.0

    def fill_core(c):
        b, ih = c // 4, c % 4
        i0 = ih * NI
        irows = slice(i0, i0 + NI)
        bl = blob[c * BLOB_ROWS : (c + 1) * BLOB_ROWS]

        if simple_g:
            p0c = adj[b, irows] - pdist[b, irows]
        else:
            p0c = np.float32(ga) * adj[b, irows] - np.float32(gp) * pdist[b, irows]
        bl[R_P0 : R_P0 + NI] = p0c
        bl[R_A0 : R_A0 + NI] = angle[b, irows, :, 0]
        bl[R_A1 : R_A1 + NI] = angle[b, irows, :, 1]
        bl[R_XQ : R_XQ + 128] = xT_b[b][:, irows].reshape(128, 1024)
        bl[R_XP : R_XP + 128] = xT_b[b][ih * 128 : (ih + 1) * 128]
        bl[R_W : R_W + 128] = W[c * 128 : (c + 1) * 128]

        s = smalls[c * 6 * 512 : (c + 1) * 6 * 512]
        s[0:512] = ln_w
        s[512:1024] = ln_b
        s[1024:1536] = ff_b
        s[1536:2560] = maskf[b]
        s[2560 : 2560 + 2 * H] = w_bias[:, 0:2].reshape(-1)

    # numpy cast/copy loops release the GIL; parallelize the per-core fill
    futs = [_PREP_POOL.submit(fill_core, c) for c in range(N_CORES)]
    for f in futs:
        f.result()
    return {"blob": blob, "smalls": smalls}


def _reference_numpy(x, pdist, angle, adj, mask, gamma_p, gamma_adj, w_bias,
                     att_w, ff_w, ff_b, ln_w, ln_b):
    """Exact fallback (used only for non-head-uniform gammas)."""
    f8 = np.float64
    x64 = x.astype(f8)
    qkv = x64 @ att_w.astype(f8).T
    wq, wk, wv = np.split(qkv, 3, axis=-1)
    bsz, n = x.shape[0], x.shape[1]
    wq = wq.reshape(bsz, n, H, DH)
    wk = wk.reshape(bsz, n, H, DH)
    wv = wv.reshape(bsz, n, H, DH)
    score = np.einsum('bihd,bjhd->bhij', wq, wk, optimize=True) / np.sqrt(f8(DH))
    score = score - gamma_p.astype(f8)[None, :, None, None] * pdist.astype(f8)[:, None]
    score = score + np.einsum('bijc,hc->bhij', angle.astype(f8), w_bias.astype(f8),
                              optimize=True)
    score = score + gamma_adj.astype(f8)[None, :, None, None] * adj.astype(f8)[:, None]
    score = np.where(mask, NEG_INF, score)
    score -= score.max(-1, keepdims=True)
    p = np.exp(score)
    p /= p.sum(-1, keepdims=True)
    att = np.einsum('bhij,bjhd->bihd', p, wv, optimize=True).reshape(bsz, n, H * DH)
    y = x64 + att @ ff_w.astype(f8).T + ff_b.astype(f8)
    mu = y.mean(-1, keepdims=True)
    var = np.square(y - mu).mean(-1, keepdims=True)
    out = (y - mu) / np.sqrt(var + LN_EPS) * ln_w.astype(f8) + ln_b.astype(f8)
    return out.astype(np.float32)


def kernel(x, pdist, angle, adj, mask, gamma_p, gamma_adj, w_bias,
           att_w, ff_w, ff_b, ln_w, ln_b, **_unused):
    x = np.asarray(x, dtype=np.float32)
    pdist = np.asarray(pdist, dtype=np.float32)
    angle = np.asarray(angle, dtype=np.float32)
    adj = np.asarray(adj, dtype=np.float32)
    mask = np.asarray(mask)
    gamma_p = np.asarray(gamma_p, dtype=np.float32)
    gamma_adj = np.asarray(gamma_adj, dtype=np.float32)
    w_bias = np.asarray(w_bias, dtype=np.float32)
    att_w = np.asarray(att_w, dtype=np.float32)
    ff_w = np.asarray(ff_w, dtype=np.float32)
    ff_b = np.asarray(ff_b, dtype=np.float32)
    ln_w = np.asarray(ln_w, dtype=np.float32)
    ln_b = np.asarray(ln_b, dtype=np.float32)

    uniform = bool(
        np.all(gamma_p == gamma_p.flat[0]) and np.all(gamma_adj == gamma_adj.flat[0])
    )
    if not uniform:
        return _reference_numpy(x, pdist, angle, adj, mask, gamma_p, gamma_adj,
                                w_bias, att_w, ff_w, ff_b, ln_w, ln_b)
    gp = float(gamma_p.flat[0])
    ga = float(gamma_adj.flat[0])

    trivial_ln = bool(np.all(ln_w == 1.0) and np.all(ln_b == 0.0))
    nc = _get_program(trivial_ln)
    run = _get_runner(nc)
    g = _shard_globals(x, pdist, angle, adj, mask, gp, ga, w_bias,
                       att_w, ff_w, ff_b, ln_w, ln_b)
    res = run(g)  # out0/out1: [8, 128, D] bf16

    out = np.empty((B, N, D), dtype=np.float32)
    for c in range(N_CORES):
        b, ih = c // 4, c % 4
        i0 = ih * NI
        out[b, i0 : i0 + 128, :] = res["out0"][c]
        out[b, i0 + 128 : i0 + 256, :] = res["out1"][c]
    return out


# revision 32
# speedup vs baseline: 2.2402x; 1.0333x over previous
"""Trainium2 Bass kernel for nn_MultiHeadAttention_52862457480066.

Reference computation (B=2, N=1024, D=512, H=16, DH=32):
    qkv = x @ att_w.T ; q,k,v per head
    score = q.k/sqrt(DH) - gamma_p*pdist + angle@w_bias.T + gamma_adj*adj
    score = where(mask, -1e9, score) ; prob = softmax_j(score)
    att = prob @ v ; ff = att @ ff_w.T + ff_b ; y = x + ff ; out = LayerNorm(y)*ln_w+ln_b

Sharding over 8 cores: (batch b in 2) x (query-quarter ih in 4). Each core owns
ALL 16 heads for its 256 query rows, so its FF output rows are complete and no
cross-core reduction of activations is needed.

End-to-end wall time is dominated by host->device transfer over the axon
tunnel (~115-170 MB/s, ~85 ms fixed, ~6 ms per extra jit arg), so the design
minimizes uploaded bytes and arg count:
- All large tensors ship as ONE bf16 blob arg per core (bias slices, x.T
  slices, weight shard); bf16 halves bytes against a 2e-2 rel-err budget.
- pdist and adj only appear as P0 = gamma_adj*adj - gamma_p*pdist when the
  gammas are head-uniform (they are for this module's inputs), so the host
  combines them into ONE tensor. Non-uniform gammas fall back to exact numpy.
- Bias slices are per-core-unique; with the all-heads sharding nothing is
  uploaded twice. x[b].T (needed in full for K/V) is uploaded as per-core
  quarters and AllGathered on-device within each batch's 4-core group; the
  weights are uploaded as 1/8 shards and AllGathered across all 8 cores.
- Bias tensors upload in natural [i,j] layout (contiguous host slices) and
  are transposed to the [j,i] score layout on-device by the PE, which has
  large headroom. x rows for the residual are likewise recovered on-device by
  transposing the uploaded x[b,irows].T slice.
- Scores are computed TRANSPOSED ([j_part, i_free]) so softmax'd probs feed
  the attention*V matmul directly as the moving operand. All score-bias terms
  enter via PE identity matmuls (angle features) or a DVE add fused with the
  PSUM evacuation (P0), so the hot softmax path is one DVE + one ACT pass.
- Softmax denominators come from N=1 matmuls (ones moving operand) giving
  rowsums in [i_part, head_free] layout; normalization is deferred to after
  the AV matmul (divides 16*256 values per core instead of 4.2M).
- PSUM accumulators written by interleaved matmul chains are zero-initialized
  by one full-coverage start=True matmul; everything after runs start=False.
- The jitted PJRT executable is built once and cached; per-call work is host
  slicing/casting, one sharded transfer, execution, and two parallel bf16
  fetches.
"""

import math

import numpy as np

import concourse.bass as bass
import concourse.tile as tile
from concourse import bacc, mybir
from concourse.masks import make_identity

B, N, D, H, DH = 2, 1024, 512, 16, 32
NI = 256             # query rows per core
NJT = N // 128       # key tiles (partition dim j)
NEG_INF = -1e9
LN_EPS = 1e-5
QSCALE = 1.0 / math.sqrt(DH)
F32 = mybir.dt.float32
BF16 = mybir.dt.bfloat16
N_CORES = 8
XG_GROUPS = [[0, 1, 2, 3], [4, 5, 6, 7]]   # x[b].T AllGather within batch
WG_GROUPS = [[0, 1, 2, 3, 4, 5, 6, 7]]     # weight AllGather across all cores

BF = np.dtype(mybir.dt.np(BF16))  # ml_dtypes.bfloat16

# blob row ranges (per core, [1152, 1024] bf16)
R_P0, R_A0, R_A1 = 0, 256, 512       # bias slices [256, N] natural
R_XQ = 768                           # x[b,irows].T packed [512,256]->[128,1024]
R_XP = 896                           # x[b].T rows [ih*128,(ih+1)*128) for AllGather
R_W = 1024                           # weight-pack shard W[c*128:(c+1)*128]
BLOB_ROWS = 1152
# weight pack W [1024, 1024] bf16 (same on all cores before sharding):
#   rows 0:512   att_w.T[:, 0:1024]            (q feats 0:512 | k feats 512:1024)
#   rows 512:768 att_w.T[:, 1024:1536] flat    (v)
#   rows 768:1024 ff_w.T flat
# smalls [6*512] f32: lnw, lnb, ffb, maskb(1024), hcoef w0/w1 interleaved (32)


def build_program(trivial_ln: bool):
    """Build the SPMD bass program (identical on all 8 cores)."""
    nc = bacc.Bacc("TRN2", target_bir_lowering=False, debug=False, num_devices=N_CORES)

    t = {}
    t["h_blob"] = nc.dram_tensor("blob", [BLOB_ROWS, 1024], BF16, kind="ExternalInput")
    t["h_smalls"] = nc.dram_tensor("smalls", [6 * 512], F32, kind="ExternalInput")
    # collectives may not read IO tensors: bounce the blob slices to internal
    t["d_xp"] = nc.dram_tensor("xp", [128, N], BF16).ap()
    t["d_wp"] = nc.dram_tensor("wp", [128, 1024], BF16).ap()
    t["d_xg"] = nc.dram_tensor("xg", [512, N], BF16).ap()
    t["h_wg"] = nc.dram_tensor("wg", [1024, 1024], BF16, addr_space="Shared")
    t["d_out0"] = nc.dram_tensor("out0", [128, D], BF16, kind="ExternalOutput").ap()
    t["d_out1"] = nc.dram_tensor("out1", [128, D], BF16, kind="ExternalOutput").ap()

    with tile.TileContext(nc) as tc:
        _emit(nc, tc, t, trivial_ln)
    nc.compile()
    return nc


def _emit(nc, tc, t, trivial_ln):
    AL = mybir.AluOpType
    AF = mybir.ActivationFunctionType
    from contextlib import ExitStack

    blob = t["h_blob"]
    sm = t["h_smalls"]
    wg = t["h_wg"]

    def blob_ap(row0, shape3):
        """AP over blob rows: [128, k, cols] with partition-major packing."""
        _, k, cols = shape3
        return bass.AP(tensor=blob, offset=row0 * 1024,
                       ap=[[cols, 128], [128 * cols, k], [1, cols]])

    def wg_ap(off, k, cols):
        return bass.AP(tensor=wg, offset=off,
                       ap=[[cols, 128], [128 * cols, k], [1, cols]])

    ctx = ExitStack()
    with ctx:
        consts = ctx.enter_context(tc.tile_pool(name="consts", bufs=1))
        big = ctx.enter_context(tc.tile_pool(name="big", bufs=1))
        stream = ctx.enter_context(tc.tile_pool(name="stream", bufs=6))
        tiny = ctx.enter_context(tc.tile_pool(name="tiny", bufs=8))
        ppool = ctx.enter_context(tc.tile_pool(name="ppool", bufs=6))
        ps_mm = ctx.enter_context(tc.tile_pool(name="ps_mm", bufs=4, space="PSUM"))
        ps_sc = ps_mm
        ps_av = ctx.enter_context(tc.tile_pool(name="ps_av", bufs=4, space="PSUM"))
        ps_rs = ps_av

        # ---------------- collectives: gather x[b].T and the weight pack ------
        nc.sync.dma_start(
            out=t["d_xp"],
            in_=bass.AP(tensor=blob, offset=R_XP * 1024, ap=[[1024, 128], [1, 1024]]),
        )
        nc.sync.dma_start(
            out=t["d_wp"],
            in_=bass.AP(tensor=blob, offset=R_W * 1024, ap=[[1024, 128], [1, 1024]]),
        )
        nc.gpsimd.collective_compute(
            "AllGather", AL.bypass, replica_groups=XG_GROUPS,
            ins=[t["d_xp"]], outs=[t["d_xg"]],
        )
        nc.gpsimd.collective_compute(
            "AllGather", AL.bypass, replica_groups=WG_GROUPS,
            ins=[t["d_wp"]],
            outs=[bass.AP(tensor=wg, offset=0, ap=[[1024, 1024], [1, 1024]])],
        )

        # ---------------- constants / small tiles ----------------
        identity_f = consts.tile([128, 128], F32)  # f32 transposes (recip path)
        make_identity(nc, identity_f[:])
        identity_b = consts.tile([128, 128], BF16)  # bf16 transposes (loads)
        nc.vector.tensor_copy(identity_b[:], identity_f[:])
        ind4 = consts.tile([4, 128], F32)  # ind4[k, m] = (m//32 == k)
        nc.gpsimd.memset(ind4[:], 1.0)
        nc.gpsimd.affine_select(
            out=ind4[:], in_=ind4[:], compare_op=AL.is_ge, fill=0.0,
            base=0, pattern=[[1, 128]], channel_multiplier=-32,
        )
        nc.gpsimd.affine_select(
            out=ind4[:], in_=ind4[:], compare_op=AL.is_ge, fill=0.0,
            base=31, pattern=[[-1, 128]], channel_multiplier=32,
        )
        ones_col = consts.tile([128, 1], BF16)
        nc.gpsimd.memset(ones_col[:], 1.0)
        ones_row_f = consts.tile([1, 128], F32)
        nc.gpsimd.memset(ones_row_f[:], 1.0)
        ones_row_b = consts.tile([1, 128], BF16)
        nc.vector.tensor_copy(ones_row_b[:], ones_row_f[:])
        zeros_row_b = consts.tile([1, 512], BF16)
        nc.gpsimd.memset(zeros_row_b[:], 0.0)

        maskb = consts.tile([128, NJT], F32)
        nc.gpsimd.dma_start(
            out=maskb[:],
            in_=bass.AP(tensor=sm, offset=3 * 512, ap=[[1, 128], [128, NJT]]),
        )
        hbc = []  # w0, w1 broadcast [128, H]
        for c in range(2):
            bc = consts.tile([128, H], F32, tag=f"hbc{c}")
            nc.gpsimd.dma_start(
                out=bc[:], in_=bass.AP(tensor=sm, offset=5 * 512 + c, ap=[[0, 128], [2, H]])
            )
            hbc.append(bc)

        # per-head scaled identities for the angle-feature PSUM adds
        idw = []  # idw[c][hl] = identity * w_bias[head, c]
        for c, wbc in enumerate(hbc):
            row = []
            for hl in range(H):
                it_ = consts.tile([128, 128], BF16, tag=f"idw{c}_{hl}")
                nc.vector.tensor_scalar(
                    it_[:], identity_b[:], wbc[:, hl : hl + 1], None, AL.mult
                )
                row.append(it_)
            idw.append(row)

        ffb_f = consts.tile([1, D], F32)
        nc.gpsimd.dma_start(
            out=ffb_f[:], in_=bass.AP(tensor=sm, offset=2 * 512, ap=[[0, 1], [1, D]])
        )
        ffb_row = consts.tile([1, D], BF16)
        nc.vector.tensor_copy(ffb_row[:], ffb_f[:])

        lnw_bc = lnb_bc = None
        if not trivial_ln:
            lnw_row = consts.tile([1, D], F32)
            nc.gpsimd.dma_start(
                out=lnw_row[:], in_=bass.AP(tensor=sm, offset=0, ap=[[0, 1], [1, D]])
            )
            lnb_row = consts.tile([1, D], F32)
            nc.gpsimd.dma_start(
                out=lnb_row[:], in_=bass.AP(tensor=sm, offset=512, ap=[[0, 1], [1, D]])
            )
            lnw_bc = consts.tile([128, D], F32)
            lnb_bc = consts.tile([128, D], F32)
            for row, bc in ((lnw_row, lnw_bc), (lnb_row, lnb_bc)):
                ps = ps_mm.tile([128, D], F32, tag="mmps")
                nc.tensor.matmul(ps[:], ones_row_f[0:1, :], row[0:1, :], start=True, stop=True)
                nc.vector.tensor_copy(bc[:], ps[:])

        # ---------------- load big bf16 inputs ----------------
        xq_t = big.tile([128, 4, NI], BF16)      # x[b,irows].T  [d-part, dc, i]
        nc.sync.dma_start(out=xq_t[:], in_=blob_ap(R_XQ, [128, 4, NI]))
        xg_t = big.tile([128, 4, N], BF16)       # gathered x[b].T [d-part, dc, n]
        nc.sync.dma_start(out=xg_t[:], in_=t["d_xg"].rearrange("(c p) n -> p c n", p=128))
        wqk_t = big.tile([128, 4, 1024], BF16)   # att_w.T[:, 0:1024]
        nc.sync.dma_start(out=wqk_t[:], in_=wg_ap(0, 4, 1024))
        wv_t = big.tile([128, 4, 512], BF16)     # att_w.T[:, 1024:1536]
        nc.sync.dma_start(out=wv_t[:], in_=wg_ap(512 * 1024, 4, 512))
        ffw_t = big.tile([128, 4, 512], BF16)    # ff_w.T
        nc.sync.dma_start(out=ffw_t[:], in_=wg_ap(768 * 1024, 4, 512))

        # ---------------- q/k projection (transposed: [feat, n]) ----------------
        qT = big.tile([128, 4, NI], BF16)   # [dh-part(4h), ft, i]
        for ft in range(4):
            ps = ps_mm.tile([128, NI], F32, tag="mmps")
            for dc in range(4):
                nc.tensor.matmul(
                    ps[:], wqk_t[:, dc, ft * 128 : (ft + 1) * 128], xq_t[:, dc, :],
                    start=(dc == 0), stop=(dc == 3),
                )
            nc.vector.tensor_scalar(qT[:, ft, :], ps[:], QSCALE, None, AL.mult)
        kT = big.tile([128, 4, N], BF16)    # [dh-part(4h), ft, n]
        for ft in range(4):
            for nc_i in range(2):
                ps = ps_mm.tile([128, 512], F32, tag="mmps")
                for dc in range(4):
                    nc.tensor.matmul(
                        ps[:], wqk_t[:, dc, 512 + ft * 128 : 512 + (ft + 1) * 128],
                        xg_t[:, dc, nc_i * 512 : nc_i * 512 + 512],
                        start=(dc == 0), stop=(dc == 3),
                    )
                nc.vector.tensor_copy(kT[:, ft, nc_i * 512 : nc_i * 512 + 512], ps[:])

        # ---------------- v projection (natural: [n, feat]) ----------------
        v = big.tile([128, NJT, 512], BF16)  # [j-part, jt, 16h*32]
        for nt in range(NJT):
            ps = ps_mm.tile([128, 512], F32, tag="mmps")
            for dc in range(4):
                nc.tensor.matmul(
                    ps[:], xg_t[:, dc, nt * 128 : (nt + 1) * 128], wv_t[:, dc, :],
                    start=(dc == 0), stop=(dc == 3),
                )
            nc.scalar.copy(v[:, nt, :], ps[:])

        # ---------------- x rows for the residual: transpose xq_t -------------
        xrows_t = big.tile([128, 2, D], BF16)  # [i-part, it, d]
        for it in range(2):
            ps = ps_mm.tile([128, D], F32, tag="mmps")
            for dc in range(4):
                nc.tensor.matmul(
                    ps[:, dc * 128 : (dc + 1) * 128],
                    xq_t[:, dc, it * 128 : (it + 1) * 128],
                    identity_b[:],
                    start=True, stop=True, skip_group_check=True,
                )
            nc.scalar.copy(xrows_t[:, it, :], ps[:])

        # ------- bias features: load natural, transpose to [j, i] on the PE ------
        nat_ctx = ExitStack()
        nat = nat_ctx.enter_context(tc.tile_pool(name="nat", bufs=1))
        p0nat = nat.tile([128, 2, N], BF16)
        nc.sync.dma_start(out=p0nat[:], in_=blob_ap(R_P0, [128, 2, N]))
        a0nat = nat.tile([128, 2, N], BF16)
        nc.sync.dma_start(out=a0nat[:], in_=blob_ap(R_A0, [128, 2, N]))
        a1nat = nat.tile([128, 2, N], BF16)
        nc.sync.dma_start(out=a1nat[:], in_=blob_ap(R_A1, [128, 2, N]))

        P0 = big.tile([128, NJT, NI], BF16)
        a0 = big.tile([128, NJT, NI], BF16)
        a1 = big.tile([128, NJT, NI], BF16)
        for natt, dst in ((p0nat, P0), (a0nat, a0), (a1nat, a1)):
            for jt in range(NJT):
                ps = ps_mm.tile([128, NI], F32, tag="mmps")
                for it in range(2):
                    nc.tensor.matmul(
                        ps[:, it * 128 : (it + 1) * 128],
                        natt[:, it, jt * 128 : (jt + 1) * 128],
                        identity_b[:],
                        start=True, stop=True, skip_group_check=True,
                    )
                nc.scalar.copy(dst[:, jt, :], ps[:])
        nat_ctx.close()

        # ---------------- attention: 4 waves of 4 heads ----------------
        attn = big.tile([128, 4, NI], BF16)  # normalized att.T  [4h*32dh, wave, i]
        for w in range(4):
            av_ps = ps_av.tile([128, NI], F32, tag="avps")
            rs_ps = ps_rs.tile([128, 8], F32, tag="avps")
            # zero-init accumulator banks (see module docstring)
            nc.tensor.matmul(
                av_ps[:], ones_row_b[0:1, :], zeros_row_b[0:1, 0:NI],
                start=True, stop=False, skip_group_check=True,
            )
            nc.tensor.matmul(
                rs_ps[:], ones_row_b[0:1, :], zeros_row_b[0:1, 0:8],
                start=True, stop=False, skip_group_check=True,
            )
            for jt in range(NJT):
                p_tiles = []
                for hh in range(4):
                    hl = w * 4 + hh
                    sc = ps_sc.tile([128, NI], F32, tag="mmps")
                    nc.tensor.matmul(
                        sc[:],
                        kT[hh * 32 : (hh + 1) * 32, w, jt * 128 : (jt + 1) * 128],
                        qT[hh * 32 : (hh + 1) * 32, w, :],
                        start=True, stop=False, tile_position=(hh * 32, 0),
                    )
                    nc.tensor.matmul(
                        sc[:], idw[0][hl][:], a0[:, jt, :], start=False, stop=False,
                    )
                    nc.tensor.matmul(
                        sc[:], idw[1][hl][:], a1[:, jt, :], start=False, stop=True,
                    )
                    # P0 add on the DVE, fused with the PSUM evacuation the
                    # exp would otherwise need.
                    xs = stream.tile([128, NI], F32, tag="xs")
                    nc.vector.scalar_tensor_tensor(
                        xs[:], P0[:, jt, :], 1.0, sc[:], AL.mult, AL.add
                    )
                    pT = ppool.tile([128, NI], BF16, tag="pT")
                    nc.scalar.activation(
                        pT[:], xs[:], AF.Exp, bias=maskb[:, jt : jt + 1], scale=1.0
                    )
                    p_tiles.append(pT)
                for hh in range(4):
                    pT = p_tiles[hh]
                    vcol = (w * 4 + hh) * 32
                    nc.tensor.matmul(
                        av_ps[hh * 32 : (hh + 1) * 32, :],
                        v[:, jt, vcol : vcol + 32],
                        pT[:],
                        start=False, stop=(jt == NJT - 1 and hh == 3),
                        tile_position=(0, hh * 32),
                        skip_group_check=True,
                    )
                    for ic in range(2):
                        col = ic * 4 + hh
                        nc.tensor.matmul(
                            rs_ps[:, col : col + 1],
                            pT[:, ic * 128 : (ic + 1) * 128],
                            ones_col[:],
                            start=False,
                            stop=(jt == NJT - 1 and hh == 3 and ic == 1),
                            skip_group_check=True,
                        )
            # normalize: attn = av / rowsum
            rs_sb = stream.tile([128, 8], F32, tag="t512")
            nc.vector.tensor_copy(rs_sb[:], rs_ps[:])
            recip = stream.tile([128, 8], F32, tag="t512")
            nc.vector.reciprocal(recip[:], rs_sb[:])
            recipT = stream.tile([4, NI], F32, tag="t512")
            for ic in range(2):
                trp = ps_mm.tile([4, 128], F32, tag="mmps")
                nc.tensor.transpose(trp[:], recip[:, ic * 4 : (ic + 1) * 4], identity_f[:])
                nc.vector.tensor_copy(recipT[:, ic * 128 : (ic + 1) * 128], trp[:])
            rbc_ps = ps_mm.tile([128, NI], F32, tag="mmps")
            nc.tensor.matmul(rbc_ps[:], ind4[:], recipT[:], start=True, stop=True)
            rbc = stream.tile([128, NI], F32, tag="t512")
            nc.vector.tensor_copy(rbc[:], rbc_ps[:])
            nc.vector.scalar_tensor_tensor(
                attn[:, w, :], rbc[:], 1.0, av_ps[:], AL.mult, AL.mult
            )

        # -------- FF projection + ff_b + residual + LayerNorm, direct out -------
        for it in range(2):
            ps = ps_mm.tile([128, D], F32, tag="mmps")
            for w in range(4):
                nc.tensor.matmul(
                    ps[:],
                    attn[:, w, it * 128 : (it + 1) * 128],
                    ffw_t[:, w, :],
                    start=(w == 0), stop=False,
                )
            nc.tensor.matmul(
                ps[:], ones_row_b[0:1, :], ffb_row[0:1, :], start=False, stop=True
            )
            x_ld = stream.tile([128, D], F32, tag="t512")
            nc.scalar.copy(x_ld[:], xrows_t[:, it, :])
            y = stream.tile([128, D], F32, tag="t512")
            ysum = tiny.tile([128, 1], F32, tag="t1")
            nc.vector.scalar_tensor_tensor(
                y[:], x_ld[:], 1.0, ps[:], AL.mult, AL.add, accum_out=ysum[:],
            )
            negmu = tiny.tile([128, 1], F32, tag="t1")
            nc.vector.tensor_scalar(negmu[:], ysum[:], -1.0 / D, None, AL.mult)
            sq = stream.tile([128, D], F32, tag="t512")
            ssq = tiny.tile([128, 1], F32, tag="t1")
            nc.scalar.activation(
                sq[:], y[:], AF.Square, bias=negmu[:], scale=1.0, accum_out=ssq[:]
            )
            veps = tiny.tile([128, 1], F32, tag="t1")
            nc.vector.tensor_scalar(veps[:], ssq[:], 1.0 / D, LN_EPS, AL.mult, AL.add)
            std = tiny.tile([128, 1], F32, tag="t1")
            nc.scalar.activation(std[:], veps[:], AF.Sqrt)
            rstd = tiny.tile([128, 1], F32, tag="t1")
            nc.vector.reciprocal(rstd[:], std[:])
            if trivial_ln:
                o = stream.tile([128, D], BF16, tag="to")
                nc.vector.tensor_scalar(o[:], y[:], negmu[:], rstd[:], AL.add, AL.mult)
            else:
                z = stream.tile([128, D], F32, tag="t512")
                nc.vector.tensor_scalar(z[:], y[:], negmu[:], rstd[:], AL.add, AL.mult)
                zw = stream.tile([128, D], F32, tag="t512")
                nc.vector.scalar_tensor_tensor(zw[:], lnw_bc[:], 1.0, z[:], AL.mult, AL.mult)
                o = stream.tile([128, D], BF16, tag="to")
                nc.vector.scalar_tensor_tensor(o[:], lnb_bc[:], 1.0, zw[:], AL.mult, AL.add)
            nc.sync.dma_start(out=t[f"d_out{it}"], in_=o[:])


# ---------------------------------------------------------------------------
# Host side: program cache, cached PJRT runner, shard prep
# ---------------------------------------------------------------------------

_PROGRAM_CACHE = {}
_RUNNER_CACHE = {}
from concurrent.futures import ThreadPoolExecutor as _TPE

_PREP_POOL = _TPE(max_workers=8)


def _get_program(trivial_ln):
    key = (bool(trivial_ln),)
    if key not in _PROGRAM_CACHE:
        _PROGRAM_CACHE[key] = build_program(bool(trivial_ln))
    return _PROGRAM_CACHE[key]


def _get_runner(nc):
    """Build (once) a persistent jitted sharded callable for `nc`.

    Mirrors concourse.bass2jax.run_bass_via_pjrt (the axon execution path of
    bass_utils.run_bass_kernel_spmd) but hoists the jax.jit out of the
    per-call path and assembles the global arrays without an extra concat.
    """
    key = id(nc)
    if key in _RUNNER_CACHE:
        return _RUNNER_CACHE[key]

    import jax
    from jax.sharding import Mesh, PartitionSpec
    from jax.experimental.shard_map import shard_map
    from concourse.bass2jax import (_bass_exec_p, install_neuronx_cc_hook,
                                    partition_id_tensor)

    install_neuronx_cc_hook()
    assert nc.dbg_addr is None or not nc.dbg_callbacks

    partition_name = nc.partition_id_tensor.name if nc.partition_id_tensor else None
    in_names, out_names, out_avals = [], [], []
    for alloc in nc.m.functions[0].allocations:
        if not isinstance(alloc, mybir.MemoryLocationSet):
            continue
        name = alloc.memorylocations[0].name
        if alloc.kind == "ExternalInput":
            if name != partition_name:
                in_names.append(name)
        elif alloc.kind == "ExternalOutput":
            out_names.append(name)
            out_avals.append(jax.core.ShapedArray(
                tuple(alloc.tensor_shape), mybir.dt.np(alloc.dtype)))
    n_params = len(in_names)
    n_outs = len(out_avals)
    all_in_names = list(in_names) + out_names
    if partition_name is not None:
        all_in_names.append(partition_name)
    donate = tuple(range(n_params, n_params + n_outs))

    def _body(*args):
        operands = list(args)
        if partition_name is not None:
            operands.append(partition_id_tensor())
        outs = _bass_exec_p.bind(
            *operands, out_avals=tuple(out_avals), in_names=tuple(all_in_names),
            out_names=tuple(out_names), lowering_input_output_aliases=(),
            sim_require_finite=True, sim_require_nnan=True, nc=nc)
        return tuple(outs)

    devices = jax.devices()[:N_CORES]
    mesh = Mesh(np.asarray(devices), ("core",))
    in_specs = (PartitionSpec("core"),) * (n_params + n_outs)
    out_specs = (PartitionSpec("core"),) * n_outs
    sharded = jax.jit(
        shard_map(_body, mesh=mesh, in_specs=in_specs, out_specs=out_specs,
                  check_rep=False),
        donate_argnums=donate, keep_unused=True)

    zero_shapes = [(N_CORES * a.shape[0], *a.shape[1:]) for a in out_avals]
    zero_dtypes = [a.dtype for a in out_avals]
    from concurrent.futures import ThreadPoolExecutor
    fetch_pool = ThreadPoolExecutor(max_workers=max(len(out_names), 1))

    def run(globals_by_name):
        concat_in = [globals_by_name[name] for name in in_names]
        concat_zeros = [np.zeros(s, d) for s, d in zip(zero_shapes, zero_dtypes)]
        out_arrs = sharded(*concat_in, *concat_zeros)
        futs = [fetch_pool.submit(np.asarray, o) for o in out_arrs]
        return {
            name: futs[i].result().reshape(N_CORES, *out_avals[i].shape)
            for i, name in enumerate(out_names)
        }

    _RUNNER_CACHE[key] = run
    return run


def _shard_globals(x, pdist, angle, adj, mask, gp, ga, w_bias,
                   att_w, ff_w, ff_b, ln_w, ln_b):
    """Build the concatenated global input arrays (blob + smalls)."""
    blob = np.empty((N_CORES * BLOB_ROWS, 1024), BF)
    smalls = np.zeros((N_CORES * 6 * 512,), np.float32)

    def build_W():
        awT = att_w.T.astype(BF)                   # [D, 1536]
        W = np.empty((1024, 1024), BF)
        W[0:512] = awT[:, 0:1024]
        W[512:768] = awT[:, 1024:1536].reshape(256, 1024)
        W[768:1024] = ff_w.T.astype(BF).reshape(256, 1024)
        return W

    fW = _PREP_POOL.submit(build_W)
    fxT = [_PREP_POOL.submit(lambda bb: x[bb].T.astype(BF), b) for b in range(B)]
    maskf = [np.where(mask[b, 0, 0, :], np.float32(NEG_INF), np.float32(0.0))
             for b in range(B)]
    simple_g = gp == 1.0 and ga == 1.0

    def fill_core(c):
        b, ih = c // 4, c % 4
        i0 = ih * NI
        irows = slice(i0, i0 + NI)
        bl = blob[c * BLOB_ROWS : (c + 1) * BLOB_ROWS]

        if simple_g:
            p0c = adj[b, irows] - pdist[b, irows]
        else:
            p0c = np.float32(ga) * adj[b, irows] - np.float32(gp) * pdist[b, irows]
        bl[R_P0 : R_P0 + NI] = p0c
        bl[R_A0 : R_A0 + NI] = angle[b, irows, :, 0]
        bl[R_A1 : R_A1 + NI] = angle[b, irows, :, 1]
        xT_b = fxT[b].result()
        bl[R_XQ : R_XQ + 128] = xT_b[:, irows].reshape(128, 1024)
        bl[R_XP : R_XP + 128] = xT_b[ih * 128 : (ih + 1) * 128]
        bl[R_W : R_W + 128] = fW.result()[c * 128 : (c + 1) * 128]

        s = smalls[c * 6 * 512 : (c + 1) * 6 * 512]
        s[0:512] = ln_w
        s[512:1024] = ln_b
        s[1024:1536] = ff_b
        s[1536:2560] = maskf[b]
        s[2560 : 2560 + 2 * H] = w_bias[:, 0:2].reshape(-1)

    # numpy cast/copy loops release the GIL; parallelize the per-core fill
    futs = [_PREP_POOL.submit(fill_core, c) for c in range(N_CORES)]
    for f in futs:
        f.result()
    return {"blob": blob, "smalls": smalls}


def _reference_numpy(x, pdist, angle, adj, mask, gamma_p, gamma_adj, w_bias,
                     att_w, ff_w, ff_b, ln_w, ln_b):
    """Exact fallback (used only for non-head-uniform gammas)."""
    f8 = np.float64
    x64 = x.astype(f8)
    qkv = x64 @ att_w.astype(f8).T
    wq, wk, wv = np.split(qkv, 3, axis=-1)
    bsz, n = x.shape[0], x.shape[1]
    wq = wq.reshape(bsz, n, H, DH)
    wk = wk.reshape(bsz, n, H, DH)
    wv = wv.reshape(bsz, n, H, DH)
    score = np.einsum('bihd,bjhd->bhij', wq, wk, optimize=True) / np.sqrt(f8(DH))
    score = score - gamma_p.astype(f8)[None, :, None, None] * pdist.astype(f8)[:, None]
    score = score + np.einsum('bijc,hc->bhij', angle.astype(f8), w_bias.astype(f8),
                              optimize=True)
    score = score + gamma_adj.astype(f8)[None, :, None, None] * adj.astype(f8)[:, None]
    score = np.where(mask, NEG_INF, score)
    score -= score.max(-1, keepdims=True)
    p = np.exp(score)
    p /= p.sum(-1, keepdims=True)
    att = np.einsum('bhij,bjhd->bihd', p, wv, optimize=True).reshape(bsz, n, H * DH)
    y = x64 + att @ ff_w.astype(f8).T + ff_b.astype(f8)
    mu = y.mean(-1, keepdims=True)
    var = np.square(y - mu).mean(-1, keepdims=True)
    out = (y - mu) / np.sqrt(var + LN_EPS) * ln_w.astype(f8) + ln_b.astype(f8)
    return out.astype(np.float32)


def kernel(x, pdist, angle, adj, mask, gamma_p, gamma_adj, w_bias,
           att_w, ff_w, ff_b, ln_w, ln_b, **_unused):
    x = np.asarray(x, dtype=np.float32)
    pdist = np.asarray(pdist, dtype=np.float32)
    angle = np.asarray(angle, dtype=np.float32)
    adj = np.asarray(adj, dtype=np.float32)
    mask = np.asarray(mask)
    gamma_p = np.asarray(gamma_p, dtype=np.float32)
    gamma_adj = np.asarray(gamma_adj, dtype=np.float32)
    w_bias = np.asarray(w_bias, dtype=np.float32)
    att_w = np.asarray(att_w, dtype=np.float32)
    ff_w = np.asarray(ff_w, dtype=np.float32)
    ff_b = np.asarray(ff_b, dtype=np.float32)
    ln_w = np.asarray(ln_w, dtype=np.float32)
    ln_b = np.asarray(ln_b, dtype=np.float32)

    uniform = bool(
        np.all(gamma_p == gamma_p.flat[0]) and np.all(gamma_adj == gamma_adj.flat[0])
    )
    if not uniform:
        return _reference_numpy(x, pdist, angle, adj, mask, gamma_p, gamma_adj,
                                w_bias, att_w, ff_w, ff_b, ln_w, ln_b)
    gp = float(gamma_p.flat[0])
    ga = float(gamma_adj.flat[0])

    trivial_ln = bool(np.all(ln_w == 1.0) and np.all(ln_b == 0.0))
    nc = _get_program(trivial_ln)
    run = _get_runner(nc)
    g = _shard_globals(x, pdist, angle, adj, mask, gp, ga, w_bias,
                       att_w, ff_w, ff_b, ln_w, ln_b)
    res = run(g)  # out0/out1: [8, 128, D] bf16

    out = np.empty((B, N, D), dtype=np.float32)
    for c in range(N_CORES):
        b, ih = c // 4, c % 4
        i0 = ih * NI
        out[b, i0 : i0 + 128, :] = res["out0"][c]
        out[b, i0 + 128 : i0 + 256, :] = res["out1"][c]
    return out


# revision 38
# speedup vs baseline: 2.5070x; 1.1191x over previous
"""Trainium2 Bass kernel for nn_MultiHeadAttention_52862457480066.

Reference computation (B=2, N=1024, D=512, H=16, DH=32):
    qkv = x @ att_w.T ; q,k,v per head
    score = q.k/sqrt(DH) - gamma_p*pdist + angle@w_bias.T + gamma_adj*adj
    score = where(mask, -1e9, score) ; prob = softmax_j(score)
    att = prob @ v ; ff = att @ ff_w.T + ff_b ; y = x + ff ; out = LayerNorm(y)*ln_w+ln_b

Sharding over 8 cores: (batch b in 2) x (query-quarter ih in 4). Each core owns
ALL 16 heads for its 256 query rows, so its FF output rows are complete and no
cross-core reduction of activations is needed.

End-to-end wall time is dominated by host->device transfer over the axon
tunnel (~115-170 MB/s, ~85 ms fixed, ~6 ms per extra jit arg), so the design
minimizes uploaded bytes and arg count:
- All large tensors ship as ONE bf16 blob arg per core (bias slices, x.T
  slices, weight shard); bf16 halves bytes against a 2e-2 rel-err budget.
- pdist and adj only appear as P0 = gamma_adj*adj - gamma_p*pdist when the
  gammas are head-uniform (they are for this module's inputs), so the host
  combines them into ONE tensor. Non-uniform gammas fall back to exact numpy.
- Bias slices are per-core-unique; with the all-heads sharding nothing is
  uploaded twice. x[b].T (needed in full for K/V) is uploaded as per-core
  quarters and AllGathered on-device within each batch's 4-core group; the
  weights are uploaded as 1/8 shards and AllGathered across all 8 cores.
- Bias tensors upload in natural [i,j] layout (contiguous host slices) and
  are transposed to the [j,i] score layout on-device by the PE, which has
  large headroom. x rows for the residual are likewise recovered on-device by
  transposing the uploaded x[b,irows].T slice.
- Scores are computed TRANSPOSED ([j_part, i_free]) so softmax'd probs feed
  the attention*V matmul directly as the moving operand. All score-bias terms
  enter via PE identity matmuls (angle features) or a DVE add fused with the
  PSUM evacuation (P0), so the hot softmax path is one DVE + one ACT pass.
- Softmax denominators come from N=1 matmuls (ones moving operand) giving
  rowsums in [i_part, head_free] layout; normalization is deferred to after
  the AV matmul (divides 16*256 values per core instead of 4.2M).
- PSUM accumulators written by interleaved matmul chains are zero-initialized
  by one full-coverage start=True matmul; everything after runs start=False.
- The jitted PJRT executable is built once and cached; per-call work is host
  slicing/casting, one sharded transfer, execution, and two parallel bf16
  fetches.
"""

import math

import numpy as np

import concourse.bass as bass
import concourse.tile as tile
from concourse import bacc, mybir
from concourse.masks import make_identity

B, N, D, H, DH = 2, 1024, 512, 16, 32
NI = 256             # query rows per core
NJT = N // 128       # key tiles (partition dim j)
NEG_INF = -1e9
LN_EPS = 1e-5
QSCALE = 1.0 / math.sqrt(DH)
F32 = mybir.dt.float32
BF16 = mybir.dt.bfloat16
N_CORES = 8
XG_GROUPS = [[0, 1, 2, 3], [4, 5, 6, 7]]   # x[b].T AllGather within batch
WG_GROUPS = [[0, 1, 2, 3, 4, 5, 6, 7]]     # weight AllGather across all cores

BF = np.dtype(mybir.dt.np(BF16))  # ml_dtypes.bfloat16

# blob row ranges (per core, [384, 1024] bf16)
R_XQ = 0                             # x[b,irows].T packed [512,256]->[128,1024]
R_XP = 128                           # x[b].T rows [ih*128,(ih+1)*128) for AllGather
R_W = 256                            # weight-pack shard W[c*128:(c+1)*128]
BLOB_ROWS = 384
# qblob row ranges (per core, [768, 1024] uint8): per-slice-quantized biases,
# dequantized on-device as q*scale+lo with scale/lo shipped in smalls
Q_P0, Q_A0, Q_A1 = 0, 256, 512
QBLOB_ROWS = 768
U8 = mybir.dt.uint8
# weight pack W [1024, 1024] bf16 (same on all cores before sharding):
#   rows 0:512   att_w.T[:, 0:1024]            (q feats 0:512 | k feats 512:1024)
#   rows 512:768 att_w.T[:, 1024:1536] flat    (v)
#   rows 768:1024 ff_w.T flat
# smalls [6*512] f32: lnw, lnb, ffb, maskb(1024), hcoef w0/w1 interleaved (32)


def build_program(trivial_ln: bool):
    """Build the SPMD bass program (identical on all 8 cores)."""
    nc = bacc.Bacc("TRN2", target_bir_lowering=False, debug=False, num_devices=N_CORES)

    t = {}
    t["h_blob"] = nc.dram_tensor("blob", [BLOB_ROWS, 1024], BF16, kind="ExternalInput")
    t["h_qblob"] = nc.dram_tensor("qblob", [QBLOB_ROWS, 1024], U8, kind="ExternalInput")
    t["h_smalls"] = nc.dram_tensor("smalls", [6 * 512], F32, kind="ExternalInput")
    # collectives may not read IO tensors: bounce the blob slices to internal
    t["d_xp"] = nc.dram_tensor("xp", [128, N], BF16).ap()
    t["d_wp"] = nc.dram_tensor("wp", [128, 1024], BF16).ap()
    t["d_xg"] = nc.dram_tensor("xg", [512, N], BF16).ap()
    t["h_wg"] = nc.dram_tensor("wg", [1024, 1024], BF16, addr_space="Shared")
    t["d_out0"] = nc.dram_tensor("out0", [128, D], BF16, kind="ExternalOutput").ap()
    t["d_out1"] = nc.dram_tensor("out1", [128, D], BF16, kind="ExternalOutput").ap()

    with tile.TileContext(nc) as tc:
        _emit(nc, tc, t, trivial_ln)
    nc.compile()
    return nc


def _emit(nc, tc, t, trivial_ln):
    AL = mybir.AluOpType
    AF = mybir.ActivationFunctionType
    from contextlib import ExitStack

    blob = t["h_blob"]
    sm = t["h_smalls"]
    wg = t["h_wg"]

    def blob_ap(row0, shape3):
        """AP over blob rows: [128, k, cols] with partition-major packing."""
        _, k, cols = shape3
        return bass.AP(tensor=blob, offset=row0 * 1024,
                       ap=[[cols, 128], [128 * cols, k], [1, cols]])

    def wg_ap(off, k, cols):
        return bass.AP(tensor=wg, offset=off,
                       ap=[[cols, 128], [128 * cols, k], [1, cols]])

    ctx = ExitStack()
    with ctx:
        consts = ctx.enter_context(tc.tile_pool(name="consts", bufs=1))
        big = ctx.enter_context(tc.tile_pool(name="big", bufs=1))
        stream = ctx.enter_context(tc.tile_pool(name="stream", bufs=6))
        tiny = ctx.enter_context(tc.tile_pool(name="tiny", bufs=8))
        ppool = ctx.enter_context(tc.tile_pool(name="ppool", bufs=6))
        ps_mm = ctx.enter_context(tc.tile_pool(name="ps_mm", bufs=4, space="PSUM"))
        ps_sc = ps_mm
        ps_av = ctx.enter_context(tc.tile_pool(name="ps_av", bufs=4, space="PSUM"))
        ps_rs = ps_av

        # ---------------- collectives: gather x[b].T and the weight pack ------
        nc.sync.dma_start(
            out=t["d_xp"],
            in_=bass.AP(tensor=blob, offset=R_XP * 1024, ap=[[1024, 128], [1, 1024]]),
        )
        nc.sync.dma_start(
            out=t["d_wp"],
            in_=bass.AP(tensor=blob, offset=R_W * 1024, ap=[[1024, 128], [1, 1024]]),
        )
        nc.gpsimd.collective_compute(
            "AllGather", AL.bypass, replica_groups=XG_GROUPS,
            ins=[t["d_xp"]], outs=[t["d_xg"]],
        )
        nc.gpsimd.collective_compute(
            "AllGather", AL.bypass, replica_groups=WG_GROUPS,
            ins=[t["d_wp"]],
            outs=[bass.AP(tensor=wg, offset=0, ap=[[1024, 1024], [1, 1024]])],
        )

        # ---------------- constants / small tiles ----------------
        identity_f = consts.tile([128, 128], F32)  # f32 transposes (recip path)
        make_identity(nc, identity_f[:])
        identity_b = consts.tile([128, 128], BF16)  # bf16 transposes (loads)
        nc.vector.tensor_copy(identity_b[:], identity_f[:])
        ind4 = consts.tile([4, 128], F32)  # ind4[k, m] = (m//32 == k)
        nc.gpsimd.memset(ind4[:], 1.0)
        nc.gpsimd.affine_select(
            out=ind4[:], in_=ind4[:], compare_op=AL.is_ge, fill=0.0,
            base=0, pattern=[[1, 128]], channel_multiplier=-32,
        )
        nc.gpsimd.affine_select(
            out=ind4[:], in_=ind4[:], compare_op=AL.is_ge, fill=0.0,
            base=31, pattern=[[-1, 128]], channel_multiplier=32,
        )
        ones_col = consts.tile([128, 1], BF16)
        nc.gpsimd.memset(ones_col[:], 1.0)
        ones_row_f = consts.tile([1, 128], F32)
        nc.gpsimd.memset(ones_row_f[:], 1.0)
        ones_row_b = consts.tile([1, 128], BF16)
        nc.vector.tensor_copy(ones_row_b[:], ones_row_f[:])
        zeros_row_b = consts.tile([1, 512], BF16)
        nc.gpsimd.memset(zeros_row_b[:], 0.0)

        maskb = consts.tile([128, NJT], F32)
        nc.gpsimd.dma_start(
            out=maskb[:],
            in_=bass.AP(tensor=sm, offset=3 * 512, ap=[[1, 128], [128, NJT]]),
        )
        hbc = []  # w0, w1 broadcast [128, H]
        for c in range(2):
            bc = consts.tile([128, H], F32, tag=f"hbc{c}")
            nc.gpsimd.dma_start(
                out=bc[:], in_=bass.AP(tensor=sm, offset=5 * 512 + c, ap=[[0, 128], [2, H]])
            )
            hbc.append(bc)

        # per-head scaled identities for the angle-feature PSUM adds
        idw = []  # idw[c][hl] = identity * w_bias[head, c]
        for c, wbc in enumerate(hbc):
            row = []
            for hl in range(H):
                it_ = consts.tile([128, 128], BF16, tag=f"idw{c}_{hl}")
                nc.vector.tensor_scalar(
                    it_[:], identity_b[:], wbc[:, hl : hl + 1], None, AL.mult
                )
                row.append(it_)
            idw.append(row)

        ffb_f = consts.tile([1, D], F32)
        nc.gpsimd.dma_start(
            out=ffb_f[:], in_=bass.AP(tensor=sm, offset=2 * 512, ap=[[0, 1], [1, D]])
        )
        ffb_row = consts.tile([1, D], BF16)
        nc.vector.tensor_copy(ffb_row[:], ffb_f[:])

        lnw_bc = lnb_bc = None
        if not trivial_ln:
            lnw_row = consts.tile([1, D], F32)
            nc.gpsimd.dma_start(
                out=lnw_row[:], in_=bass.AP(tensor=sm, offset=0, ap=[[0, 1], [1, D]])
            )
            lnb_row = consts.tile([1, D], F32)
            nc.gpsimd.dma_start(
                out=lnb_row[:], in_=bass.AP(tensor=sm, offset=512, ap=[[0, 1], [1, D]])
            )
            lnw_bc = consts.tile([128, D], F32)
            lnb_bc = consts.tile([128, D], F32)
            for row, bc in ((lnw_row, lnw_bc), (lnb_row, lnb_bc)):
                ps = ps_mm.tile([128, D], F32, tag="mmps")
                nc.tensor.matmul(ps[:], ones_row_f[0:1, :], row[0:1, :], start=True, stop=True)
                nc.vector.tensor_copy(bc[:], ps[:])

        # ---------------- load big bf16 inputs ----------------
        xq_t = big.tile([128, 4, NI], BF16)      # x[b,irows].T  [d-part, dc, i]
        nc.sync.dma_start(out=xq_t[:], in_=blob_ap(R_XQ, [128, 4, NI]))
        xg_t = big.tile([128, 4, N], BF16)       # gathered x[b].T [d-part, dc, n]
        nc.sync.dma_start(out=xg_t[:], in_=t["d_xg"].rearrange("(c p) n -> p c n", p=128))
        wqk_t = big.tile([128, 4, 1024], BF16)   # att_w.T[:, 0:1024]
        nc.sync.dma_start(out=wqk_t[:], in_=wg_ap(0, 4, 1024))
        wv_t = big.tile([128, 4, 512], BF16)     # att_w.T[:, 1024:1536]
        nc.sync.dma_start(out=wv_t[:], in_=wg_ap(512 * 1024, 4, 512))
        ffw_t = big.tile([128, 4, 512], BF16)    # ff_w.T
        nc.sync.dma_start(out=ffw_t[:], in_=wg_ap(768 * 1024, 4, 512))

        # ---------------- q/k projection (transposed: [feat, n]) ----------------
        qT = big.tile([128, 4, NI], BF16)   # [dh-part(4h), ft, i]
        for ft in range(4):
            ps = ps_mm.tile([128, NI], F32, tag="mmps")
            for dc in range(4):
                nc.tensor.matmul(
                    ps[:], wqk_t[:, dc, ft * 128 : (ft + 1) * 128], xq_t[:, dc, :],
                    start=(dc == 0), stop=(dc == 3),
                )
            nc.vector.tensor_scalar(qT[:, ft, :], ps[:], QSCALE, None, AL.mult)
        kT = big.tile([128, 4, N], BF16)    # [dh-part(4h), ft, n]
        for ft in range(4):
            for nc_i in range(2):
                ps = ps_mm.tile([128, 512], F32, tag="mmps")
                for dc in range(4):
                    nc.tensor.matmul(
                        ps[:], wqk_t[:, dc, 512 + ft * 128 : 512 + (ft + 1) * 128],
                        xg_t[:, dc, nc_i * 512 : nc_i * 512 + 512],
                        start=(dc == 0), stop=(dc == 3),
                    )
                nc.vector.tensor_copy(kT[:, ft, nc_i * 512 : nc_i * 512 + 512], ps[:])

        # ---------------- v projection (natural: [n, feat]) ----------------
        v = big.tile([128, NJT, 512], BF16)  # [j-part, jt, 16h*32]
        for nt in range(NJT):
            ps = ps_mm.tile([128, 512], F32, tag="mmps")
            for dc in range(4):
                nc.tensor.matmul(
                    ps[:], xg_t[:, dc, nt * 128 : (nt + 1) * 128], wv_t[:, dc, :],
                    start=(dc == 0), stop=(dc == 3),
                )
            nc.scalar.copy(v[:, nt, :], ps[:])

        # ---------------- x rows for the residual: transpose xq_t -------------
        xrows_t = big.tile([128, 2, D], BF16)  # [i-part, it, d]
        for it in range(2):
            ps = ps_mm.tile([128, D], F32, tag="mmps")
            for dc in range(4):
                nc.tensor.matmul(
                    ps[:, dc * 128 : (dc + 1) * 128],
                    xq_t[:, dc, it * 128 : (it + 1) * 128],
                    identity_b[:],
                    start=True, stop=True, skip_group_check=True,
                )
            nc.scalar.copy(xrows_t[:, it, :], ps[:])

        # ------- bias features: load u8, dequant, transpose to [j, i] on the PE --
        nat_ctx = ExitStack()
        nat = nat_ctx.enter_context(tc.tile_pool(name="nat", bufs=1))
        qb = t["h_qblob"]
        nats = []
        for qi, row0 in enumerate((Q_P0, Q_A0, Q_A1)):
            qt = nat.tile([128, 2, N], U8, tag=f"q{qi}")
            nc.sync.dma_start(
                out=qt[:],
                in_=bass.AP(tensor=qb, offset=row0 * 1024,
                            ap=[[1024, 128], [128 * 1024, 2], [1, 1024]]),
            )
            lo = tiny.tile([128, 1], F32, tag=f"qlo{qi}")
            nc.gpsimd.dma_start(
                out=lo[:],
                in_=bass.AP(tensor=sm, offset=2592 + 2 * qi, ap=[[0, 128], [1, 1]]),
            )
            sc_ = tiny.tile([128, 1], F32, tag=f"qsc{qi}")
            nc.gpsimd.dma_start(
                out=sc_[:],
                in_=bass.AP(tensor=sm, offset=2593 + 2 * qi, ap=[[0, 128], [1, 1]]),
            )
            natt = nat.tile([128, 2, N], BF16, tag=f"n{qi}")
            nc.vector.tensor_scalar(natt[:], qt[:], sc_[:], lo[:], AL.mult, AL.add)
            nats.append(natt)
        p0nat, a0nat, a1nat = nats

        P0 = big.tile([128, NJT, NI], BF16)
        a0 = big.tile([128, NJT, NI], BF16)
        a1 = big.tile([128, NJT, NI], BF16)
        for natt, dst in ((p0nat, P0), (a0nat, a0), (a1nat, a1)):
            for jt in range(NJT):
                ps = ps_mm.tile([128, NI], F32, tag="mmps")
                for it in range(2):
                    nc.tensor.matmul(
                        ps[:, it * 128 : (it + 1) * 128],
                        natt[:, it, jt * 128 : (jt + 1) * 128],
                        identity_b[:],
                        start=True, stop=True, skip_group_check=True,
                    )
                nc.scalar.copy(dst[:, jt, :], ps[:])
        nat_ctx.close()

        # ---------------- attention: 4 waves of 4 heads ----------------
        attn = big.tile([128, 4, NI], BF16)  # normalized att.T  [4h*32dh, wave, i]
        for w in range(4):
            av_ps = ps_av.tile([128, NI], F32, tag="avps")
            rs_ps = ps_rs.tile([128, 8], F32, tag="avps")
            # zero-init accumulator banks (see module docstring)
            nc.tensor.matmul(
                av_ps[:], ones_row_b[0:1, :], zeros_row_b[0:1, 0:NI],
                start=True, stop=False, skip_group_check=True,
            )
            nc.tensor.matmul(
                rs_ps[:], ones_row_b[0:1, :], zeros_row_b[0:1, 0:8],
                start=True, stop=False, skip_group_check=True,
            )
            for jt in range(NJT):
                p_tiles = []
                for hh in range(4):
                    hl = w * 4 + hh
                    sc = ps_sc.tile([128, NI], F32, tag="mmps")
                    nc.tensor.matmul(
                        sc[:],
                        kT[hh * 32 : (hh + 1) * 32, w, jt * 128 : (jt + 1) * 128],
                        qT[hh * 32 : (hh + 1) * 32, w, :],
                        start=True, stop=False, tile_position=(hh * 32, 0),
                    )
                    nc.tensor.matmul(
                        sc[:], idw[0][hl][:], a0[:, jt, :], start=False, stop=False,
                    )
                    nc.tensor.matmul(
                        sc[:], idw[1][hl][:], a1[:, jt, :], start=False, stop=True,
                    )
                    # P0 add on the DVE, fused with the PSUM evacuation the
                    # exp would otherwise need.
                    xs = stream.tile([128, NI], F32, tag="xs")
                    nc.vector.scalar_tensor_tensor(
                        xs[:], P0[:, jt, :], 1.0, sc[:], AL.mult, AL.add
                    )
                    pT = ppool.tile([128, NI], BF16, tag="pT")
                    nc.scalar.activation(
                        pT[:], xs[:], AF.Exp, bias=maskb[:, jt : jt + 1], scale=1.0
                    )
                    p_tiles.append(pT)
                for hh in range(4):
                    pT = p_tiles[hh]
                    vcol = (w * 4 + hh) * 32
                    nc.tensor.matmul(
                        av_ps[hh * 32 : (hh + 1) * 32, :],
                        v[:, jt, vcol : vcol + 32],
                        pT[:],
                        start=False, stop=(jt == NJT - 1 and hh == 3),
                        tile_position=(0, hh * 32),
                        skip_group_check=True,
                    )
                    for ic in range(2):
                        col = ic * 4 + hh
                        nc.tensor.matmul(
                            rs_ps[:, col : col + 1],
                            pT[:, ic * 128 : (ic + 1) * 128],
                            ones_col[:],
                            start=False,
                            stop=(jt == NJT - 1 and hh == 3 and ic == 1),
                            skip_group_check=True,
                        )
            # normalize: attn = av / rowsum
            rs_sb = stream.tile([128, 8], F32, tag="t512")
            nc.vector.tensor_copy(rs_sb[:], rs_ps[:])
            recip = stream.tile([128, 8], F32, tag="t512")
            nc.vector.reciprocal(recip[:], rs_sb[:])
            recipT = stream.tile([4, NI], F32, tag="t512")
            for ic in range(2):
                trp = ps_mm.tile([4, 128], F32, tag="mmps")
                nc.tensor.transpose(trp[:], recip[:, ic * 4 : (ic + 1) * 4], identity_f[:])
                nc.vector.tensor_copy(recipT[:, ic * 128 : (ic + 1) * 128], trp[:])
            rbc_ps = ps_mm.tile([128, NI], F32, tag="mmps")
            nc.tensor.matmul(rbc_ps[:], ind4[:], recipT[:], start=True, stop=True)
            rbc = stream.tile([128, NI], F32, tag="t512")
            nc.vector.tensor_copy(rbc[:], rbc_ps[:])
            nc.vector.scalar_tensor_tensor(
                attn[:, w, :], rbc[:], 1.0, av_ps[:], AL.mult, AL.mult
            )

        # -------- FF projection + ff_b + residual + LayerNorm, direct out -------
        for it in range(2):
            ps = ps_mm.tile([128, D], F32, tag="mmps")
            for w in range(4):
                nc.tensor.matmul(
                    ps[:],
                    attn[:, w, it * 128 : (it + 1) * 128],
                    ffw_t[:, w, :],
                    start=(w == 0), stop=False,
                )
            nc.tensor.matmul(
                ps[:], ones_row_b[0:1, :], ffb_row[0:1, :], start=False, stop=True
            )
            x_ld = stream.tile([128, D], F32, tag="t512")
            nc.scalar.copy(x_ld[:], xrows_t[:, it, :])
            y = stream.tile([128, D], F32, tag="t512")
            ysum = tiny.tile([128, 1], F32, tag="t1")
            nc.vector.scalar_tensor_tensor(
                y[:], x_ld[:], 1.0, ps[:], AL.mult, AL.add, accum_out=ysum[:],
            )
            negmu = tiny.tile([128, 1], F32, tag="t1")
            nc.vector.tensor_scalar(negmu[:], ysum[:], -1.0 / D, None, AL.mult)
            sq = stream.tile([128, D], F32, tag="t512")
            ssq = tiny.tile([128, 1], F32, tag="t1")
            nc.scalar.activation(
                sq[:], y[:], AF.Square, bias=negmu[:], scale=1.0, accum_out=ssq[:]
            )
            veps = tiny.tile([128, 1], F32, tag="t1")
            nc.vector.tensor_scalar(veps[:], ssq[:], 1.0 / D, LN_EPS, AL.mult, AL.add)
            std = tiny.tile([128, 1], F32, tag="t1")
            nc.scalar.activation(std[:], veps[:], AF.Sqrt)
            rstd = tiny.tile([128, 1], F32, tag="t1")
            nc.vector.reciprocal(rstd[:], std[:])
            if trivial_ln:
                o = stream.tile([128, D], BF16, tag="to")
                nc.vector.tensor_scalar(o[:], y[:], negmu[:], rstd[:], AL.add, AL.mult)
            else:
                z = stream.tile([128, D], F32, tag="t512")
                nc.vector.tensor_scalar(z[:], y[:], negmu[:], rstd[:], AL.add, AL.mult)
                zw = stream.tile([128, D], F32, tag="t512")
                nc.vector.scalar_tensor_tensor(zw[:], lnw_bc[:], 1.0, z[:], AL.mult, AL.mult)
                o = stream.tile([128, D], BF16, tag="to")
                nc.vector.scalar_tensor_tensor(o[:], lnb_bc[:], 1.0, zw[:], AL.mult, AL.add)
            nc.sync.dma_start(out=t[f"d_out{it}"], in_=o[:])


# ---------------------------------------------------------------------------
# Host side: program cache, cached PJRT runner, shard prep
# ---------------------------------------------------------------------------

_PROGRAM_CACHE = {}
_RUNNER_CACHE = {}
from concurrent.futures import ThreadPoolExecutor as _TPE

_PREP_POOL = _TPE(max_workers=8)


def _get_program(trivial_ln):
    key = (bool(trivial_ln),)
    if key not in _PROGRAM_CACHE:
        _PROGRAM_CACHE[key] = build_program(bool(trivial_ln))
    return _PROGRAM_CACHE[key]


def _get_runner(nc):
    """Build (once) a persistent jitted sharded callable for `nc`.

    Mirrors concourse.bass2jax.run_bass_via_pjrt (the axon execution path of
    bass_utils.run_bass_kernel_spmd) but hoists the jax.jit out of the
    per-call path and assembles the global arrays without an extra concat.
    """
    key = id(nc)
    if key in _RUNNER_CACHE:
        return _RUNNER_CACHE[key]

    import jax
    from jax.sharding import Mesh, PartitionSpec
    from jax.experimental.shard_map import shard_map
    from concourse.bass2jax import (_bass_exec_p, install_neuronx_cc_hook,
                                    partition_id_tensor)

    install_neuronx_cc_hook()
    assert nc.dbg_addr is None or not nc.dbg_callbacks

    partition_name = nc.partition_id_tensor.name if nc.partition_id_tensor else None
    in_names, out_names, out_avals = [], [], []
    for alloc in nc.m.functions[0].allocations:
        if not isinstance(alloc, mybir.MemoryLocationSet):
            continue
        name = alloc.memorylocations[0].name
        if alloc.kind == "ExternalInput":
            if name != partition_name:
                in_names.append(name)
        elif alloc.kind == "ExternalOutput":
            out_names.append(name)
            out_avals.append(jax.core.ShapedArray(
                tuple(alloc.tensor_shape), mybir.dt.np(alloc.dtype)))
    n_params = len(in_names)
    n_outs = len(out_avals)
    all_in_names = list(in_names) + out_names
    if partition_name is not None:
        all_in_names.append(partition_name)
    donate = tuple(range(n_params, n_params + n_outs))

    def _body(*args):
        operands = list(args)
        if partition_name is not None:
            operands.append(partition_id_tensor())
        outs = _bass_exec_p.bind(
            *operands, out_avals=tuple(out_avals), in_names=tuple(all_in_names),
            out_names=tuple(out_names), lowering_input_output_aliases=(),
            sim_require_finite=True, sim_require_nnan=True, nc=nc)
        return tuple(outs)

    devices = jax.devices()[:N_CORES]
    mesh = Mesh(np.asarray(devices), ("core",))
    in_specs = (PartitionSpec("core"),) * (n_params + n_outs)
    out_specs = (PartitionSpec("core"),) * n_outs
    sharded = jax.jit(
        shard_map(_body, mesh=mesh, in_specs=in_specs, out_specs=out_specs,
                  check_rep=False),
        donate_argnums=donate, keep_unused=True)

    zero_shapes = [(N_CORES * a.shape[0], *a.shape[1:]) for a in out_avals]
    zero_dtypes = [a.dtype for a in out_avals]
    from concurrent.futures import ThreadPoolExecutor
    fetch_pool = ThreadPoolExecutor(max_workers=max(len(out_names), 1))

    def run(globals_by_name):
        concat_in = [globals_by_name[name] for name in in_names]
        concat_zeros = [np.zeros(s, d) for s, d in zip(zero_shapes, zero_dtypes)]
        out_arrs = sharded(*concat_in, *concat_zeros)
        futs = [fetch_pool.submit(np.asarray, o) for o in out_arrs]
        return {
            name: futs[i].result().reshape(N_CORES, *out_avals[i].shape)
            for i, name in enumerate(out_names)
        }

    _RUNNER_CACHE[key] = run
    return run


def _shard_globals(x, pdist, angle, adj, mask, gp, ga, w_bias,
                   att_w, ff_w, ff_b, ln_w, ln_b):
    """Build the concatenated global input arrays (blob + qblob + smalls)."""
    blob = np.empty((N_CORES * BLOB_ROWS, 1024), BF)
    qblob = np.empty((N_CORES * QBLOB_ROWS, 1024), np.uint8)
    smalls = np.zeros((N_CORES * 6 * 512,), np.float32)

    def build_W():
        awT = att_w.T.astype(BF)                   # [D, 1536]
        W = np.empty((1024, 1024), BF)
        W[0:512] = awT[:, 0:1024]
        W[512:768] = awT[:, 1024:1536].reshape(256, 1024)
        W[768:1024] = ff_w.T.astype(BF).reshape(256, 1024)
        return W

    fW = _PREP_POOL.submit(build_W)
    fxT = [_PREP_POOL.submit(lambda bb: x[bb].T.astype(BF), b) for b in range(B)]
    maskf = [np.where(mask[b, 0, 0, :], np.float32(NEG_INF), np.float32(0.0))
             for b in range(B)]
    simple_g = gp == 1.0 and ga == 1.0

    def fill_core(c):
        b, ih = c // 4, c % 4
        i0 = ih * NI
        irows = slice(i0, i0 + NI)
        bl = blob[c * BLOB_ROWS : (c + 1) * BLOB_ROWS]
        qb = qblob[c * QBLOB_ROWS : (c + 1) * QBLOB_ROWS]
        s = smalls[c * 6 * 512 : (c + 1) * 6 * 512]

        if simple_g:
            p0c = adj[b, irows] - pdist[b, irows]
        else:
            p0c = np.float32(ga) * adj[b, irows] - np.float32(gp) * pdist[b, irows]
        for qi, (row0, src) in enumerate(((Q_P0, p0c),
                                          (Q_A0, angle[b, irows, :, 0]),
                                          (Q_A1, angle[b, irows, :, 1]))):
            lo = float(src.min())
            hi = float(src.max())
            sc = (hi - lo) / 255.0 if hi > lo else 1.0
            qb[row0 : row0 + NI] = (src - lo) * (1.0 / sc) + 0.5
            s[2592 + 2 * qi] = lo
            s[2593 + 2 * qi] = sc
        xT_b = fxT[b].result()
        bl[R_XQ : R_XQ + 128] = xT_b[:, irows].reshape(128, 1024)
        bl[R_XP : R_XP + 128] = xT_b[ih * 128 : (ih + 1) * 128]
        bl[R_W : R_W + 128] = fW.result()[c * 128 : (c + 1) * 128]

        s[0:512] = ln_w
        s[512:1024] = ln_b
        s[1024:1536] = ff_b
        s[1536:2560] = maskf[b]
        s[2560 : 2560 + 2 * H] = w_bias[:, 0:2].reshape(-1)

    # numpy cast/copy loops release the GIL; parallelize the per-core fill
    futs = [_PREP_POOL.submit(fill_core, c) for c in range(N_CORES)]
    for f in futs:
        f.result()
    return {"blob": blob, "qblob": qblob, "smalls": smalls}


def _reference_numpy(x, pdist, angle, adj, mask, gamma_p, gamma_adj, w_bias,
                     att_w, ff_w, ff_b, ln_w, ln_b):
    """Exact fallback (used only for non-head-uniform gammas)."""
    f8 = np.float64
    x64 = x.astype(f8)
    qkv = x64 @ att_w.astype(f8).T
    wq, wk, wv = np.split(qkv, 3, axis=-1)
    bsz, n = x.shape[0], x.shape[1]
    wq = wq.reshape(bsz, n, H, DH)
    wk = wk.reshape(bsz, n, H, DH)
    wv = wv.reshape(bsz, n, H, DH)
    score = np.einsum('bihd,bjhd->bhij', wq, wk, optimize=True) / np.sqrt(f8(DH))
    score = score - gamma_p.astype(f8)[None, :, None, None] * pdist.astype(f8)[:, None]
    score = score + np.einsum('bijc,hc->bhij', angle.astype(f8), w_bias.astype(f8),
                              optimize=True)
    score = score + gamma_adj.astype(f8)[None, :, None, None] * adj.astype(f8)[:, None]
    score = np.where(mask, NEG_INF, score)
    score -= score.max(-1, keepdims=True)
    p = np.exp(score)
    p /= p.sum(-1, keepdims=True)
    att = np.einsum('bhij,bjhd->bihd', p, wv, optimize=True).reshape(bsz, n, H * DH)
    y = x64 + att @ ff_w.astype(f8).T + ff_b.astype(f8)
    mu = y.mean(-1, keepdims=True)
    var = np.square(y - mu).mean(-1, keepdims=True)
    out = (y - mu) / np.sqrt(var + LN_EPS) * ln_w.astype(f8) + ln_b.astype(f8)
    return out.astype(np.float32)


def kernel(x, pdist, angle, adj, mask, gamma_p, gamma_adj, w_bias,
           att_w, ff_w, ff_b, ln_w, ln_b, **_unused):
    x = np.asarray(x, dtype=np.float32)
    pdist = np.asarray(pdist, dtype=np.float32)
    angle = np.asarray(angle, dtype=np.float32)
    adj = np.asarray(adj, dtype=np.float32)
    mask = np.asarray(mask)
    gamma_p = np.asarray(gamma_p, dtype=np.float32)
    gamma_adj = np.asarray(gamma_adj, dtype=np.float32)
    w_bias = np.asarray(w_bias, dtype=np.float32)
    att_w = np.asarray(att_w, dtype=np.float32)
    ff_w = np.asarray(ff_w, dtype=np.float32)
    ff_b = np.asarray(ff_b, dtype=np.float32)
    ln_w = np.asarray(ln_w, dtype=np.float32)
    ln_b = np.asarray(ln_b, dtype=np.float32)

    uniform = bool(
        np.all(gamma_p == gamma_p.flat[0]) and np.all(gamma_adj == gamma_adj.flat[0])
    )
    if not uniform:
        return _reference_numpy(x, pdist, angle, adj, mask, gamma_p, gamma_adj,
                                w_bias, att_w, ff_w, ff_b, ln_w, ln_b)
    gp = float(gamma_p.flat[0])
    ga = float(gamma_adj.flat[0])

    trivial_ln = bool(np.all(ln_w == 1.0) and np.all(ln_b == 0.0))
    nc = _get_program(trivial_ln)
    run = _get_runner(nc)
    g = _shard_globals(x, pdist, angle, adj, mask, gp, ga, w_bias,
                       att_w, ff_w, ff_b, ln_w, ln_b)
    res = run(g)  # out0/out1: [8, 128, D] bf16

    out = np.empty((B, N, D), dtype=np.float32)
    for c in range(N_CORES):
        b, ih = c // 4, c % 4
        i0 = ih * NI
        out[b, i0 : i0 + 128, :] = res["out0"][c]
        out[b, i0 + 128 : i0 + 256, :] = res["out1"][c]
    return out


# revision 39
# speedup vs baseline: 2.6321x; 1.0499x over previous
"""Trainium2 Bass kernel for nn_MultiHeadAttention_52862457480066.

Reference computation (B=2, N=1024, D=512, H=16, DH=32):
    qkv = x @ att_w.T ; q,k,v per head
    score = q.k/sqrt(DH) - gamma_p*pdist + angle@w_bias.T + gamma_adj*adj
    score = where(mask, -1e9, score) ; prob = softmax_j(score)
    att = prob @ v ; ff = att @ ff_w.T + ff_b ; y = x + ff ; out = LayerNorm(y)*ln_w+ln_b

Sharding over 8 cores: (batch b in 2) x (query-quarter ih in 4). Each core owns
ALL 16 heads for its 256 query rows, so its FF output rows are complete and no
cross-core reduction of activations is needed.

End-to-end wall time is dominated by host->device transfer over the axon
tunnel (~115-170 MB/s, ~85 ms fixed, ~6 ms per extra jit arg), so the design
minimizes uploaded bytes and arg count:
- All large tensors ship as ONE bf16 blob arg per core (bias slices, x.T
  slices, weight shard); bf16 halves bytes against a 2e-2 rel-err budget.
- pdist and adj only appear as P0 = gamma_adj*adj - gamma_p*pdist when the
  gammas are head-uniform (they are for this module's inputs), so the host
  combines them into ONE tensor. Non-uniform gammas fall back to exact numpy.
- Bias slices are per-core-unique; with the all-heads sharding nothing is
  uploaded twice. x[b].T (needed in full for K/V) is uploaded as per-core
  quarters and AllGathered on-device within each batch's 4-core group; the
  weights are uploaded as 1/8 shards and AllGathered across all 8 cores.
- Bias tensors upload in natural [i,j] layout (contiguous host slices) and
  are transposed to the [j,i] score layout on-device by the PE, which has
  large headroom. x rows for the residual are likewise recovered on-device by
  transposing the uploaded x[b,irows].T slice.
- Scores are computed TRANSPOSED ([j_part, i_free]) so softmax'd probs feed
  the attention*V matmul directly as the moving operand. All score-bias terms
  enter via PE identity matmuls (angle features) or a DVE add fused with the
  PSUM evacuation (P0), so the hot softmax path is one DVE + one ACT pass.
- Softmax denominators come from N=1 matmuls (ones moving operand) giving
  rowsums in [i_part, head_free] layout; normalization is deferred to after
  the AV matmul (divides 16*256 values per core instead of 4.2M).
- PSUM accumulators written by interleaved matmul chains are zero-initialized
  by one full-coverage start=True matmul; everything after runs start=False.
- The jitted PJRT executable is built once and cached; per-call work is host
  slicing/casting, one sharded transfer, execution, and two parallel bf16
  fetches.
"""

import math

import numpy as np

import concourse.bass as bass
import concourse.tile as tile
from concourse import bacc, mybir
from concourse.masks import make_identity

B, N, D, H, DH = 2, 1024, 512, 16, 32
NI = 256             # query rows per core
NJT = N // 128       # key tiles (partition dim j)
NEG_INF = -1e9
LN_EPS = 1e-5
QSCALE = 1.0 / math.sqrt(DH)
F32 = mybir.dt.float32
BF16 = mybir.dt.bfloat16
N_CORES = 8
XG_GROUPS = [[0, 1, 2, 3], [4, 5, 6, 7]]   # x[b].T AllGather within batch
WG_GROUPS = [[0, 1, 2, 3, 4, 5, 6, 7]]     # weight AllGather across all cores

BF = np.dtype(mybir.dt.np(BF16))  # ml_dtypes.bfloat16

# blob row ranges (per core, [384, 1024] bf16)
R_XQ = 0                             # x[b,irows].T packed [512,256]->[128,1024]
R_XP = 128                           # x[b].T rows [ih*128,(ih+1)*128) for AllGather
R_W = 256                            # weight-pack shard W[c*128:(c+1)*128]
BLOB_ROWS = 384
# qblob row ranges (per core, [768, 1024] uint8): per-slice-quantized biases,
# dequantized on-device as q*scale+lo with scale/lo shipped in smalls
Q_P0, Q_A0, Q_A1 = 0, 256, 512
QBLOB_ROWS = 768
U8 = mybir.dt.uint8
# weight pack W [1024, 1024] bf16 (same on all cores before sharding):
#   rows 0:512   att_w.T[:, 0:1024]            (q feats 0:512 | k feats 512:1024)
#   rows 512:768 att_w.T[:, 1024:1536] flat    (v)
#   rows 768:1024 ff_w.T flat
# smalls [6*512] f32: lnw, lnb, ffb, maskb(1024), hcoef w0/w1 interleaved (32)


def build_program(trivial_ln: bool):
    """Build the SPMD bass program (identical on all 8 cores)."""
    nc = bacc.Bacc("TRN2", target_bir_lowering=False, debug=False, num_devices=N_CORES)

    t = {}
    t["h_blob"] = nc.dram_tensor("blob", [BLOB_ROWS, 1024], BF16, kind="ExternalInput")
    t["h_qblob"] = nc.dram_tensor("qblob", [QBLOB_ROWS, 1024], U8, kind="ExternalInput")
    t["h_smalls"] = nc.dram_tensor("smalls", [6 * 512], F32, kind="ExternalInput")
    # collectives may not read IO tensors: bounce the blob slices to internal
    t["d_xp"] = nc.dram_tensor("xp", [128, N], BF16).ap()
    t["d_wp"] = nc.dram_tensor("wp", [128, 1024], BF16).ap()
    t["d_xg"] = nc.dram_tensor("xg", [512, N], BF16).ap()
    t["h_wg"] = nc.dram_tensor("wg", [1024, 1024], BF16, addr_space="Shared")
    t["d_out0"] = nc.dram_tensor("out0", [128, D], BF16, kind="ExternalOutput").ap()
    t["d_out1"] = nc.dram_tensor("out1", [128, D], BF16, kind="ExternalOutput").ap()

    with tile.TileContext(nc) as tc:
        _emit(nc, tc, t, trivial_ln)
    nc.compile()
    return nc


def _emit(nc, tc, t, trivial_ln):
    AL = mybir.AluOpType
    AF = mybir.ActivationFunctionType
    from contextlib import ExitStack

    blob = t["h_blob"]
    sm = t["h_smalls"]
    wg = t["h_wg"]

    def blob_ap(row0, shape3):
        """AP over blob rows: [128, k, cols] with partition-major packing."""
        _, k, cols = shape3
        return bass.AP(tensor=blob, offset=row0 * 1024,
                       ap=[[cols, 128], [128 * cols, k], [1, cols]])

    def wg_ap(off, k, cols):
        return bass.AP(tensor=wg, offset=off,
                       ap=[[cols, 128], [128 * cols, k], [1, cols]])

    ctx = ExitStack()
    with ctx:
        consts = ctx.enter_context(tc.tile_pool(name="consts", bufs=1))
        big = ctx.enter_context(tc.tile_pool(name="big", bufs=1))
        stream = ctx.enter_context(tc.tile_pool(name="stream", bufs=6))
        tiny = ctx.enter_context(tc.tile_pool(name="tiny", bufs=8))
        ppool = ctx.enter_context(tc.tile_pool(name="ppool", bufs=6))
        ps_mm = ctx.enter_context(tc.tile_pool(name="ps_mm", bufs=4, space="PSUM"))
        ps_sc = ps_mm
        ps_av = ctx.enter_context(tc.tile_pool(name="ps_av", bufs=4, space="PSUM"))
        ps_rs = ps_av

        # ---------------- collectives: gather x[b].T and the weight pack ------
        nc.sync.dma_start(
            out=t["d_xp"],
            in_=bass.AP(tensor=blob, offset=R_XP * 1024, ap=[[1024, 128], [1, 1024]]),
        )
        nc.sync.dma_start(
            out=t["d_wp"],
            in_=bass.AP(tensor=blob, offset=R_W * 1024, ap=[[1024, 128], [1, 1024]]),
        )
        nc.gpsimd.collective_compute(
            "AllGather", AL.bypass, replica_groups=XG_GROUPS,
            ins=[t["d_xp"]], outs=[t["d_xg"]],
        )
        nc.gpsimd.collective_compute(
            "AllGather", AL.bypass, replica_groups=WG_GROUPS,
            ins=[t["d_wp"]],
            outs=[bass.AP(tensor=wg, offset=0, ap=[[1024, 1024], [1, 1024]])],
        )

        # ---------------- constants / small tiles ----------------
        identity_f = consts.tile([128, 128], F32)  # f32 transposes (recip path)
        make_identity(nc, identity_f[:])
        identity_b = consts.tile([128, 128], BF16)  # bf16 transposes (loads)
        nc.vector.tensor_copy(identity_b[:], identity_f[:])
        ind4 = consts.tile([4, 128], F32)  # ind4[k, m] = (m//32 == k)
        nc.gpsimd.memset(ind4[:], 1.0)
        nc.gpsimd.affine_select(
            out=ind4[:], in_=ind4[:], compare_op=AL.is_ge, fill=0.0,
            base=0, pattern=[[1, 128]], channel_multiplier=-32,
        )
        nc.gpsimd.affine_select(
            out=ind4[:], in_=ind4[:], compare_op=AL.is_ge, fill=0.0,
            base=31, pattern=[[-1, 128]], channel_multiplier=32,
        )
        ones_col = consts.tile([128, 1], BF16)
        nc.gpsimd.memset(ones_col[:], 1.0)
        ones_row_f = consts.tile([1, 128], F32)
        nc.gpsimd.memset(ones_row_f[:], 1.0)
        ones_row_b = consts.tile([1, 128], BF16)
        nc.vector.tensor_copy(ones_row_b[:], ones_row_f[:])
        zeros_row_b = consts.tile([1, 512], BF16)
        nc.gpsimd.memset(zeros_row_b[:], 0.0)

        maskb = consts.tile([128, NJT], F32)
        nc.gpsimd.dma_start(
            out=maskb[:],
            in_=bass.AP(tensor=sm, offset=3 * 512, ap=[[1, 128], [128, NJT]]),
        )
        hbc = []  # w0, w1 broadcast [128, H]
        for c in range(2):
            bc = consts.tile([128, H], F32, tag=f"hbc{c}")
            nc.gpsimd.dma_start(
                out=bc[:], in_=bass.AP(tensor=sm, offset=5 * 512 + c, ap=[[0, 128], [2, H]])
            )
            hbc.append(bc)

        # per-head scaled identities for the angle-feature PSUM adds
        idw = []  # idw[c][hl] = identity * w_bias[head, c]
        for c, wbc in enumerate(hbc):
            row = []
            for hl in range(H):
                it_ = consts.tile([128, 128], BF16, tag=f"idw{c}_{hl}")
                nc.vector.tensor_scalar(
                    it_[:], identity_b[:], wbc[:, hl : hl + 1], None, AL.mult
                )
                row.append(it_)
            idw.append(row)

        ffb_f = consts.tile([1, D], F32)
        nc.gpsimd.dma_start(
            out=ffb_f[:], in_=bass.AP(tensor=sm, offset=2 * 512, ap=[[0, 1], [1, D]])
        )
        ffb_row = consts.tile([1, D], BF16)
        nc.vector.tensor_copy(ffb_row[:], ffb_f[:])

        lnw_bc = lnb_bc = None
        if not trivial_ln:
            lnw_row = consts.tile([1, D], F32)
            nc.gpsimd.dma_start(
                out=lnw_row[:], in_=bass.AP(tensor=sm, offset=0, ap=[[0, 1], [1, D]])
            )
            lnb_row = consts.tile([1, D], F32)
            nc.gpsimd.dma_start(
                out=lnb_row[:], in_=bass.AP(tensor=sm, offset=512, ap=[[0, 1], [1, D]])
            )
            lnw_bc = consts.tile([128, D], F32)
            lnb_bc = consts.tile([128, D], F32)
            for row, bc in ((lnw_row, lnw_bc), (lnb_row, lnb_bc)):
                ps = ps_mm.tile([128, D], F32, tag="mmps")
                nc.tensor.matmul(ps[:], ones_row_f[0:1, :], row[0:1, :], start=True, stop=True)
                nc.vector.tensor_copy(bc[:], ps[:])

        # ---------------- load big bf16 inputs ----------------
        xq_t = big.tile([128, 4, NI], BF16)      # x[b,irows].T  [d-part, dc, i]
        nc.sync.dma_start(out=xq_t[:], in_=blob_ap(R_XQ, [128, 4, NI]))
        xg_t = big.tile([128, 4, N], BF16)       # gathered x[b].T [d-part, dc, n]
        nc.sync.dma_start(out=xg_t[:], in_=t["d_xg"].rearrange("(c p) n -> p c n", p=128))
        wqk_t = big.tile([128, 4, 1024], BF16)   # att_w.T[:, 0:1024]
        nc.sync.dma_start(out=wqk_t[:], in_=wg_ap(0, 4, 1024))
        wv_t = big.tile([128, 4, 512], BF16)     # att_w.T[:, 1024:1536]
        nc.sync.dma_start(out=wv_t[:], in_=wg_ap(512 * 1024, 4, 512))
        ffw_t = big.tile([128, 4, 512], BF16)    # ff_w.T
        nc.sync.dma_start(out=ffw_t[:], in_=wg_ap(768 * 1024, 4, 512))

        # ---------------- q/k projection (transposed: [feat, n]) ----------------
        qT = big.tile([128, 4, NI], BF16)   # [dh-part(4h), ft, i]
        for ft in range(4):
            ps = ps_mm.tile([128, NI], F32, tag="mmps")
            for dc in range(4):
                nc.tensor.matmul(
                    ps[:], wqk_t[:, dc, ft * 128 : (ft + 1) * 128], xq_t[:, dc, :],
                    start=(dc == 0), stop=(dc == 3),
                )
            nc.vector.tensor_scalar(qT[:, ft, :], ps[:], QSCALE, None, AL.mult)
        kT = big.tile([128, 4, N], BF16)    # [dh-part(4h), ft, n]
        for ft in range(4):
            for nc_i in range(2):
                ps = ps_mm.tile([128, 512], F32, tag="mmps")
                for dc in range(4):
                    nc.tensor.matmul(
                        ps[:], wqk_t[:, dc, 512 + ft * 128 : 512 + (ft + 1) * 128],
                        xg_t[:, dc, nc_i * 512 : nc_i * 512 + 512],
                        start=(dc == 0), stop=(dc == 3),
                    )
                nc.vector.tensor_copy(kT[:, ft, nc_i * 512 : nc_i * 512 + 512], ps[:])

        # ---------------- v projection (natural: [n, feat]) ----------------
        v = big.tile([128, NJT, 512], BF16)  # [j-part, jt, 16h*32]
        for nt in range(NJT):
            ps = ps_mm.tile([128, 512], F32, tag="mmps")
            for dc in range(4):
                nc.tensor.matmul(
                    ps[:], xg_t[:, dc, nt * 128 : (nt + 1) * 128], wv_t[:, dc, :],
                    start=(dc == 0), stop=(dc == 3),
                )
            nc.scalar.copy(v[:, nt, :], ps[:])

        # ---------------- x rows for the residual: transpose xq_t -------------
        xrows_t = big.tile([128, 2, D], BF16)  # [i-part, it, d]
        for it in range(2):
            ps = ps_mm.tile([128, D], F32, tag="mmps")
            for dc in range(4):
                nc.tensor.matmul(
                    ps[:, dc * 128 : (dc + 1) * 128],
                    xq_t[:, dc, it * 128 : (it + 1) * 128],
                    identity_b[:],
                    start=True, stop=True, skip_group_check=True,
                )
            nc.scalar.copy(xrows_t[:, it, :], ps[:])

        # ------- bias features: load u8, dequant, transpose to [j, i] on the PE --
        nat_ctx = ExitStack()
        nat = nat_ctx.enter_context(tc.tile_pool(name="nat", bufs=1))
        qb = t["h_qblob"]
        nats = []
        for qi, row0 in enumerate((Q_P0, Q_A0, Q_A1)):
            qt = nat.tile([128, 2, N], U8, tag=f"q{qi}")
            nc.sync.dma_start(
                out=qt[:],
                in_=bass.AP(tensor=qb, offset=row0 * 1024,
                            ap=[[1024, 128], [128 * 1024, 2], [1, 1024]]),
            )
            lo = tiny.tile([128, 1], F32, tag=f"qlo{qi}")
            nc.gpsimd.dma_start(
                out=lo[:],
                in_=bass.AP(tensor=sm, offset=2592 + 2 * qi, ap=[[0, 128], [1, 1]]),
            )
            sc_ = tiny.tile([128, 1], F32, tag=f"qsc{qi}")
            nc.gpsimd.dma_start(
                out=sc_[:],
                in_=bass.AP(tensor=sm, offset=2593 + 2 * qi, ap=[[0, 128], [1, 1]]),
            )
            natt = nat.tile([128, 2, N], BF16, tag=f"n{qi}")
            nc.vector.tensor_scalar(natt[:], qt[:], sc_[:], lo[:], AL.mult, AL.add)
            nats.append(natt)
        p0nat, a0nat, a1nat = nats

        P0 = big.tile([128, NJT, NI], BF16)
        a0 = big.tile([128, NJT, NI], BF16)
        a1 = big.tile([128, NJT, NI], BF16)
        for natt, dst in ((p0nat, P0), (a0nat, a0), (a1nat, a1)):
            for jt in range(NJT):
                ps = ps_mm.tile([128, NI], F32, tag="mmps")
                for it in range(2):
                    nc.tensor.matmul(
                        ps[:, it * 128 : (it + 1) * 128],
                        natt[:, it, jt * 128 : (jt + 1) * 128],
                        identity_b[:],
                        start=True, stop=True, skip_group_check=True,
                    )
                nc.scalar.copy(dst[:, jt, :], ps[:])
        nat_ctx.close()

        # ---------------- attention: 4 waves of 4 heads ----------------
        attn = big.tile([128, 4, NI], BF16)  # normalized att.T  [4h*32dh, wave, i]
        for w in range(4):
            av_ps = ps_av.tile([128, NI], F32, tag="avps")
            rs_ps = ps_rs.tile([128, 8], F32, tag="avps")
            # zero-init accumulator banks (see module docstring)
            nc.tensor.matmul(
                av_ps[:], ones_row_b[0:1, :], zeros_row_b[0:1, 0:NI],
                start=True, stop=False, skip_group_check=True,
            )
            nc.tensor.matmul(
                rs_ps[:], ones_row_b[0:1, :], zeros_row_b[0:1, 0:8],
                start=True, stop=False, skip_group_check=True,
            )
            for jt in range(NJT):
                p_tiles = []
                for hh in range(4):
                    hl = w * 4 + hh
                    sc = ps_sc.tile([128, NI], F32, tag="mmps")
                    nc.tensor.matmul(
                        sc[:],
                        kT[hh * 32 : (hh + 1) * 32, w, jt * 128 : (jt + 1) * 128],
                        qT[hh * 32 : (hh + 1) * 32, w, :],
                        start=True, stop=False, tile_position=(hh * 32, 0),
                    )
                    nc.tensor.matmul(
                        sc[:], idw[0][hl][:], a0[:, jt, :], start=False, stop=False,
                    )
                    nc.tensor.matmul(
                        sc[:], idw[1][hl][:], a1[:, jt, :], start=False, stop=True,
                    )
                    # P0 add on the DVE, fused with the PSUM evacuation the
                    # exp would otherwise need.
                    xs = stream.tile([128, NI], F32, tag="xs")
                    nc.vector.scalar_tensor_tensor(
                        xs[:], P0[:, jt, :], 1.0, sc[:], AL.mult, AL.add
                    )
                    pT = ppool.tile([128, NI], BF16, tag="pT")
                    nc.scalar.activation(
                        pT[:], xs[:], AF.Exp, bias=maskb[:, jt : jt + 1], scale=1.0
                    )
                    p_tiles.append(pT)
                for hh in range(4):
                    pT = p_tiles[hh]
                    vcol = (w * 4 + hh) * 32
                    nc.tensor.matmul(
                        av_ps[hh * 32 : (hh + 1) * 32, :],
                        v[:, jt, vcol : vcol + 32],
                        pT[:],
                        start=False, stop=(jt == NJT - 1 and hh == 3),
                        tile_position=(0, hh * 32),
                        skip_group_check=True,
                    )
                    for ic in range(2):
                        col = ic * 4 + hh
                        nc.tensor.matmul(
                            rs_ps[:, col : col + 1],
                            pT[:, ic * 128 : (ic + 1) * 128],
                            ones_col[:],
                            start=False,
                            stop=(jt == NJT - 1 and hh == 3 and ic == 1),
                            skip_group_check=True,
                        )
            # normalize: attn = av / rowsum
            rs_sb = stream.tile([128, 8], F32, tag="t512")
            nc.vector.tensor_copy(rs_sb[:], rs_ps[:])
            recip = stream.tile([128, 8], F32, tag="t512")
            nc.vector.reciprocal(recip[:], rs_sb[:])
            recipT = stream.tile([4, NI], F32, tag="t512")
            for ic in range(2):
                trp = ps_mm.tile([4, 128], F32, tag="mmps")
                nc.tensor.transpose(trp[:], recip[:, ic * 4 : (ic + 1) * 4], identity_f[:])
                nc.vector.tensor_copy(recipT[:, ic * 128 : (ic + 1) * 128], trp[:])
            rbc_ps = ps_mm.tile([128, NI], F32, tag="mmps")
            nc.tensor.matmul(rbc_ps[:], ind4[:], recipT[:], start=True, stop=True)
            rbc = stream.tile([128, NI], F32, tag="t512")
            nc.vector.tensor_copy(rbc[:], rbc_ps[:])
            nc.vector.scalar_tensor_tensor(
                attn[:, w, :], rbc[:], 1.0, av_ps[:], AL.mult, AL.mult
            )

        # -------- FF projection + ff_b + residual + LayerNorm, direct out -------
        for it in range(2):
            ps = ps_mm.tile([128, D], F32, tag="mmps")
            for w in range(4):
                nc.tensor.matmul(
                    ps[:],
                    attn[:, w, it * 128 : (it + 1) * 128],
                    ffw_t[:, w, :],
                    start=(w == 0), stop=False,
                )
            nc.tensor.matmul(
                ps[:], ones_row_b[0:1, :], ffb_row[0:1, :], start=False, stop=True
            )
            x_ld = stream.tile([128, D], F32, tag="t512")
            nc.scalar.copy(x_ld[:], xrows_t[:, it, :])
            y = stream.tile([128, D], F32, tag="t512")
            ysum = tiny.tile([128, 1], F32, tag="t1")
            nc.vector.scalar_tensor_tensor(
                y[:], x_ld[:], 1.0, ps[:], AL.mult, AL.add, accum_out=ysum[:],
            )
            negmu = tiny.tile([128, 1], F32, tag="t1")
            nc.vector.tensor_scalar(negmu[:], ysum[:], -1.0 / D, None, AL.mult)
            sq = stream.tile([128, D], F32, tag="t512")
            ssq = tiny.tile([128, 1], F32, tag="t1")
            nc.scalar.activation(
                sq[:], y[:], AF.Square, bias=negmu[:], scale=1.0, accum_out=ssq[:]
            )
            veps = tiny.tile([128, 1], F32, tag="t1")
            nc.vector.tensor_scalar(veps[:], ssq[:], 1.0 / D, LN_EPS, AL.mult, AL.add)
            std = tiny.tile([128, 1], F32, tag="t1")
            nc.scalar.activation(std[:], veps[:], AF.Sqrt)
            rstd = tiny.tile([128, 1], F32, tag="t1")
            nc.vector.reciprocal(rstd[:], std[:])
            if trivial_ln:
                o = stream.tile([128, D], BF16, tag="to")
                nc.vector.tensor_scalar(o[:], y[:], negmu[:], rstd[:], AL.add, AL.mult)
            else:
                z = stream.tile([128, D], F32, tag="t512")
                nc.vector.tensor_scalar(z[:], y[:], negmu[:], rstd[:], AL.add, AL.mult)
                zw = stream.tile([128, D], F32, tag="t512")
                nc.vector.scalar_tensor_tensor(zw[:], lnw_bc[:], 1.0, z[:], AL.mult, AL.mult)
                o = stream.tile([128, D], BF16, tag="to")
                nc.vector.scalar_tensor_tensor(o[:], lnb_bc[:], 1.0, zw[:], AL.mult, AL.add)
            nc.sync.dma_start(out=t[f"d_out{it}"], in_=o[:])


# ---------------------------------------------------------------------------
# Host side: program cache, cached PJRT runner, shard prep
# ---------------------------------------------------------------------------

_PROGRAM_CACHE = {}
_RUNNER_CACHE = {}
from concurrent.futures import ThreadPoolExecutor as _TPE

_PREP_POOL = _TPE(max_workers=8)


def _get_program(trivial_ln):
    key = (bool(trivial_ln),)
    if key not in _PROGRAM_CACHE:
        _PROGRAM_CACHE[key] = build_program(bool(trivial_ln))
    return _PROGRAM_CACHE[key]


def _get_runner(nc):
    """Build (once) a persistent jitted sharded callable for `nc`.

    Mirrors concourse.bass2jax.run_bass_via_pjrt (the axon execution path of
    bass_utils.run_bass_kernel_spmd) but hoists the jax.jit out of the
    per-call path and assembles the global arrays without an extra concat.
    """
    key = id(nc)
    if key in _RUNNER_CACHE:
        return _RUNNER_CACHE[key]

    import jax
    from jax.sharding import Mesh, PartitionSpec
    from jax.experimental.shard_map import shard_map
    from concourse.bass2jax import (_bass_exec_p, install_neuronx_cc_hook,
                                    partition_id_tensor)

    install_neuronx_cc_hook()
    assert nc.dbg_addr is None or not nc.dbg_callbacks

    partition_name = nc.partition_id_tensor.name if nc.partition_id_tensor else None
    in_names, out_names, out_avals = [], [], []
    for alloc in nc.m.functions[0].allocations:
        if not isinstance(alloc, mybir.MemoryLocationSet):
            continue
        name = alloc.memorylocations[0].name
        if alloc.kind == "ExternalInput":
            if name != partition_name:
                in_names.append(name)
        elif alloc.kind == "ExternalOutput":
            out_names.append(name)
            out_avals.append(jax.core.ShapedArray(
                tuple(alloc.tensor_shape), mybir.dt.np(alloc.dtype)))
    n_params = len(in_names)
    n_outs = len(out_avals)
    # No donated zero buffers for the outputs: run_bass_via_pjrt ships them
    # for kernels that leave output elements unwritten, but this kernel fully
    # writes out0/out1, so skipping them saves their upload.
    all_in_names = list(in_names)
    if partition_name is not None:
        all_in_names.append(partition_name)

    def _body(*args):
        operands = list(args)
        if partition_name is not None:
            operands.append(partition_id_tensor())
        outs = _bass_exec_p.bind(
            *operands, out_avals=tuple(out_avals), in_names=tuple(all_in_names),
            out_names=tuple(out_names), lowering_input_output_aliases=(),
            sim_require_finite=True, sim_require_nnan=True, nc=nc)
        return tuple(outs)

    devices = jax.devices()[:N_CORES]
    mesh = Mesh(np.asarray(devices), ("core",))
    in_specs = (PartitionSpec("core"),) * n_params
    out_specs = (PartitionSpec("core"),) * n_outs
    sharded = jax.jit(
        shard_map(_body, mesh=mesh, in_specs=in_specs, out_specs=out_specs,
                  check_rep=False),
        keep_unused=True)

    from concurrent.futures import ThreadPoolExecutor
    fetch_pool = ThreadPoolExecutor(max_workers=max(len(out_names), 1))

    def run(globals_by_name):
        concat_in = [globals_by_name[name] for name in in_names]
        out_arrs = sharded(*concat_in)
        futs = [fetch_pool.submit(np.asarray, o) for o in out_arrs]
        return {
            name: futs[i].result().reshape(N_CORES, *out_avals[i].shape)
            for i, name in enumerate(out_names)
        }

    _RUNNER_CACHE[key] = run
    return run


def _shard_globals(x, pdist, angle, adj, mask, gp, ga, w_bias,
                   att_w, ff_w, ff_b, ln_w, ln_b):
    """Build the concatenated global input arrays (blob + qblob + smalls)."""
    blob = np.empty((N_CORES * BLOB_ROWS, 1024), BF)
    qblob = np.empty((N_CORES * QBLOB_ROWS, 1024), np.uint8)
    smalls = np.zeros((N_CORES * 6 * 512,), np.float32)

    def build_W():
        awT = att_w.T.astype(BF)                   # [D, 1536]
        W = np.empty((1024, 1024), BF)
        W[0:512] = awT[:, 0:1024]
        W[512:768] = awT[:, 1024:1536].reshape(256, 1024)
        W[768:1024] = ff_w.T.astype(BF).reshape(256, 1024)
        return W

    fW = _PREP_POOL.submit(build_W)
    fxT = [_PREP_POOL.submit(lambda bb: x[bb].T.astype(BF), b) for b in range(B)]
    maskf = [np.where(mask[b, 0, 0, :], np.float32(NEG_INF), np.float32(0.0))
             for b in range(B)]
    simple_g = gp == 1.0 and ga == 1.0

    def fill_core(c):
        b, ih = c // 4, c % 4
        i0 = ih * NI
        irows = slice(i0, i0 + NI)
        bl = blob[c * BLOB_ROWS : (c + 1) * BLOB_ROWS]
        qb = qblob[c * QBLOB_ROWS : (c + 1) * QBLOB_ROWS]
        s = smalls[c * 6 * 512 : (c + 1) * 6 * 512]

        if simple_g:
            p0c = adj[b, irows] - pdist[b, irows]
        else:
            p0c = np.float32(ga) * adj[b, irows] - np.float32(gp) * pdist[b, irows]
        for qi, (row0, src) in enumerate(((Q_P0, p0c),
                                          (Q_A0, angle[b, irows, :, 0]),
                                          (Q_A1, angle[b, irows, :, 1]))):
            lo = float(src.min())
            hi = float(src.max())
            sc = (hi - lo) / 255.0 if hi > lo else 1.0
            qb[row0 : row0 + NI] = (src - lo) * (1.0 / sc) + 0.5
            s[2592 + 2 * qi] = lo
            s[2593 + 2 * qi] = sc
        xT_b = fxT[b].result()
        bl[R_XQ : R_XQ + 128] = xT_b[:, irows].reshape(128, 1024)
        bl[R_XP : R_XP + 128] = xT_b[ih * 128 : (ih + 1) * 128]
        bl[R_W : R_W + 128] = fW.result()[c * 128 : (c + 1) * 128]

        s[0:512] = ln_w
        s[512:1024] = ln_b
        s[1024:1536] = ff_b
        s[1536:2560] = maskf[b]
        s[2560 : 2560 + 2 * H] = w_bias[:, 0:2].reshape(-1)

    # numpy cast/copy loops release the GIL; parallelize the per-core fill
    futs = [_PREP_POOL.submit(fill_core, c) for c in range(N_CORES)]
    for f in futs:
        f.result()
    return {"blob": blob, "qblob": qblob, "smalls": smalls}


def _reference_numpy(x, pdist, angle, adj, mask, gamma_p, gamma_adj, w_bias,
                     att_w, ff_w, ff_b, ln_w, ln_b):
    """Exact fallback (used only for non-head-uniform gammas)."""
    f8 = np.float64
    x64 = x.astype(f8)
    qkv = x64 @ att_w.astype(f8).T
    wq, wk, wv = np.split(qkv, 3, axis=-1)
    bsz, n = x.shape[0], x.shape[1]
    wq = wq.reshape(bsz, n, H, DH)
    wk = wk.reshape(bsz, n, H, DH)
    wv = wv.reshape(bsz, n, H, DH)
    score = np.einsum('bihd,bjhd->bhij', wq, wk, optimize=True) / np.sqrt(f8(DH))
    score = score - gamma_p.astype(f8)[None, :, None, None] * pdist.astype(f8)[:, None]
    score = score + np.einsum('bijc,hc->bhij', angle.astype(f8), w_bias.astype(f8),
                              optimize=True)
    score = score + gamma_adj.astype(f8)[None, :, None, None] * adj.astype(f8)[:, None]
    score = np.where(mask, NEG_INF, score)
    score -= score.max(-1, keepdims=True)
    p = np.exp(score)
    p /= p.sum(-1, keepdims=True)
    att = np.einsum('bhij,bjhd->bihd', p, wv, optimize=True).reshape(bsz, n, H * DH)
    y = x64 + att @ ff_w.astype(f8).T + ff_b.astype(f8)
    mu = y.mean(-1, keepdims=True)
    var = np.square(y - mu).mean(-1, keepdims=True)
    out = (y - mu) / np.sqrt(var + LN_EPS) * ln_w.astype(f8) + ln_b.astype(f8)
    return out.astype(np.float32)


def kernel(x, pdist, angle, adj, mask, gamma_p, gamma_adj, w_bias,
           att_w, ff_w, ff_b, ln_w, ln_b, **_unused):
    x = np.asarray(x, dtype=np.float32)
    pdist = np.asarray(pdist, dtype=np.float32)
    angle = np.asarray(angle, dtype=np.float32)
    adj = np.asarray(adj, dtype=np.float32)
    mask = np.asarray(mask)
    gamma_p = np.asarray(gamma_p, dtype=np.float32)
    gamma_adj = np.asarray(gamma_adj, dtype=np.float32)
    w_bias = np.asarray(w_bias, dtype=np.float32)
    att_w = np.asarray(att_w, dtype=np.float32)
    ff_w = np.asarray(ff_w, dtype=np.float32)
    ff_b = np.asarray(ff_b, dtype=np.float32)
    ln_w = np.asarray(ln_w, dtype=np.float32)
    ln_b = np.asarray(ln_b, dtype=np.float32)

    uniform = bool(
        np.all(gamma_p == gamma_p.flat[0]) and np.all(gamma_adj == gamma_adj.flat[0])
    )
    if not uniform:
        return _reference_numpy(x, pdist, angle, adj, mask, gamma_p, gamma_adj,
                                w_bias, att_w, ff_w, ff_b, ln_w, ln_b)
    gp = float(gamma_p.flat[0])
    ga = float(gamma_adj.flat[0])

    trivial_ln = bool(np.all(ln_w == 1.0) and np.all(ln_b == 0.0))
    nc = _get_program(trivial_ln)
    run = _get_runner(nc)
    g = _shard_globals(x, pdist, angle, adj, mask, gp, ga, w_bias,
                       att_w, ff_w, ff_b, ln_w, ln_b)
    res = run(g)  # out0/out1: [8, 128, D] bf16

    out = np.empty((B, N, D), dtype=np.float32)
    for c in range(N_CORES):
        b, ih = c // 4, c % 4
        i0 = ih * NI
        out[b, i0 : i0 + 128, :] = res["out0"][c]
        out[b, i0 + 128 : i0 + 256, :] = res["out1"][c]
    return out


# revision 40
# speedup vs baseline: 3.3393x; 1.2687x over previous
"""Trainium2 Bass kernel for nn_MultiHeadAttention_52862457480066.

Reference computation (B=2, N=1024, D=512, H=16, DH=32):
    qkv = x @ att_w.T ; q,k,v per head
    score = q.k/sqrt(DH) - gamma_p*pdist + angle@w_bias.T + gamma_adj*adj
    score = where(mask, -1e9, score) ; prob = softmax_j(score)
    att = prob @ v ; ff = att @ ff_w.T + ff_b ; y = x + ff ; out = LayerNorm(y)*ln_w+ln_b

Sharding over 8 cores: (batch b in 2) x (query-quarter ih in 4). Each core owns
ALL 16 heads for its 256 query rows, so its FF output rows are complete and no
cross-core reduction of activations is needed.

End-to-end wall time is dominated by host->device transfer over the axon
tunnel (~115-170 MB/s, ~85 ms fixed, ~6 ms per extra jit arg), so the design
minimizes uploaded bytes and arg count (3 args: bf16 blob, u8 bias blob,
f32 smalls):
- x.T slices and the weight shard ship as bf16 (halves bytes against a 2e-2
  rel-err budget); the score-bias tensors ship u8-quantized with per-slice
  scale/offset (their value ranges make the u8 step comparable to bf16's)
  and are dequantized on-device by the DVE.
- pdist and adj only appear as P0 = gamma_adj*adj - gamma_p*pdist when the
  gammas are head-uniform (they are for this module's inputs), so the host
  combines them into ONE tensor. Non-uniform gammas fall back to exact numpy.
- Bias slices are per-core-unique; with the all-heads sharding nothing is
  uploaded twice. x[b].T (needed in full for K/V) is uploaded as per-core
  quarters and AllGathered on-device within each batch's 4-core group; the
  weights are uploaded as 1/8 shards and AllGathered across all 8 cores.
- The donated zero output buffers that run_bass_via_pjrt ships are skipped:
  this kernel fully writes both outputs.
- Bias tensors upload in natural [i,j] layout (contiguous host slices) and
  are transposed to the [j,i] score layout on-device by the PE, which has
  large headroom. x rows for the residual are likewise recovered on-device by
  transposing the uploaded x[b,irows].T slice.
- Scores are computed TRANSPOSED ([j_part, i_free]) so softmax'd probs feed
  the attention*V matmul directly as the moving operand. All score-bias terms
  enter via PE identity matmuls (angle features) or a DVE add fused with the
  PSUM evacuation (P0), so the hot softmax path is one DVE + one ACT pass.
- Softmax denominators come from N=1 matmuls (ones moving operand) giving
  rowsums in [i_part, head_free] layout; normalization is deferred to after
  the AV matmul (divides 16*256 values per core instead of 4.2M).
- PSUM accumulators written by interleaved matmul chains are zero-initialized
  by one full-coverage start=True matmul; everything after runs start=False.
- The jitted PJRT executable is built once and cached; per-call work is host
  slicing/casting, one sharded transfer, execution, and two parallel bf16
  fetches.
"""

import math

import numpy as np

import concourse.bass as bass
import concourse.tile as tile
from concourse import bacc, mybir
from concourse.masks import make_identity

B, N, D, H, DH = 2, 1024, 512, 16, 32
NI = 256             # query rows per core
NJT = N // 128       # key tiles (partition dim j)
NEG_INF = -1e9
LN_EPS = 1e-5
QSCALE = 1.0 / math.sqrt(DH)
F32 = mybir.dt.float32
BF16 = mybir.dt.bfloat16
N_CORES = 8
XG_GROUPS = [[0, 1, 2, 3], [4, 5, 6, 7]]   # x[b].T AllGather within batch
WG_GROUPS = [[0, 1, 2, 3, 4, 5, 6, 7]]     # weight AllGather across all cores

BF = np.dtype(mybir.dt.np(BF16))  # ml_dtypes.bfloat16

# blob row ranges (per core, [384, 1024] bf16)
R_XQ = 0                             # x[b,irows].T packed [512,256]->[128,1024]
R_XP = 128                           # x[b].T rows [ih*128,(ih+1)*128) for AllGather
R_W = 256                            # weight-pack shard W[c*128:(c+1)*128]
BLOB_ROWS = 384
# qblob row ranges (per core, [768, 1024] uint8): per-slice-quantized biases,
# dequantized on-device as q*scale+lo with scale/lo shipped in smalls
Q_P0, Q_A0, Q_A1 = 0, 256, 512
QBLOB_ROWS = 768
U8 = mybir.dt.uint8
# weight pack W [1024, 1024] bf16 (same on all cores before sharding):
#   rows 0:512   att_w.T[:, 0:1024]            (q feats 0:512 | k feats 512:1024)
#   rows 512:768 att_w.T[:, 1024:1536] flat    (v)
#   rows 768:1024 ff_w.T flat
# smalls [6*512] f32: lnw, lnb, ffb, maskb(1024), hcoef w0/w1 interleaved (32)


def build_program(trivial_ln: bool):
    """Build the SPMD bass program (identical on all 8 cores)."""
    nc = bacc.Bacc("TRN2", target_bir_lowering=False, debug=False, num_devices=N_CORES)

    t = {}
    t["h_blob"] = nc.dram_tensor("blob", [BLOB_ROWS, 1024], BF16, kind="ExternalInput")
    t["h_qblob"] = nc.dram_tensor("qblob", [QBLOB_ROWS, 1024], U8, kind="ExternalInput")
    t["h_smalls"] = nc.dram_tensor("smalls", [6 * 512], F32, kind="ExternalInput")
    # collectives may not read IO tensors: bounce the blob slices to internal
    t["d_xp"] = nc.dram_tensor("xp", [128, N], BF16).ap()
    t["d_wp"] = nc.dram_tensor("wp", [128, 1024], BF16).ap()
    t["d_xg"] = nc.dram_tensor("xg", [512, N], BF16).ap()
    t["h_wg"] = nc.dram_tensor("wg", [1024, 1024], BF16, addr_space="Shared")
    t["d_out0"] = nc.dram_tensor("out0", [128, D], BF16, kind="ExternalOutput").ap()
    t["d_out1"] = nc.dram_tensor("out1", [128, D], BF16, kind="ExternalOutput").ap()

    with tile.TileContext(nc) as tc:
        _emit(nc, tc, t, trivial_ln)
    nc.compile()
    return nc


def _emit(nc, tc, t, trivial_ln):
    AL = mybir.AluOpType
    AF = mybir.ActivationFunctionType
    from contextlib import ExitStack

    blob = t["h_blob"]
    sm = t["h_smalls"]
    wg = t["h_wg"]

    def blob_ap(row0, shape3):
        """AP over blob rows: [128, k, cols] with partition-major packing."""
        _, k, cols = shape3
        return bass.AP(tensor=blob, offset=row0 * 1024,
                       ap=[[cols, 128], [128 * cols, k], [1, cols]])

    def wg_ap(off, k, cols):
        return bass.AP(tensor=wg, offset=off,
                       ap=[[cols, 128], [128 * cols, k], [1, cols]])

    ctx = ExitStack()
    with ctx:
        consts = ctx.enter_context(tc.tile_pool(name="consts", bufs=1))
        big = ctx.enter_context(tc.tile_pool(name="big", bufs=1))
        stream = ctx.enter_context(tc.tile_pool(name="stream", bufs=6))
        tiny = ctx.enter_context(tc.tile_pool(name="tiny", bufs=8))
        ppool = ctx.enter_context(tc.tile_pool(name="ppool", bufs=6))
        ps_mm = ctx.enter_context(tc.tile_pool(name="ps_mm", bufs=4, space="PSUM"))
        ps_sc = ps_mm
        ps_av = ctx.enter_context(tc.tile_pool(name="ps_av", bufs=4, space="PSUM"))
        ps_rs = ps_av

        # ---------------- collectives: gather x[b].T and the weight pack ------
        nc.sync.dma_start(
            out=t["d_xp"],
            in_=bass.AP(tensor=blob, offset=R_XP * 1024, ap=[[1024, 128], [1, 1024]]),
        )
        nc.sync.dma_start(
            out=t["d_wp"],
            in_=bass.AP(tensor=blob, offset=R_W * 1024, ap=[[1024, 128], [1, 1024]]),
        )
        nc.gpsimd.collective_compute(
            "AllGather", AL.bypass, replica_groups=XG_GROUPS,
            ins=[t["d_xp"]], outs=[t["d_xg"]],
        )
        nc.gpsimd.collective_compute(
            "AllGather", AL.bypass, replica_groups=WG_GROUPS,
            ins=[t["d_wp"]],
            outs=[bass.AP(tensor=wg, offset=0, ap=[[1024, 1024], [1, 1024]])],
        )

        # ---------------- constants / small tiles ----------------
        identity_f = consts.tile([128, 128], F32)  # f32 transposes (recip path)
        make_identity(nc, identity_f[:])
        identity_b = consts.tile([128, 128], BF16)  # bf16 transposes (loads)
        nc.vector.tensor_copy(identity_b[:], identity_f[:])
        ind4 = consts.tile([4, 128], F32)  # ind4[k, m] = (m//32 == k)
        nc.gpsimd.memset(ind4[:], 1.0)
        nc.gpsimd.affine_select(
            out=ind4[:], in_=ind4[:], compare_op=AL.is_ge, fill=0.0,
            base=0, pattern=[[1, 128]], channel_multiplier=-32,
        )
        nc.gpsimd.affine_select(
            out=ind4[:], in_=ind4[:], compare_op=AL.is_ge, fill=0.0,
            base=31, pattern=[[-1, 128]], channel_multiplier=32,
        )
        ones_col = consts.tile([128, 1], BF16)
        nc.gpsimd.memset(ones_col[:], 1.0)
        ones_row_f = consts.tile([1, 128], F32)
        nc.gpsimd.memset(ones_row_f[:], 1.0)
        ones_row_b = consts.tile([1, 128], BF16)
        nc.vector.tensor_copy(ones_row_b[:], ones_row_f[:])
        zeros_row_b = consts.tile([1, 512], BF16)
        nc.gpsimd.memset(zeros_row_b[:], 0.0)

        maskb = consts.tile([128, NJT], F32)
        nc.gpsimd.dma_start(
            out=maskb[:],
            in_=bass.AP(tensor=sm, offset=3 * 512, ap=[[1, 128], [128, NJT]]),
        )
        hbc = []  # w0, w1 broadcast [128, H]
        for c in range(2):
            bc = consts.tile([128, H], F32, tag=f"hbc{c}")
            nc.gpsimd.dma_start(
                out=bc[:], in_=bass.AP(tensor=sm, offset=5 * 512 + c, ap=[[0, 128], [2, H]])
            )
            hbc.append(bc)

        # per-head scaled identities for the angle-feature PSUM adds
        idw = []  # idw[c][hl] = identity * w_bias[head, c]
        for c, wbc in enumerate(hbc):
            row = []
            for hl in range(H):
                it_ = consts.tile([128, 128], BF16, tag=f"idw{c}_{hl}")
                nc.vector.tensor_scalar(
                    it_[:], identity_b[:], wbc[:, hl : hl + 1], None, AL.mult
                )
                row.append(it_)
            idw.append(row)

        ffb_f = consts.tile([1, D], F32)
        nc.gpsimd.dma_start(
            out=ffb_f[:], in_=bass.AP(tensor=sm, offset=2 * 512, ap=[[0, 1], [1, D]])
        )
        ffb_row = consts.tile([1, D], BF16)
        nc.vector.tensor_copy(ffb_row[:], ffb_f[:])

        lnw_bc = lnb_bc = None
        if not trivial_ln:
            lnw_row = consts.tile([1, D], F32)
            nc.gpsimd.dma_start(
                out=lnw_row[:], in_=bass.AP(tensor=sm, offset=0, ap=[[0, 1], [1, D]])
            )
            lnb_row = consts.tile([1, D], F32)
            nc.gpsimd.dma_start(
                out=lnb_row[:], in_=bass.AP(tensor=sm, offset=512, ap=[[0, 1], [1, D]])
            )
            lnw_bc = consts.tile([128, D], F32)
            lnb_bc = consts.tile([128, D], F32)
            for row, bc in ((lnw_row, lnw_bc), (lnb_row, lnb_bc)):
                ps = ps_mm.tile([128, D], F32, tag="mmps")
                nc.tensor.matmul(ps[:], ones_row_f[0:1, :], row[0:1, :], start=True, stop=True)
                nc.vector.tensor_copy(bc[:], ps[:])

        # ---------------- load big bf16 inputs ----------------
        xq_t = big.tile([128, 4, NI], BF16)      # x[b,irows].T  [d-part, dc, i]
        nc.sync.dma_start(out=xq_t[:], in_=blob_ap(R_XQ, [128, 4, NI]))
        xg_t = big.tile([128, 4, N], BF16)       # gathered x[b].T [d-part, dc, n]
        nc.sync.dma_start(out=xg_t[:], in_=t["d_xg"].rearrange("(c p) n -> p c n", p=128))
        wqk_t = big.tile([128, 4, 1024], BF16)   # att_w.T[:, 0:1024]
        nc.sync.dma_start(out=wqk_t[:], in_=wg_ap(0, 4, 1024))
        wv_t = big.tile([128, 4, 512], BF16)     # att_w.T[:, 1024:1536]
        nc.sync.dma_start(out=wv_t[:], in_=wg_ap(512 * 1024, 4, 512))
        ffw_t = big.tile([128, 4, 512], BF16)    # ff_w.T
        nc.sync.dma_start(out=ffw_t[:], in_=wg_ap(768 * 1024, 4, 512))

        # ---------------- q/k projection (transposed: [feat, n]) ----------------
        qT = big.tile([128, 4, NI], BF16)   # [dh-part(4h), ft, i]
        for ft in range(4):
            ps = ps_mm.tile([128, NI], F32, tag="mmps")
            for dc in range(4):
                nc.tensor.matmul(
                    ps[:], wqk_t[:, dc, ft * 128 : (ft + 1) * 128], xq_t[:, dc, :],
                    start=(dc == 0), stop=(dc == 3),
                )
            nc.vector.tensor_scalar(qT[:, ft, :], ps[:], QSCALE, None, AL.mult)
        kT = big.tile([128, 4, N], BF16)    # [dh-part(4h), ft, n]
        for ft in range(4):
            for nc_i in range(2):
                ps = ps_mm.tile([128, 512], F32, tag="mmps")
                for dc in range(4):
                    nc.tensor.matmul(
                        ps[:], wqk_t[:, dc, 512 + ft * 128 : 512 + (ft + 1) * 128],
                        xg_t[:, dc, nc_i * 512 : nc_i * 512 + 512],
                        start=(dc == 0), stop=(dc == 3),
                    )
                nc.vector.tensor_copy(kT[:, ft, nc_i * 512 : nc_i * 512 + 512], ps[:])

        # ---------------- v projection (natural: [n, feat]) ----------------
        v = big.tile([128, NJT, 512], BF16)  # [j-part, jt, 16h*32]
        for nt in range(NJT):
            ps = ps_mm.tile([128, 512], F32, tag="mmps")
            for dc in range(4):
                nc.tensor.matmul(
                    ps[:], xg_t[:, dc, nt * 128 : (nt + 1) * 128], wv_t[:, dc, :],
                    start=(dc == 0), stop=(dc == 3),
                )
            nc.scalar.copy(v[:, nt, :], ps[:])

        # ---------------- x rows for the residual: transpose xq_t -------------
        xrows_t = big.tile([128, 2, D], BF16)  # [i-part, it, d]
        for it in range(2):
            ps = ps_mm.tile([128, D], F32, tag="mmps")
            for dc in range(4):
                nc.tensor.matmul(
                    ps[:, dc * 128 : (dc + 1) * 128],
                    xq_t[:, dc, it * 128 : (it + 1) * 128],
                    identity_b[:],
                    start=True, stop=True, skip_group_check=True,
                )
            nc.scalar.copy(xrows_t[:, it, :], ps[:])

        # ------- bias features: load u8, dequant, transpose to [j, i] on the PE --
        nat_ctx = ExitStack()
        nat = nat_ctx.enter_context(tc.tile_pool(name="nat", bufs=1))
        qb = t["h_qblob"]
        nats = []
        for qi, row0 in enumerate((Q_P0, Q_A0, Q_A1)):
            qt = nat.tile([128, 2, N], U8, tag=f"q{qi}")
            nc.sync.dma_start(
                out=qt[:],
                in_=bass.AP(tensor=qb, offset=row0 * 1024,
                            ap=[[1024, 128], [128 * 1024, 2], [1, 1024]]),
            )
            lo = tiny.tile([128, 1], F32, tag=f"qlo{qi}")
            nc.gpsimd.dma_start(
                out=lo[:],
                in_=bass.AP(tensor=sm, offset=2592 + 2 * qi, ap=[[0, 128], [1, 1]]),
            )
            sc_ = tiny.tile([128, 1], F32, tag=f"qsc{qi}")
            nc.gpsimd.dma_start(
                out=sc_[:],
                in_=bass.AP(tensor=sm, offset=2593 + 2 * qi, ap=[[0, 128], [1, 1]]),
            )
            natt = nat.tile([128, 2, N], BF16, tag=f"n{qi}")
            nc.vector.tensor_scalar(natt[:], qt[:], sc_[:], lo[:], AL.mult, AL.add)
            nats.append(natt)
        p0nat, a0nat, a1nat = nats

        P0 = big.tile([128, NJT, NI], BF16)
        a0 = big.tile([128, NJT, NI], BF16)
        a1 = big.tile([128, NJT, NI], BF16)
        for natt, dst in ((p0nat, P0), (a0nat, a0), (a1nat, a1)):
            for jt in range(NJT):
                ps = ps_mm.tile([128, NI], F32, tag="mmps")
                for it in range(2):
                    nc.tensor.matmul(
                        ps[:, it * 128 : (it + 1) * 128],
                        natt[:, it, jt * 128 : (jt + 1) * 128],
                        identity_b[:],
                        start=True, stop=True, skip_group_check=True,
                    )
                nc.scalar.copy(dst[:, jt, :], ps[:])
        nat_ctx.close()

        # ---------------- attention: 4 waves of 4 heads ----------------
        attn = big.tile([128, 4, NI], BF16)  # normalized att.T  [4h*32dh, wave, i]
        for w in range(4):
            av_ps = ps_av.tile([128, NI], F32, tag="avps")
            rs_ps = ps_rs.tile([128, 8], F32, tag="avps")
            # zero-init accumulator banks (see module docstring)
            nc.tensor.matmul(
                av_ps[:], ones_row_b[0:1, :], zeros_row_b[0:1, 0:NI],
                start=True, stop=False, skip_group_check=True,
            )
            nc.tensor.matmul(
                rs_ps[:], ones_row_b[0:1, :], zeros_row_b[0:1, 0:8],
                start=True, stop=False, skip_group_check=True,
            )
            for jt in range(NJT):
                p_tiles = []
                for hh in range(4):
                    hl = w * 4 + hh
                    sc = ps_sc.tile([128, NI], F32, tag="mmps")
                    nc.tensor.matmul(
                        sc[:],
                        kT[hh * 32 : (hh + 1) * 32, w, jt * 128 : (jt + 1) * 128],
                        qT[hh * 32 : (hh + 1) * 32, w, :],
                        start=True, stop=False, tile_position=(hh * 32, 0),
                    )
                    nc.tensor.matmul(
                        sc[:], idw[0][hl][:], a0[:, jt, :], start=False, stop=False,
                    )
                    nc.tensor.matmul(
                        sc[:], idw[1][hl][:], a1[:, jt, :], start=False, stop=True,
                    )
                    # P0 add on the DVE, fused with the PSUM evacuation the
                    # exp would otherwise need.
                    xs = stream.tile([128, NI], F32, tag="xs")
                    nc.vector.scalar_tensor_tensor(
                        xs[:], P0[:, jt, :], 1.0, sc[:], AL.mult, AL.add
                    )
                    pT = ppool.tile([128, NI], BF16, tag="pT")
                    nc.scalar.activation(
                        pT[:], xs[:], AF.Exp, bias=maskb[:, jt : jt + 1], scale=1.0
                    )
                    p_tiles.append(pT)
                for hh in range(4):
                    pT = p_tiles[hh]
                    vcol = (w * 4 + hh) * 32
                    nc.tensor.matmul(
                        av_ps[hh * 32 : (hh + 1) * 32, :],
                        v[:, jt, vcol : vcol + 32],
                        pT[:],
                        start=False, stop=(jt == NJT - 1 and hh == 3),
                        tile_position=(0, hh * 32),
                        skip_group_check=True,
                    )
                    for ic in range(2):
                        col = ic * 4 + hh
                        nc.tensor.matmul(
                            rs_ps[:, col : col + 1],
                            pT[:, ic * 128 : (ic + 1) * 128],
                            ones_col[:],
                            start=False,
                            stop=(jt == NJT - 1 and hh == 3 and ic == 1),
                            skip_group_check=True,
                        )
            # normalize: attn = av / rowsum
            rs_sb = stream.tile([128, 8], F32, tag="t512")
            nc.vector.tensor_copy(rs_sb[:], rs_ps[:])
            recip = stream.tile([128, 8], F32, tag="t512")
            nc.vector.reciprocal(recip[:], rs_sb[:])
            recipT = stream.tile([4, NI], F32, tag="t512")
            for ic in range(2):
                trp = ps_mm.tile([4, 128], F32, tag="mmps")
                nc.tensor.transpose(trp[:], recip[:, ic * 4 : (ic + 1) * 4], identity_f[:])
                nc.vector.tensor_copy(recipT[:, ic * 128 : (ic + 1) * 128], trp[:])
            rbc_ps = ps_mm.tile([128, NI], F32, tag="mmps")
            nc.tensor.matmul(rbc_ps[:], ind4[:], recipT[:], start=True, stop=True)
            rbc = stream.tile([128, NI], F32, tag="t512")
            nc.vector.tensor_copy(rbc[:], rbc_ps[:])
            nc.vector.scalar_tensor_tensor(
                attn[:, w, :], rbc[:], 1.0, av_ps[:], AL.mult, AL.mult
            )

        # -------- FF projection + ff_b + residual + LayerNorm, direct out -------
        for it in range(2):
            ps = ps_mm.tile([128, D], F32, tag="mmps")
            for w in range(4):
                nc.tensor.matmul(
                    ps[:],
                    attn[:, w, it * 128 : (it + 1) * 128],
                    ffw_t[:, w, :],
                    start=(w == 0), stop=False,
                )
            nc.tensor.matmul(
                ps[:], ones_row_b[0:1, :], ffb_row[0:1, :], start=False, stop=True
            )
            x_ld = stream.tile([128, D], F32, tag="t512")
            nc.scalar.copy(x_ld[:], xrows_t[:, it, :])
            y = stream.tile([128, D], F32, tag="t512")
            ysum = tiny.tile([128, 1], F32, tag="t1")
            nc.vector.scalar_tensor_tensor(
                y[:], x_ld[:], 1.0, ps[:], AL.mult, AL.add, accum_out=ysum[:],
            )
            negmu = tiny.tile([128, 1], F32, tag="t1")
            nc.vector.tensor_scalar(negmu[:], ysum[:], -1.0 / D, None, AL.mult)
            sq = stream.tile([128, D], F32, tag="t512")
            ssq = tiny.tile([128, 1], F32, tag="t1")
            nc.scalar.activation(
                sq[:], y[:], AF.Square, bias=negmu[:], scale=1.0, accum_out=ssq[:]
            )
            veps = tiny.tile([128, 1], F32, tag="t1")
            nc.vector.tensor_scalar(veps[:], ssq[:], 1.0 / D, LN_EPS, AL.mult, AL.add)
            std = tiny.tile([128, 1], F32, tag="t1")
            nc.scalar.activation(std[:], veps[:], AF.Sqrt)
            rstd = tiny.tile([128, 1], F32, tag="t1")
            nc.vector.reciprocal(rstd[:], std[:])
            if trivial_ln:
                o = stream.tile([128, D], BF16, tag="to")
                nc.vector.tensor_scalar(o[:], y[:], negmu[:], rstd[:], AL.add, AL.mult)
            else:
                z = stream.tile([128, D], F32, tag="t512")
                nc.vector.tensor_scalar(z[:], y[:], negmu[:], rstd[:], AL.add, AL.mult)
                zw = stream.tile([128, D], F32, tag="t512")
                nc.vector.scalar_tensor_tensor(zw[:], lnw_bc[:], 1.0, z[:], AL.mult, AL.mult)
                o = stream.tile([128, D], BF16, tag="to")
                nc.vector.scalar_tensor_tensor(o[:], lnb_bc[:], 1.0, zw[:], AL.mult, AL.add)
            nc.sync.dma_start(out=t[f"d_out{it}"], in_=o[:])


# ---------------------------------------------------------------------------
# Host side: program cache, cached PJRT runner, shard prep
# ---------------------------------------------------------------------------

_PROGRAM_CACHE = {}
_RUNNER_CACHE = {}
from concurrent.futures import ThreadPoolExecutor as _TPE

_PREP_POOL = _TPE(max_workers=8)


def _get_program(trivial_ln):
    key = (bool(trivial_ln),)
    if key not in _PROGRAM_CACHE:
        _PROGRAM_CACHE[key] = build_program(bool(trivial_ln))
    return _PROGRAM_CACHE[key]


def _get_runner(nc):
    """Build (once) a persistent jitted sharded callable for `nc`.

    Mirrors concourse.bass2jax.run_bass_via_pjrt (the axon execution path of
    bass_utils.run_bass_kernel_spmd) but hoists the jax.jit out of the
    per-call path and assembles the global arrays without an extra concat.
    """
    key = id(nc)
    if key in _RUNNER_CACHE:
        return _RUNNER_CACHE[key]

    import jax
    from jax.sharding import Mesh, PartitionSpec
    from jax.experimental.shard_map import shard_map
    from concourse.bass2jax import (_bass_exec_p, install_neuronx_cc_hook,
                                    partition_id_tensor)

    install_neuronx_cc_hook()
    assert nc.dbg_addr is None or not nc.dbg_callbacks

    partition_name = nc.partition_id_tensor.name if nc.partition_id_tensor else None
    in_names, out_names, out_avals = [], [], []
    for alloc in nc.m.functions[0].allocations:
        if not isinstance(alloc, mybir.MemoryLocationSet):
            continue
        name = alloc.memorylocations[0].name
        if alloc.kind == "ExternalInput":
            if name != partition_name:
                in_names.append(name)
        elif alloc.kind == "ExternalOutput":
            out_names.append(name)
            out_avals.append(jax.core.ShapedArray(
                tuple(alloc.tensor_shape), mybir.dt.np(alloc.dtype)))
    n_params = len(in_names)
    n_outs = len(out_avals)
    # No donated zero buffers for the outputs: run_bass_via_pjrt ships them
    # for kernels that leave output elements unwritten, but this kernel fully
    # writes out0/out1, so skipping them saves their upload.
    all_in_names = list(in_names)
    if partition_name is not None:
        all_in_names.append(partition_name)

    def _body(*args):
        operands = list(args)
        if partition_name is not None:
            operands.append(partition_id_tensor())
        outs = _bass_exec_p.bind(
            *operands, out_avals=tuple(out_avals), in_names=tuple(all_in_names),
            out_names=tuple(out_names), lowering_input_output_aliases=(),
            sim_require_finite=True, sim_require_nnan=True, nc=nc)
        return tuple(outs)

    devices = jax.devices()[:N_CORES]
    mesh = Mesh(np.asarray(devices), ("core",))
    in_specs = (PartitionSpec("core"),) * n_params
    out_specs = (PartitionSpec("core"),) * n_outs
    sharded = jax.jit(
        shard_map(_body, mesh=mesh, in_specs=in_specs, out_specs=out_specs,
                  check_rep=False),
        keep_unused=True)

    from concurrent.futures import ThreadPoolExecutor
    fetch_pool = ThreadPoolExecutor(max_workers=max(len(out_names), 1))

    def run(globals_by_name):
        concat_in = [globals_by_name[name] for name in in_names]
        out_arrs = sharded(*concat_in)
        futs = [fetch_pool.submit(np.asarray, o) for o in out_arrs]
        return {
            name: futs[i].result().reshape(N_CORES, *out_avals[i].shape)
            for i, name in enumerate(out_names)
        }

    _RUNNER_CACHE[key] = run
    return run


def _shard_globals(x, pdist, angle, adj, mask, gp, ga, w_bias,
                   att_w, ff_w, ff_b, ln_w, ln_b):
    """Build the concatenated global input arrays (blob + qblob + smalls)."""
    blob = np.empty((N_CORES * BLOB_ROWS, 1024), BF)
    qblob = np.empty((N_CORES * QBLOB_ROWS, 1024), np.uint8)
    smalls = np.zeros((N_CORES * 6 * 512,), np.float32)

    def build_W():
        awT = att_w.T.astype(BF)                   # [D, 1536]
        W = np.empty((1024, 1024), BF)
        W[0:512] = awT[:, 0:1024]
        W[512:768] = awT[:, 1024:1536].reshape(256, 1024)
        W[768:1024] = ff_w.T.astype(BF).reshape(256, 1024)
        return W

    fW = _PREP_POOL.submit(build_W)
    fxT = [_PREP_POOL.submit(lambda bb: x[bb].T.astype(BF), b) for b in range(B)]
    maskf = [np.where(mask[b, 0, 0, :], np.float32(NEG_INF), np.float32(0.0))
             for b in range(B)]
    simple_g = gp == 1.0 and ga == 1.0

    def fill_core(c):
        b, ih = c // 4, c % 4
        i0 = ih * NI
        irows = slice(i0, i0 + NI)
        bl = blob[c * BLOB_ROWS : (c + 1) * BLOB_ROWS]
        qb = qblob[c * QBLOB_ROWS : (c + 1) * QBLOB_ROWS]
        s = smalls[c * 6 * 512 : (c + 1) * 6 * 512]

        if simple_g:
            p0c = adj[b, irows] - pdist[b, irows]
        else:
            p0c = np.float32(ga) * adj[b, irows] - np.float32(gp) * pdist[b, irows]
        for qi, (row0, src) in enumerate(((Q_P0, p0c),
                                          (Q_A0, angle[b, irows, :, 0]),
                                          (Q_A1, angle[b, irows, :, 1]))):
            lo = float(src.min())
            hi = float(src.max())
            sc = (hi - lo) / 255.0 if hi > lo else 1.0
            qb[row0 : row0 + NI] = (src - lo) * (1.0 / sc) + 0.5
            s[2592 + 2 * qi] = lo
            s[2593 + 2 * qi] = sc
        xT_b = fxT[b].result()
        bl[R_XQ : R_XQ + 128] = xT_b[:, irows].reshape(128, 1024)
        bl[R_XP : R_XP + 128] = xT_b[ih * 128 : (ih + 1) * 128]
        bl[R_W : R_W + 128] = fW.result()[c * 128 : (c + 1) * 128]

        s[0:512] = ln_w
        s[512:1024] = ln_b
        s[1024:1536] = ff_b
        s[1536:2560] = maskf[b]
        s[2560 : 2560 + 2 * H] = w_bias[:, 0:2].reshape(-1)

    # numpy cast/copy loops release the GIL; parallelize the per-core fill
    futs = [_PREP_POOL.submit(fill_core, c) for c in range(N_CORES)]
    for f in futs:
        f.result()
    return {"blob": blob, "qblob": qblob, "smalls": smalls}


def _reference_numpy(x, pdist, angle, adj, mask, gamma_p, gamma_adj, w_bias,
                     att_w, ff_w, ff_b, ln_w, ln_b):
    """Exact fallback (used only for non-head-uniform gammas)."""
    f8 = np.float64
    x64 = x.astype(f8)
    qkv = x64 @ att_w.astype(f8).T
    wq, wk, wv = np.split(qkv, 3, axis=-1)
    bsz, n = x.shape[0], x.shape[1]
    wq = wq.reshape(bsz, n, H, DH)
    wk = wk.reshape(bsz, n, H, DH)
    wv = wv.reshape(bsz, n, H, DH)
    score = np.einsum('bihd,bjhd->bhij', wq, wk, optimize=True) / np.sqrt(f8(DH))
    score = score - gamma_p.astype(f8)[None, :, None, None] * pdist.astype(f8)[:, None]
    score = score + np.einsum('bijc,hc->bhij', angle.astype(f8), w_bias.astype(f8),
                              optimize=True)
    score = score + gamma_adj.astype(f8)[None, :, None, None] * adj.astype(f8)[:, None]
    score = np.where(mask, NEG_INF, score)
    score -= score.max(-1, keepdims=True)
    p = np.exp(score)
    p /= p.sum(-1, keepdims=True)
    att = np.einsum('bhij,bjhd->bihd', p, wv, optimize=True).reshape(bsz, n, H * DH)
    y = x64 + att @ ff_w.astype(f8).T + ff_b.astype(f8)
    mu = y.mean(-1, keepdims=True)
    var = np.square(y - mu).mean(-1, keepdims=True)
    out = (y - mu) / np.sqrt(var + LN_EPS) * ln_w.astype(f8) + ln_b.astype(f8)
    return out.astype(np.float32)


def kernel(x, pdist, angle, adj, mask, gamma_p, gamma_adj, w_bias,
           att_w, ff_w, ff_b, ln_w, ln_b, **_unused):
    x = np.asarray(x, dtype=np.float32)
    pdist = np.asarray(pdist, dtype=np.float32)
    angle = np.asarray(angle, dtype=np.float32)
    adj = np.asarray(adj, dtype=np.float32)
    mask = np.asarray(mask)
    gamma_p = np.asarray(gamma_p, dtype=np.float32)
    gamma_adj = np.asarray(gamma_adj, dtype=np.float32)
    w_bias = np.asarray(w_bias, dtype=np.float32)
    att_w = np.asarray(att_w, dtype=np.float32)
    ff_w = np.asarray(ff_w, dtype=np.float32)
    ff_b = np.asarray(ff_b, dtype=np.float32)
    ln_w = np.asarray(ln_w, dtype=np.float32)
    ln_b = np.asarray(ln_b, dtype=np.float32)

    uniform = bool(
        np.all(gamma_p == gamma_p.flat[0]) and np.all(gamma_adj == gamma_adj.flat[0])
    )
    if not uniform:
        return _reference_numpy(x, pdist, angle, adj, mask, gamma_p, gamma_adj,
                                w_bias, att_w, ff_w, ff_b, ln_w, ln_b)
    gp = float(gamma_p.flat[0])
    ga = float(gamma_adj.flat[0])

    trivial_ln = bool(np.all(ln_w == 1.0) and np.all(ln_b == 0.0))
    nc = _get_program(trivial_ln)
    run = _get_runner(nc)
    g = _shard_globals(x, pdist, angle, adj, mask, gp, ga, w_bias,
                       att_w, ff_w, ff_b, ln_w, ln_b)
    res = run(g)  # out0/out1: [8, 128, D] bf16

    out = np.empty((B, N, D), dtype=np.float32)
    for c in range(N_CORES):
        b, ih = c // 4, c % 4
        i0 = ih * NI
        out[b, i0 : i0 + 128, :] = res["out0"][c]
        out[b, i0 + 128 : i0 + 256, :] = res["out1"][c]
    return out


# revision 43
# speedup vs baseline: 3.4437x; 1.0313x over previous
"""Trainium2 Bass kernel for nn_MultiHeadAttention_52862457480066.

Reference computation (B=2, N=1024, D=512, H=16, DH=32):
    qkv = x @ att_w.T ; q,k,v per head
    score = q.k/sqrt(DH) - gamma_p*pdist + angle@w_bias.T + gamma_adj*adj
    score = where(mask, -1e9, score) ; prob = softmax_j(score)
    att = prob @ v ; ff = att @ ff_w.T + ff_b ; y = x + ff ; out = LayerNorm(y)*ln_w+ln_b

Sharding over 8 cores: (batch b in 2) x (query-quarter ih in 4). Each core owns
ALL 16 heads for its 256 query rows, so its FF output rows are complete and no
cross-core reduction of activations is needed.

End-to-end wall time is dominated by host->device transfer over the axon
tunnel (~115-170 MB/s, ~85 ms fixed, ~6 ms per extra jit arg), so the design
minimizes uploaded bytes and arg count (3 args: bf16 blob, u8 bias blob,
f32 smalls):
- x.T slices and the weight shard ship as bf16 (halves bytes against a 2e-2
  rel-err budget); the score-bias tensors ship u8-quantized with per-slice
  scale/offset (their value ranges make the u8 step comparable to bf16's)
  and are dequantized on-device by the DVE.
- pdist and adj only appear as P0 = gamma_adj*adj - gamma_p*pdist when the
  gammas are head-uniform (they are for this module's inputs), so the host
  combines them into ONE tensor. Non-uniform gammas fall back to exact numpy.
- Bias slices are per-core-unique; with the all-heads sharding nothing is
  uploaded twice. x[b].T (needed in full for K/V) is uploaded as per-core
  quarters and AllGathered on-device within each batch's 4-core group; the
  weights are uploaded as 1/8 shards and AllGathered across all 8 cores.
- The donated zero output buffers that run_bass_via_pjrt ships are skipped:
  this kernel fully writes both outputs.
- Bias tensors upload in natural [i,j] layout (contiguous host slices) and
  are transposed to the [j,i] score layout on-device by the PE, which has
  large headroom. x rows for the residual are likewise recovered on-device by
  transposing the uploaded x[b,irows].T slice.
- Scores are computed TRANSPOSED ([j_part, i_free]) so softmax'd probs feed
  the attention*V matmul directly as the moving operand. All score-bias terms
  enter via PE identity matmuls (angle features) or a DVE add fused with the
  PSUM evacuation (P0), so the hot softmax path is one DVE + one ACT pass.
- Softmax denominators come from N=1 matmuls (ones moving operand) giving
  rowsums in [i_part, head_free] layout; normalization is deferred to after
  the AV matmul (divides 16*256 values per core instead of 4.2M).
- PSUM accumulators written by interleaved matmul chains are zero-initialized
  by one full-coverage start=True matmul; everything after runs start=False.
- The jitted PJRT executable is built once and cached; per-call work is host
  slicing/casting, one sharded transfer, execution, and two parallel bf16
  fetches.
"""

import math

import numpy as np

import concourse.bass as bass
import concourse.tile as tile
from concourse import bacc, mybir
from concourse.masks import make_identity

B, N, D, H, DH = 2, 1024, 512, 16, 32
NI = 256             # query rows per core
NJT = N // 128       # key tiles (partition dim j)
NEG_INF = -1e9
LN_EPS = 1e-5
QSCALE = 1.0 / math.sqrt(DH)
F32 = mybir.dt.float32
BF16 = mybir.dt.bfloat16
N_CORES = 8
XG_GROUPS = [[0, 1, 2, 3], [4, 5, 6, 7]]   # x[b].T AllGather within batch
WG_GROUPS = [[0, 1, 2, 3, 4, 5, 6, 7]]     # weight AllGather across all cores

BF = np.dtype(mybir.dt.np(BF16))  # ml_dtypes.bfloat16

# blob row ranges (per core, [384, 1024] bf16)
R_XQ = 0                             # x[b,irows].T packed [512,256]->[128,1024]
R_XP = 128                           # x[b].T rows [ih*128,(ih+1)*128) for AllGather
R_W = 256                            # weight-pack shard W[c*128:(c+1)*128]
BLOB_ROWS = 384
# qblob row ranges (per core, [768, 1024] uint8): per-slice-quantized biases,
# dequantized on-device as q*scale+lo with scale/lo shipped in smalls
Q_P0, Q_A0, Q_A1 = 0, 256, 512
QBLOB_ROWS = 768
U8 = mybir.dt.uint8
# weight pack W [1024, 1024] bf16 (same on all cores before sharding), shipped
# in NATURAL row order (contiguous host casts; the PE transposes on-device):
#   rows 0:768   att_w [1536, 512] flat        (row r = att_w rows 2r, 2r+1)
#   rows 768:1024 ff_w [512, 512] flat
# smalls [6*512] f32: lnw, lnb, ffb, maskb(1024), hcoef w0/w1 interleaved (32)


def build_program(trivial_ln: bool):
    """Build the SPMD bass program (identical on all 8 cores)."""
    nc = bacc.Bacc("TRN2", target_bir_lowering=False, debug=False, num_devices=N_CORES)

    t = {}
    t["h_blob"] = nc.dram_tensor("blob", [BLOB_ROWS, 1024], BF16, kind="ExternalInput")
    t["h_qblob"] = nc.dram_tensor("qblob", [QBLOB_ROWS, 1024], U8, kind="ExternalInput")
    t["h_smalls"] = nc.dram_tensor("smalls", [6 * 512], F32, kind="ExternalInput")
    # collectives may not read IO tensors: bounce the blob slices to internal
    t["d_xp"] = nc.dram_tensor("xp", [128, N], BF16).ap()
    t["d_wp"] = nc.dram_tensor("wp", [128, 1024], BF16).ap()
    t["d_xg"] = nc.dram_tensor("xg", [512, N], BF16).ap()
    t["h_wg"] = nc.dram_tensor("wg", [1024, 1024], BF16, addr_space="Shared")
    t["d_out0"] = nc.dram_tensor("out0", [128, D], BF16, kind="ExternalOutput").ap()
    t["d_out1"] = nc.dram_tensor("out1", [128, D], BF16, kind="ExternalOutput").ap()

    with tile.TileContext(nc) as tc:
        _emit(nc, tc, t, trivial_ln)
    nc.compile()
    return nc


def _emit(nc, tc, t, trivial_ln):
    AL = mybir.AluOpType
    AF = mybir.ActivationFunctionType
    from contextlib import ExitStack

    blob = t["h_blob"]
    sm = t["h_smalls"]
    wg = t["h_wg"]

    def blob_ap(row0, shape3):
        """AP over blob rows: [128, k, cols] with partition-major packing."""
        _, k, cols = shape3
        return bass.AP(tensor=blob, offset=row0 * 1024,
                       ap=[[cols, 128], [128 * cols, k], [1, cols]])

    def wg_ap(off, k, cols):
        return bass.AP(tensor=wg, offset=off,
                       ap=[[cols, 128], [128 * cols, k], [1, cols]])

    ctx = ExitStack()
    with ctx:
        consts = ctx.enter_context(tc.tile_pool(name="consts", bufs=1))
        big = ctx.enter_context(tc.tile_pool(name="big", bufs=1))
        stream = ctx.enter_context(tc.tile_pool(name="stream", bufs=6))
        tiny = ctx.enter_context(tc.tile_pool(name="tiny", bufs=8))
        ppool = ctx.enter_context(tc.tile_pool(name="ppool", bufs=6))
        ps_mm = ctx.enter_context(tc.tile_pool(name="ps_mm", bufs=4, space="PSUM"))
        ps_sc = ps_mm
        ps_av = ctx.enter_context(tc.tile_pool(name="ps_av", bufs=4, space="PSUM"))
        ps_rs = ps_av

        # ---------------- collectives: gather x[b].T and the weight pack ------
        nc.sync.dma_start(
            out=t["d_xp"],
            in_=bass.AP(tensor=blob, offset=R_XP * 1024, ap=[[1024, 128], [1, 1024]]),
        )
        nc.sync.dma_start(
            out=t["d_wp"],
            in_=bass.AP(tensor=blob, offset=R_W * 1024, ap=[[1024, 128], [1, 1024]]),
        )
        nc.gpsimd.collective_compute(
            "AllGather", AL.bypass, replica_groups=XG_GROUPS,
            ins=[t["d_xp"]], outs=[t["d_xg"]],
        )
        nc.gpsimd.collective_compute(
            "AllGather", AL.bypass, replica_groups=WG_GROUPS,
            ins=[t["d_wp"]],
            outs=[bass.AP(tensor=wg, offset=0, ap=[[1024, 1024], [1, 1024]])],
        )

        # ---------------- constants / small tiles ----------------
        identity_f = consts.tile([128, 128], F32)  # f32 transposes (recip path)
        make_identity(nc, identity_f[:])
        identity_b = consts.tile([128, 128], BF16)  # bf16 transposes (loads)
        nc.vector.tensor_copy(identity_b[:], identity_f[:])
        ind4 = consts.tile([4, 128], F32)  # ind4[k, m] = (m//32 == k)
        nc.gpsimd.memset(ind4[:], 1.0)
        nc.gpsimd.affine_select(
            out=ind4[:], in_=ind4[:], compare_op=AL.is_ge, fill=0.0,
            base=0, pattern=[[1, 128]], channel_multiplier=-32,
        )
        nc.gpsimd.affine_select(
            out=ind4[:], in_=ind4[:], compare_op=AL.is_ge, fill=0.0,
            base=31, pattern=[[-1, 128]], channel_multiplier=32,
        )
        ones_col = consts.tile([128, 1], BF16)
        nc.gpsimd.memset(ones_col[:], 1.0)
        ones_row_f = consts.tile([1, 128], F32)
        nc.gpsimd.memset(ones_row_f[:], 1.0)
        ones_row_b = consts.tile([1, 128], BF16)
        nc.vector.tensor_copy(ones_row_b[:], ones_row_f[:])
        zeros_row_b = consts.tile([1, 512], BF16)
        nc.gpsimd.memset(zeros_row_b[:], 0.0)

        maskb = consts.tile([128, NJT], F32)
        nc.gpsimd.dma_start(
            out=maskb[:],
            in_=bass.AP(tensor=sm, offset=3 * 512, ap=[[1, 128], [128, NJT]]),
        )
        hbc = []  # w0, w1 broadcast [128, H]
        for c in range(2):
            bc = consts.tile([128, H], F32, tag=f"hbc{c}")
            nc.gpsimd.dma_start(
                out=bc[:], in_=bass.AP(tensor=sm, offset=5 * 512 + c, ap=[[0, 128], [2, H]])
            )
            hbc.append(bc)

        # per-head scaled identities for the angle-feature PSUM adds
        idw = []  # idw[c][hl] = identity * w_bias[head, c]
        for c, wbc in enumerate(hbc):
            row = []
            for hl in range(H):
                it_ = consts.tile([128, 128], BF16, tag=f"idw{c}_{hl}")
                nc.vector.tensor_scalar(
                    it_[:], identity_b[:], wbc[:, hl : hl + 1], None, AL.mult
                )
                row.append(it_)
            idw.append(row)

        ffb_f = consts.tile([1, D], F32)
        nc.gpsimd.dma_start(
            out=ffb_f[:], in_=bass.AP(tensor=sm, offset=2 * 512, ap=[[0, 1], [1, D]])
        )
        ffb_row = consts.tile([1, D], BF16)
        nc.vector.tensor_copy(ffb_row[:], ffb_f[:])

        lnw_bc = lnb_bc = None
        if not trivial_ln:
            lnw_row = consts.tile([1, D], F32)
            nc.gpsimd.dma_start(
                out=lnw_row[:], in_=bass.AP(tensor=sm, offset=0, ap=[[0, 1], [1, D]])
            )
            lnb_row = consts.tile([1, D], F32)
            nc.gpsimd.dma_start(
                out=lnb_row[:], in_=bass.AP(tensor=sm, offset=512, ap=[[0, 1], [1, D]])
            )
            lnw_bc = consts.tile([128, D], F32)
            lnb_bc = consts.tile([128, D], F32)
            for row, bc in ((lnw_row, lnw_bc), (lnb_row, lnb_bc)):
                ps = ps_mm.tile([128, D], F32, tag="mmps")
                nc.tensor.matmul(ps[:], ones_row_f[0:1, :], row[0:1, :], start=True, stop=True)
                nc.vector.tensor_copy(bc[:], ps[:])

        # ---------------- load big bf16 inputs ----------------
        xq_t = big.tile([128, 4, NI], BF16)      # x[b,irows].T  [d-part, dc, i]
        nc.sync.dma_start(out=xq_t[:], in_=blob_ap(R_XQ, [128, 4, NI]))
        xg_t = big.tile([128, 4, N], BF16)       # gathered x[b].T [d-part, dc, n]
        nc.sync.dma_start(out=xg_t[:], in_=t["d_xg"].rearrange("(c p) n -> p c n", p=128))
        # the pack arrives in natural row order; transpose to [d-part, feat]
        # layouts on the PE (64+16 identity matmuls, ~15us)
        wn_ctx = ExitStack()
        wn = wn_ctx.enter_context(tc.tile_pool(name="wn", bufs=1))
        awn = wn.tile([128, 12, 512], BF16)   # awn[p,t,d] = att_w[t*128+p, d]
        nc.sync.dma_start(out=awn[:], in_=wg_ap(0, 12, 512))
        ffn = wn.tile([128, 4, 512], BF16)    # ffn[p,t,d] = ff_w[t*128+p, d]
        nc.sync.dma_start(out=ffn[:], in_=wg_ap(768 * 1024, 4, 512))

        wqk_t = big.tile([128, 4, 1024], BF16)   # att_w.T[:, 0:1024]
        wv_t = big.tile([128, 4, 512], BF16)     # att_w.T[:, 1024:1536]
        ffw_t = big.tile([128, 4, 512], BF16)    # ff_w.T
        for dc in range(4):
            for half in range(2):
                ps = ps_mm.tile([128, 512], F32, tag="mmps")
                for fb in range(4):
                    nc.tensor.matmul(
                        ps[:, fb * 128 : (fb + 1) * 128],
                        awn[:, half * 4 + fb, dc * 128 : (dc + 1) * 128],
                        identity_b[:],
                        start=True, stop=True, skip_group_check=True,
                    )
                nc.scalar.copy(wqk_t[:, dc, half * 512 : (half + 1) * 512], ps[:])
            ps = ps_mm.tile([128, 512], F32, tag="mmps")
            for fb in range(4):
                nc.tensor.matmul(
                    ps[:, fb * 128 : (fb + 1) * 128],
                    awn[:, 8 + fb, dc * 128 : (dc + 1) * 128],
                    identity_b[:],
                    start=True, stop=True, skip_group_check=True,
                )
            nc.scalar.copy(wv_t[:, dc, :], ps[:])
        for w in range(4):
            ps = ps_mm.tile([128, 512], F32, tag="mmps")
            for t_ in range(4):
                nc.tensor.matmul(
                    ps[:, t_ * 128 : (t_ + 1) * 128],
                    ffn[:, t_, w * 128 : (w + 1) * 128],
                    identity_b[:],
                    start=True, stop=True, skip_group_check=True,
                )
            nc.scalar.copy(ffw_t[:, w, :], ps[:])
        wn_ctx.close()

        # ---------------- q/k projection (transposed: [feat, n]) ----------------
        qT = big.tile([128, 4, NI], BF16)   # [dh-part(4h), ft, i]
        for ft in range(4):
            ps = ps_mm.tile([128, NI], F32, tag="mmps")
            for dc in range(4):
                nc.tensor.matmul(
                    ps[:], wqk_t[:, dc, ft * 128 : (ft + 1) * 128], xq_t[:, dc, :],
                    start=(dc == 0), stop=(dc == 3),
                )
            nc.vector.tensor_scalar(qT[:, ft, :], ps[:], QSCALE, None, AL.mult)
        kT = big.tile([128, 4, N], BF16)    # [dh-part(4h), ft, n]
        for ft in range(4):
            for nc_i in range(2):
                ps = ps_mm.tile([128, 512], F32, tag="mmps")
                for dc in range(4):
                    nc.tensor.matmul(
                        ps[:], wqk_t[:, dc, 512 + ft * 128 : 512 + (ft + 1) * 128],
                        xg_t[:, dc, nc_i * 512 : nc_i * 512 + 512],
                        start=(dc == 0), stop=(dc == 3),
                    )
                nc.vector.tensor_copy(kT[:, ft, nc_i * 512 : nc_i * 512 + 512], ps[:])

        # ---------------- v projection (natural: [n, feat]) ----------------
        v = big.tile([128, NJT, 512], BF16)  # [j-part, jt, 16h*32]
        for nt in range(NJT):
            ps = ps_mm.tile([128, 512], F32, tag="mmps")
            for dc in range(4):
                nc.tensor.matmul(
                    ps[:], xg_t[:, dc, nt * 128 : (nt + 1) * 128], wv_t[:, dc, :],
                    start=(dc == 0), stop=(dc == 3),
                )
            nc.scalar.copy(v[:, nt, :], ps[:])

        # ---------------- x rows for the residual: transpose xq_t -------------
        xrows_t = big.tile([128, 2, D], BF16)  # [i-part, it, d]
        for it in range(2):
            ps = ps_mm.tile([128, D], F32, tag="mmps")
            for dc in range(4):
                nc.tensor.matmul(
                    ps[:, dc * 128 : (dc + 1) * 128],
                    xq_t[:, dc, it * 128 : (it + 1) * 128],
                    identity_b[:],
                    start=True, stop=True, skip_group_check=True,
                )
            nc.scalar.copy(xrows_t[:, it, :], ps[:])

        # ------- bias features: load u8, dequant, transpose to [j, i] on the PE --
        nat_ctx = ExitStack()
        nat = nat_ctx.enter_context(tc.tile_pool(name="nat", bufs=1))
        qb = t["h_qblob"]
        nats = []
        for qi, row0 in enumerate((Q_P0, Q_A0, Q_A1)):
            qt = nat.tile([128, 2, N], U8, tag=f"q{qi}")
            nc.sync.dma_start(
                out=qt[:],
                in_=bass.AP(tensor=qb, offset=row0 * 1024,
                            ap=[[1024, 128], [128 * 1024, 2], [1, 1024]]),
            )
            lo = tiny.tile([128, 1], F32, tag=f"qlo{qi}")
            nc.gpsimd.dma_start(
                out=lo[:],
                in_=bass.AP(tensor=sm, offset=2592 + 2 * qi, ap=[[0, 128], [1, 1]]),
            )
            sc_ = tiny.tile([128, 1], F32, tag=f"qsc{qi}")
            nc.gpsimd.dma_start(
                out=sc_[:],
                in_=bass.AP(tensor=sm, offset=2593 + 2 * qi, ap=[[0, 128], [1, 1]]),
            )
            natt = nat.tile([128, 2, N], BF16, tag=f"n{qi}")
            nc.vector.tensor_scalar(natt[:], qt[:], sc_[:], lo[:], AL.mult, AL.add)
            nats.append(natt)
        p0nat, a0nat, a1nat = nats

        P0 = big.tile([128, NJT, NI], BF16)
        a0 = big.tile([128, NJT, NI], BF16)
        a1 = big.tile([128, NJT, NI], BF16)
        for natt, dst in ((p0nat, P0), (a0nat, a0), (a1nat, a1)):
            for jt in range(NJT):
                ps = ps_mm.tile([128, NI], F32, tag="mmps")
                for it in range(2):
                    nc.tensor.matmul(
                        ps[:, it * 128 : (it + 1) * 128],
                        natt[:, it, jt * 128 : (jt + 1) * 128],
                        identity_b[:],
                        start=True, stop=True, skip_group_check=True,
                    )
                nc.scalar.copy(dst[:, jt, :], ps[:])
        nat_ctx.close()

        # ---------------- attention: 4 waves of 4 heads ----------------
        attn = big.tile([128, 4, NI], BF16)  # normalized att.T  [4h*32dh, wave, i]
        for w in range(4):
            av_ps = ps_av.tile([128, NI], F32, tag="avps")
            rs_ps = ps_rs.tile([128, 8], F32, tag="avps")
            # zero-init accumulator banks (see module docstring)
            nc.tensor.matmul(
                av_ps[:], ones_row_b[0:1, :], zeros_row_b[0:1, 0:NI],
                start=True, stop=False, skip_group_check=True,
            )
            nc.tensor.matmul(
                rs_ps[:], ones_row_b[0:1, :], zeros_row_b[0:1, 0:8],
                start=True, stop=False, skip_group_check=True,
            )
            for jt in range(NJT):
                p_tiles = []
                for hh in range(4):
                    hl = w * 4 + hh
                    sc = ps_sc.tile([128, NI], F32, tag="mmps")
                    nc.tensor.matmul(
                        sc[:],
                        kT[hh * 32 : (hh + 1) * 32, w, jt * 128 : (jt + 1) * 128],
                        qT[hh * 32 : (hh + 1) * 32, w, :],
                        start=True, stop=False, tile_position=(hh * 32, 0),
                    )
                    nc.tensor.matmul(
                        sc[:], idw[0][hl][:], a0[:, jt, :], start=False, stop=False,
                    )
                    nc.tensor.matmul(
                        sc[:], idw[1][hl][:], a1[:, jt, :], start=False, stop=True,
                    )
                    # P0 add on the DVE, fused with the PSUM evacuation the
                    # exp would otherwise need.
                    xs = stream.tile([128, NI], F32, tag="xs")
                    nc.vector.scalar_tensor_tensor(
                        xs[:], P0[:, jt, :], 1.0, sc[:], AL.mult, AL.add
                    )
                    pT = ppool.tile([128, NI], BF16, tag="pT")
                    nc.scalar.activation(
                        pT[:], xs[:], AF.Exp, bias=maskb[:, jt : jt + 1], scale=1.0
                    )
                    p_tiles.append(pT)
                for hh in range(4):
                    pT = p_tiles[hh]
                    vcol = (w * 4 + hh) * 32
                    nc.tensor.matmul(
                        av_ps[hh * 32 : (hh + 1) * 32, :],
                        v[:, jt, vcol : vcol + 32],
                        pT[:],
                        start=False, stop=(jt == NJT - 1 and hh == 3),
                        tile_position=(0, hh * 32),
                        skip_group_check=True,
                    )
                    for ic in range(2):
                        col = ic * 4 + hh
                        nc.tensor.matmul(
                            rs_ps[:, col : col + 1],
                            pT[:, ic * 128 : (ic + 1) * 128],
                            ones_col[:],
                            start=False,
                            stop=(jt == NJT - 1 and hh == 3 and ic == 1),
                            skip_group_check=True,
                        )
            # normalize: attn = av / rowsum
            rs_sb = stream.tile([128, 8], F32, tag="t512")
            nc.vector.tensor_copy(rs_sb[:], rs_ps[:])
            recip = stream.tile([128, 8], F32, tag="t512")
            nc.vector.reciprocal(recip[:], rs_sb[:])
            recipT = stream.tile([4, NI], F32, tag="t512")
            for ic in range(2):
                trp = ps_mm.tile([4, 128], F32, tag="mmps")
                nc.tensor.transpose(trp[:], recip[:, ic * 4 : (ic + 1) * 4], identity_f[:])
                nc.vector.tensor_copy(recipT[:, ic * 128 : (ic + 1) * 128], trp[:])
            rbc_ps = ps_mm.tile([128, NI], F32, tag="mmps")
            nc.tensor.matmul(rbc_ps[:], ind4[:], recipT[:], start=True, stop=True)
            rbc = stream.tile([128, NI], F32, tag="t512")
            nc.vector.tensor_copy(rbc[:], rbc_ps[:])
            nc.vector.scalar_tensor_tensor(
                attn[:, w, :], rbc[:], 1.0, av_ps[:], AL.mult, AL.mult
            )

        # -------- FF projection + ff_b + residual + LayerNorm, direct out -------
        for it in range(2):
            ps = ps_mm.tile([128, D], F32, tag="mmps")
            for w in range(4):
                nc.tensor.matmul(
                    ps[:],
                    attn[:, w, it * 128 : (it + 1) * 128],
                    ffw_t[:, w, :],
                    start=(w == 0), stop=False,
                )
            nc.tensor.matmul(
                ps[:], ones_row_b[0:1, :], ffb_row[0:1, :], start=False, stop=True
            )
            x_ld = stream.tile([128, D], F32, tag="t512")
            nc.scalar.copy(x_ld[:], xrows_t[:, it, :])
            y = stream.tile([128, D], F32, tag="t512")
            ysum = tiny.tile([128, 1], F32, tag="t1")
            nc.vector.scalar_tensor_tensor(
                y[:], x_ld[:], 1.0, ps[:], AL.mult, AL.add, accum_out=ysum[:],
            )
            negmu = tiny.tile([128, 1], F32, tag="t1")
            nc.vector.tensor_scalar(negmu[:], ysum[:], -1.0 / D, None, AL.mult)
            sq = stream.tile([128, D], F32, tag="t512")
            ssq = tiny.tile([128, 1], F32, tag="t1")
            nc.scalar.activation(
                sq[:], y[:], AF.Square, bias=negmu[:], scale=1.0, accum_out=ssq[:]
            )
            veps = tiny.tile([128, 1], F32, tag="t1")
            nc.vector.tensor_scalar(veps[:], ssq[:], 1.0 / D, LN_EPS, AL.mult, AL.add)
            std = tiny.tile([128, 1], F32, tag="t1")
            nc.scalar.activation(std[:], veps[:], AF.Sqrt)
            rstd = tiny.tile([128, 1], F32, tag="t1")
            nc.vector.reciprocal(rstd[:], std[:])
            if trivial_ln:
                o = stream.tile([128, D], BF16, tag="to")
                nc.vector.tensor_scalar(o[:], y[:], negmu[:], rstd[:], AL.add, AL.mult)
            else:
                z = stream.tile([128, D], F32, tag="t512")
                nc.vector.tensor_scalar(z[:], y[:], negmu[:], rstd[:], AL.add, AL.mult)
                zw = stream.tile([128, D], F32, tag="t512")
                nc.vector.scalar_tensor_tensor(zw[:], lnw_bc[:], 1.0, z[:], AL.mult, AL.mult)
                o = stream.tile([128, D], BF16, tag="to")
                nc.vector.scalar_tensor_tensor(o[:], lnb_bc[:], 1.0, zw[:], AL.mult, AL.add)
            nc.sync.dma_start(out=t[f"d_out{it}"], in_=o[:])


# ---------------------------------------------------------------------------
# Host side: program cache, cached PJRT runner, shard prep
# ---------------------------------------------------------------------------

_PROGRAM_CACHE = {}
_RUNNER_CACHE = {}
from concurrent.futures import ThreadPoolExecutor as _TPE

_PREP_POOL = _TPE(max_workers=8)


def _get_program(trivial_ln):
    key = (bool(trivial_ln),)
    if key not in _PROGRAM_CACHE:
        _PROGRAM_CACHE[key] = build_program(bool(trivial_ln))
    return _PROGRAM_CACHE[key]


def _get_runner(nc):
    """Build (once) a persistent jitted sharded callable for `nc`.

    Mirrors concourse.bass2jax.run_bass_via_pjrt (the axon execution path of
    bass_utils.run_bass_kernel_spmd) but hoists the jax.jit out of the
    per-call path and assembles the global arrays without an extra concat.
    """
    key = id(nc)
    if key in _RUNNER_CACHE:
        return _RUNNER_CACHE[key]

    import jax
    from jax.sharding import Mesh, PartitionSpec
    from jax.experimental.shard_map import shard_map
    from concourse.bass2jax import (_bass_exec_p, install_neuronx_cc_hook,
                                    partition_id_tensor)

    install_neuronx_cc_hook()
    assert nc.dbg_addr is None or not nc.dbg_callbacks

    partition_name = nc.partition_id_tensor.name if nc.partition_id_tensor else None
    in_names, out_names, out_avals = [], [], []
    for alloc in nc.m.functions[0].allocations:
        if not isinstance(alloc, mybir.MemoryLocationSet):
            continue
        name = alloc.memorylocations[0].name
        if alloc.kind == "ExternalInput":
            if name != partition_name:
                in_names.append(name)
        elif alloc.kind == "ExternalOutput":
            out_names.append(name)
            out_avals.append(jax.core.ShapedArray(
                tuple(alloc.tensor_shape), mybir.dt.np(alloc.dtype)))
    n_params = len(in_names)
    n_outs = len(out_avals)
    # No donated zero buffers for the outputs: run_bass_via_pjrt ships them
    # for kernels that leave output elements unwritten, but this kernel fully
    # writes out0/out1, so skipping them saves their upload.
    all_in_names = list(in_names)
    if partition_name is not None:
        all_in_names.append(partition_name)

    def _body(*args):
        operands = list(args)
        if partition_name is not None:
            operands.append(partition_id_tensor())
        outs = _bass_exec_p.bind(
            *operands, out_avals=tuple(out_avals), in_names=tuple(all_in_names),
            out_names=tuple(out_names), lowering_input_output_aliases=(),
            sim_require_finite=True, sim_require_nnan=True, nc=nc)
        return tuple(outs)

    devices = jax.devices()[:N_CORES]
    mesh = Mesh(np.asarray(devices), ("core",))
    in_specs = (PartitionSpec("core"),) * n_params
    out_specs = (PartitionSpec("core"),) * n_outs
    sharded = jax.jit(
        shard_map(_body, mesh=mesh, in_specs=in_specs, out_specs=out_specs,
                  check_rep=False),
        keep_unused=True)

    from concurrent.futures import ThreadPoolExecutor
    fetch_pool = ThreadPoolExecutor(max_workers=max(len(out_names), 1))

    def run(globals_by_name):
        concat_in = [globals_by_name[name] for name in in_names]
        out_arrs = sharded(*concat_in)
        futs = [fetch_pool.submit(np.asarray, o) for o in out_arrs]
        return {
            name: futs[i].result().reshape(N_CORES, *out_avals[i].shape)
            for i, name in enumerate(out_names)
        }

    _RUNNER_CACHE[key] = run
    return run


def _shard_globals(x, pdist, angle, adj, mask, gp, ga, w_bias,
                   att_w, ff_w, ff_b, ln_w, ln_b):
    """Build the concatenated global input arrays (blob + qblob + smalls)."""
    blob = np.empty((N_CORES * BLOB_ROWS, 1024), BF)
    qblob = np.empty((N_CORES * QBLOB_ROWS, 1024), np.uint8)
    smalls = np.zeros((N_CORES * 6 * 512,), np.float32)

    def build_W():
        W = np.empty((1024, 1024), BF)
        W[0:768] = att_w.astype(BF).reshape(768, 1024)
        W[768:1024] = ff_w.astype(BF).reshape(256, 1024)
        return W

    fW = _PREP_POOL.submit(build_W)
    fxT = [_PREP_POOL.submit(lambda bb: x[bb].T.astype(BF), b) for b in range(B)]
    maskf = [np.where(mask[b, 0, 0, :], np.float32(NEG_INF), np.float32(0.0))
             for b in range(B)]
    simple_g = gp == 1.0 and ga == 1.0

    def fill_core(c):
        b, ih = c // 4, c % 4
        i0 = ih * NI
        irows = slice(i0, i0 + NI)
        bl = blob[c * BLOB_ROWS : (c + 1) * BLOB_ROWS]
        qb = qblob[c * QBLOB_ROWS : (c + 1) * QBLOB_ROWS]
        s = smalls[c * 6 * 512 : (c + 1) * 6 * 512]

        if simple_g:
            p0c = adj[b, irows] - pdist[b, irows]
        else:
            p0c = np.float32(ga) * adj[b, irows] - np.float32(gp) * pdist[b, irows]
        for qi, (row0, src) in enumerate(((Q_P0, p0c),
                                          (Q_A0, angle[b, irows, :, 0]),
                                          (Q_A1, angle[b, irows, :, 1]))):
            lo = float(src.min())
            hi = float(src.max())
            sc = (hi - lo) / 255.0 if hi > lo else 1.0
            qb[row0 : row0 + NI] = (src - lo) * (1.0 / sc) + 0.5
            s[2592 + 2 * qi] = lo
            s[2593 + 2 * qi] = sc
        xT_b = fxT[b].result()
        bl[R_XQ : R_XQ + 128] = xT_b[:, irows].reshape(128, 1024)
        bl[R_XP : R_XP + 128] = xT_b[ih * 128 : (ih + 1) * 128]
        bl[R_W : R_W + 128] = fW.result()[c * 128 : (c + 1) * 128]

        s[0:512] = ln_w
        s[512:1024] = ln_b
        s[1024:1536] = ff_b
        s[1536:2560] = maskf[b]
        s[2560 : 2560 + 2 * H] = w_bias[:, 0:2].reshape(-1)

    # numpy cast/copy loops release the GIL; parallelize the per-core fill
    futs = [_PREP_POOL.submit(fill_core, c) for c in range(N_CORES)]
    for f in futs:
        f.result()
    return {"blob": blob, "qblob": qblob, "smalls": smalls}


def _reference_numpy(x, pdist, angle, adj, mask, gamma_p, gamma_adj, w_bias,
                     att_w, ff_w, ff_b, ln_w, ln_b):
    """Exact fallback (used only for non-head-uniform gammas)."""
    f8 = np.float64
    x64 = x.astype(f8)
    qkv = x64 @ att_w.astype(f8).T
    wq, wk, wv = np.split(qkv, 3, axis=-1)
    bsz, n = x.shape[0], x.shape[1]
    wq = wq.reshape(bsz, n, H, DH)
    wk = wk.reshape(bsz, n, H, DH)
    wv = wv.reshape(bsz, n, H, DH)
    score = np.einsum('bihd,bjhd->bhij', wq, wk, optimize=True) / np.sqrt(f8(DH))
    score = score - gamma_p.astype(f8)[None, :, None, None] * pdist.astype(f8)[:, None]
    score = score + np.einsum('bijc,hc->bhij', angle.astype(f8), w_bias.astype(f8),
                              optimize=True)
    score = score + gamma_adj.astype(f8)[None, :, None, None] * adj.astype(f8)[:, None]
    score = np.where(mask, NEG_INF, score)
    score -= score.max(-1, keepdims=True)
    p = np.exp(score)
    p /= p.sum(-1, keepdims=True)
    att = np.einsum('bhij,bjhd->bihd', p, wv, optimize=True).reshape(bsz, n, H * DH)
    y = x64 + att @ ff_w.astype(f8).T + ff_b.astype(f8)
    mu = y.mean(-1, keepdims=True)
    var = np.square(y - mu).mean(-1, keepdims=True)
    out = (y - mu) / np.sqrt(var + LN_EPS) * ln_w.astype(f8) + ln_b.astype(f8)
    return out.astype(np.float32)


def kernel(x, pdist, angle, adj, mask, gamma_p, gamma_adj, w_bias,
           att_w, ff_w, ff_b, ln_w, ln_b, **_unused):
    x = np.asarray(x, dtype=np.float32)
    pdist = np.asarray(pdist, dtype=np.float32)
    angle = np.asarray(angle, dtype=np.float32)
    adj = np.asarray(adj, dtype=np.float32)
    mask = np.asarray(mask)
    gamma_p = np.asarray(gamma_p, dtype=np.float32)
    gamma_adj = np.asarray(gamma_adj, dtype=np.float32)
    w_bias = np.asarray(w_bias, dtype=np.float32)
    att_w = np.asarray(att_w, dtype=np.float32)
    ff_w = np.asarray(ff_w, dtype=np.float32)
    ff_b = np.asarray(ff_b, dtype=np.float32)
    ln_w = np.asarray(ln_w, dtype=np.float32)
    ln_b = np.asarray(ln_b, dtype=np.float32)

    uniform = bool(
        np.all(gamma_p == gamma_p.flat[0]) and np.all(gamma_adj == gamma_adj.flat[0])
    )
    if not uniform:
        return _reference_numpy(x, pdist, angle, adj, mask, gamma_p, gamma_adj,
                                w_bias, att_w, ff_w, ff_b, ln_w, ln_b)
    gp = float(gamma_p.flat[0])
    ga = float(gamma_adj.flat[0])

    trivial_ln = bool(np.all(ln_w == 1.0) and np.all(ln_b == 0.0))
    nc = _get_program(trivial_ln)
    run = _get_runner(nc)
    g = _shard_globals(x, pdist, angle, adj, mask, gp, ga, w_bias,
                       att_w, ff_w, ff_b, ln_w, ln_b)
    res = run(g)  # out0/out1: [8, 128, D] bf16

    out = np.empty((B, N, D), dtype=np.float32)
    for c in range(N_CORES):
        b, ih = c // 4, c % 4
        i0 = ih * NI
        out[b, i0 : i0 + 128, :] = res["out0"][c]
        out[b, i0 + 128 : i0 + 256, :] = res["out1"][c]
    return out
